# revision 1
# baseline (speedup 1.0000x reference)
"""Trainium2 Bass kernel for the attention-LSTM decoder (nn_Decoder).

Strategy (8 NeuronCores):
  - Attention batch-sharded: each core owns B/8 = 4 batches (enc_out slice,
    encW2 precompute, tanh energies, softmax, context).
  - LSTM tensor-parallel over the hidden dim: each core owns a 128-wide
    hidden slice -> 512 gate rows per layer; h slices are AllGathered
    (feature-major) each step.
  - Vocab projection tensor-parallel over V: deferred to one big matmul over
    all (step, batch) samples at the end; global logsumexp via one AllReduce.
Dtypes: bf16 storage for big operands, fp32 PSUM accumulation everywhere,
fp32 cell state, fp32r for the embedding table.
"""
import os
import sys

sys.path.insert(0, "/opt/trn_rl_repo")

import numpy as np
import ml_dtypes

import concourse.bass as bass
import concourse.bacc as bacc
import concourse.mybir as mybir
import concourse.tile as tile
from concourse import bass_utils
from concourse.masks import make_identity

BF = ml_dtypes.bfloat16
dt = mybir.dt
AFT = mybir.ActivationFunctionType
ALU = mybir.AluOpType

B, T, H, E, V, S = 32, 512, 1024, 300, 32000, 50
NCORES = 8
BPC = B // NCORES      # 4 batches per core
GS = H // NCORES       # 128-wide hidden slice per core
NG = 4 * GS            # 512 gate rows per core
VPC = V // NCORES      # 4000 vocab rows per core
EP = 384               # padded embedding feature dim (3 k-tiles)
KE = EP // 128         # 3
KH = H // 128          # 8
KT = T // 128          # 4
NSAMP = S * B          # 1600
S_EFF = int(os.environ.get("DECODER_STEPS", str(S)))
SIM1 = os.environ.get("DECODER_SIM", "0") == "1"
ABL = set(os.environ.get("DECODER_ABL", "").split(","))
RG = [list(range(NCORES))]
SHARED = "Local" if SIM1 else "Shared"


def _allgather(nc, in_ap, out_ap):
    if SIM1:
        # single fan-out DMA: same bytes as the 8-way copy emulation, one
        # descriptor-level replicate instead of 8 serialized HWDGE passes
        rows, cols = in_ap.shape[0], in_ap.shape[1]
        if "noag" in ABL:
            nc.sync.dma_start(out_ap[0:rows, :], in_ap)
        else:
            nc.sync.dma_start(
                out_ap.rearrange("(c r) k -> r c k", c=NCORES),
                in_ap.unsqueeze(1).broadcast_to((rows, NCORES, cols)),
            )
    else:
        nc.gpsimd.collective_compute(
            "AllGather", mybir.AluOpType.bypass, replica_groups=RG,
            ins=[in_ap.opt()], outs=[out_ap.opt()],
        )


def _allreduce(nc, in_ap, out_ap):
    if SIM1:
        nc.sync.dma_start(out_ap, in_ap)
    else:
        nc.gpsimd.collective_compute(
            "AllReduce", mybir.AluOpType.add, replica_groups=RG,
            ins=[in_ap.opt()], outs=[out_ap.opt()],
        )

# phase-4 vocab chunking
VCH = [512] * 7 + [416]
VOFF = [sum(VCH[:i]) for i in range(len(VCH))]
MTILES = [(m * 128, min(128, NSAMP - m * 128)) for m in range((NSAMP + 127) // 128)]


def build(nc):
    di = {}

    def inp(name, shape, dtype):
        di[name] = nc.dram_tensor(name, list(shape), dtype, kind="ExternalInput")
        return di[name]

    enc_nat = inp("enc_nat", (BPC, T, H), dt.bfloat16)
    enc_tr = inp("enc_tr", (BPC, H, T), dt.bfloat16)
    w2t = inp("w2t", (H, H), dt.bfloat16)
    w1t = inp("w1t", (H, H), dt.bfloat16)
    attn_bias = inp("attn_bias", (1, H), dt.bfloat16)
    vvec = inp("vvec", (H, 1), dt.bfloat16)
    emb_tab = inp("emb_tab", (V, E), dt.float32)
    qidx = inp("qidx", (NSAMP, 1), dt.int32)
    wih0e = inp("wih0e", (EP, NG), dt.bfloat16)
    wih0c = inp("wih0c", (H, NG), dt.bfloat16)
    whh0 = inp("whh0", (H, NG), dt.bfloat16)
    wih1 = inp("wih1", (H, NG), dt.bfloat16)
    whh1 = inp("whh1", (H, NG), dt.bfloat16)
    bias_g0 = inp("bias_g0", (1, NG), dt.bfloat16)
    bias_g1 = inp("bias_g1", (1, NG), dt.bfloat16)
    sel = inp("sel", (B, BPC), dt.bfloat16)
    h0t_init = inp("h0t_init", (H, B), dt.bfloat16)
    h1t_init = inp("h1t_init", (H, B), dt.bfloat16)
    c0_l0 = inp("c0_l0", (B, GS), dt.float32)
    c0_l1 = inp("c0_l1", (B, GS), dt.float32)
    genw_t = inp("genw_t", (H, VPC), dt.bfloat16)
    genb_v = inp("genb_v", (1, VPC), dt.bfloat16)
    logp = nc.dram_tensor("logp", [NSAMP, VPC], dt.float32, kind="ExternalOutput")

    with tile.TileContext(nc) as tc:
        _body(nc, tc, di, logp)
    return di


def _body(nc, tc, di, logp):
    glob_cm = tc.tile_pool(name="glob", bufs=1)
    glob = glob_cm.__enter__()
    dram_cm = tc.tile_pool(name="dram", bufs=1, space="DRAM")
    dram = dram_cm.__enter__()

    # ---- global constants ----
    id_bf = glob.tile([128, 128], dt.bfloat16, name="id_bf")
    id_f32 = glob.tile([128, 128], dt.float32, name="id_f32")
    make_identity(nc, id_bf[:])
    make_identity(nc, id_f32[:])
    ones_bf = glob.tile([1, 128], dt.bfloat16, name="ones_bf")
    nc.gpsimd.memset(ones_bf[:], 1.0)

    # h1T history lives in shared DRAM: tile s = h1T after step s
    hall_steps = [
        dram.tile([NCORES * GS, B], dt.bfloat16, name=f"hall{s}",
                  addr_space=SHARED)
        for s in range(S)
    ]
    sume_all = glob.tile([128, len(MTILES)], dt.float32, name="sume_all")

    # ---------------- phase 0/1: loop-scoped persistent tensors ----------------
    loopers_cm = tc.tile_pool(name="loopers", bufs=1)
    loopers = loopers_cm.__enter__()

    enc_nat_sb = loopers.tile([128, BPC, KT, H], dt.bfloat16, name="enc_nat_sb")
    for _b in range(BPC):
        nc.sync.dma_start(
            enc_nat_sb[:, _b, :, :],
            di["enc_nat"].ap()[_b].rearrange("(k p) h -> p k h", p=128),
        )
    w1t_sb = loopers.tile([128, KH, H], dt.bfloat16, name="w1t_sb")
    nc.sync.dma_start(w1t_sb[:], di["w1t"].ap().rearrange("(k p) h -> p k h", p=128))
    wih0e_sb = loopers.tile([128, KE, NG], dt.bfloat16, name="wih0e_sb")
    nc.sync.dma_start(wih0e_sb[:], di["wih0e"].ap().rearrange("(k p) g -> p k g", p=128))
    wih0c_sb = loopers.tile([128, KH, NG], dt.bfloat16, name="wih0c_sb")
    nc.sync.dma_start(wih0c_sb[:], di["wih0c"].ap().rearrange("(k p) g -> p k g", p=128))
    whh0_sb = loopers.tile([128, KH, NG], dt.bfloat16, name="whh0_sb")
    nc.sync.dma_start(whh0_sb[:], di["whh0"].ap().rearrange("(k p) g -> p k g", p=128))
    wih1_sb = loopers.tile([128, KH, NG], dt.bfloat16, name="wih1_sb")
    nc.sync.dma_start(wih1_sb[:], di["wih1"].ap().rearrange("(k p) g -> p k g", p=128))
    whh1_sb = loopers.tile([128, KH, NG], dt.bfloat16, name="whh1_sb")
    nc.sync.dma_start(whh1_sb[:], di["whh1"].ap().rearrange("(k p) g -> p k g", p=128))
    vvec_sb = loopers.tile([128, KH, 1], dt.bfloat16, name="vvec_sb")
    nc.sync.dma_start(vvec_sb[:], di["vvec"].ap().rearrange("(k p) o -> p k o", p=128))
    attn_b_sb = loopers.tile([1, H], dt.bfloat16, name="attn_b_sb")
    nc.sync.dma_start(attn_b_sb[:], di["attn_bias"].ap())
    bias_g0_sb = loopers.tile([1, NG], dt.bfloat16, name="bias_g0_sb")
    nc.sync.dma_start(bias_g0_sb[:], di["bias_g0"].ap())
    bias_g1_sb = loopers.tile([1, NG], dt.bfloat16, name="bias_g1_sb")
    nc.sync.dma_start(bias_g1_sb[:], di["bias_g1"].ap())
    sel_sb = loopers.tile([B, BPC], dt.bfloat16, name="sel_sb")
    nc.sync.dma_start(sel_sb[:], di["sel"].ap())
    h0t_pp = [
        loopers.tile([128, KH, B], dt.bfloat16, name=f"h0t_pp{i}") for i in range(2)
    ]
    nc.sync.dma_start(
        h0t_pp[0][:], di["h0t_init"].ap().rearrange("(k p) b -> p k b", p=128)
    )
    h1t_pp = [
        loopers.tile([128, KH, B], dt.bfloat16, name=f"h1t_pp{i}") for i in range(2)
    ]
    nc.sync.dma_start(
        h1t_pp[0][:], di["h1t_init"].ap().rearrange("(k p) b -> p k b", p=128)
    )
    c_l0 = loopers.tile([B, GS], dt.float32, name="c_l0")
    nc.sync.dma_start(c_l0[:], di["c0_l0"].ap())
    c_l1 = loopers.tile([B, GS], dt.float32, name="c_l1")
    nc.sync.dma_start(c_l1[:], di["c0_l1"].ap())

    emb_t = loopers.tile([128, KE, NSAMP], dt.bfloat16, name="emb_t")
    nc.gpsimd.memset(emb_t[:], 0.0)
    encw2 = loopers.tile([128, BPC, KH, T], dt.bfloat16, name="encw2")

    # ---- phase 1a: embedding gather + transpose to feature-major ----
    with tc.tile_pool(name="p1e", bufs=3) as p1e, \
         tc.tile_pool(name="p1eps", bufs=3, space="PSUM") as p1eps:
        for (m0, mr) in MTILES:
            idx = p1e.tile([128, 1], dt.int32, tag="idx")
            nc.sync.dma_start(idx[:mr, :], di["qidx"].ap()[m0:m0 + mr, :])
            gath = p1e.tile([128, E], dt.float32, tag="gath")
            nc.gpsimd.indirect_dma_start(
                out=gath[:mr, :],
                out_offset=None,
                in_=di["emb_tab"].ap(),
                in_offset=bass.IndirectOffsetOnAxis(ap=idx[:mr, 0:1], axis=0),
            )
            for k in range(KE):
                cw = min(128, E - k * 128)
                ps = p1eps.tile([128, 128], dt.float32, tag="ps")
                nc.tensor.transpose(
                    ps[:cw, :mr], gath[:mr, k * 128:k * 128 + cw], id_f32[:mr, :mr]
                )
                nc.vector.tensor_copy(emb_t[:cw, k, m0:m0 + mr], ps[:cw, :mr])

    # ---- phase 1b: encW2[b] = (enc_out[b] @ W2.T).T  (feature-major) ----
    with tc.tile_pool(name="p1w", bufs=1) as p1w, \
         tc.tile_pool(name="p1s", bufs=3) as p1s, \
         tc.tile_pool(name="p1ps", bufs=8, space="PSUM") as p1ps:
        w2t_sb = p1w.tile([128, KH, H], dt.bfloat16, name="w2t_sb")
        nc.sync.dma_start(
            w2t_sb[:], di["w2t"].ap().rearrange("(k p) h -> p k h", p=128)
        )
        for b in range(BPC):
            pss = [p1ps.tile([128, T], dt.float32, tag="p1p", name=f"p1p{_m}") for _m in range(KH)]
            for k in range(KH):
                rhs = p1s.tile([128, T], dt.bfloat16, tag="rhs")
                nc.sync.dma_start(
                    rhs[:], di["enc_tr"].ap()[b, k * 128:(k + 1) * 128, :]
                )
                for m in range(KH):
                    nc.tensor.matmul(
                        pss[m][:],
                        w2t_sb[:, k, m * 128:(m + 1) * 128],
                        rhs[:],
                        start=(k == 0),
                        stop=(k == KH - 1),
                    )
            for m in range(KH):
                nc.vector.tensor_copy(encw2[:, b, m, :], pss[m][:])

    # ---------------- phase 2: the recurrent loop ----------------
    sbw_cm = tc.tile_pool(name="sbw", bufs=2)
    sbw = sbw_cm.__enter__()
    psA_cm = tc.tile_pool(name="psA", bufs=2, space="PSUM")
    psA = psA_cm.__enter__()
    psB_cm = tc.tile_pool(name="psB", bufs=2, space="PSUM")
    psB = psB_cm.__enter__()
    psC_cm = tc.tile_pool(name="psC", bufs=2, space="PSUM")
    psC = psC_cm.__enter__()

    def transpose_to(dst_ap, src_ap, rows, cols, ident):
        """dst[cols,rows] (sbuf) = src[rows,cols].T via PE + copy."""
        ps = psC.tile([128, 128], src_ap.dtype, tag="ps_tr")
        nc.tensor.transpose(ps[:cols, :rows], src_ap, ident[:rows, :rows])
        nc.vector.tensor_copy(dst_ap, ps[:cols, :rows])

    def lstm_gates(gps, c_old, c_new, tag):
        """gate order i|f|g|o (each GS wide). returns h (B, GS) bf16 tile."""
        sif_r = sbw.tile([B, 2 * GS], dt.float32, tag=f"sifr{tag}")
        nc.scalar.activation(sif_r[:], gps[:, 0:2 * GS], AFT.Tanh, scale=0.5)
        sif = sbw.tile([B, 2 * GS], dt.float32, tag=f"sif{tag}")
        nc.vector.tensor_scalar(sif[:], sif_r[:], 0.5, 0.5, ALU.mult, ALU.add)
        tg = sbw.tile([B, GS], dt.float32, tag=f"tg{tag}")
        nc.scalar.activation(tg[:], gps[:, 2 * GS:3 * GS], AFT.Tanh)
        so_r = sbw.tile([B, GS], dt.float32, tag=f"sor{tag}")
        nc.scalar.activation(so_r[:], gps[:, 3 * GS:4 * GS], AFT.Tanh, scale=0.5)
        so = sbw.tile([B, GS], dt.float32, tag=f"so{tag}")
        nc.vector.tensor_scalar(so[:], so_r[:], 0.5, 0.5, ALU.mult, ALU.add)
        t_fc = sbw.tile([B, GS], dt.float32, tag=f"tfc{tag}")
        nc.vector.tensor_tensor(t_fc[:], sif[:, GS:2 * GS], c_old[:], op=ALU.mult)
        t_ig = sbw.tile([B, GS], dt.float32, tag=f"tig{tag}")
        nc.vector.tensor_tensor(t_ig[:], sif[:, 0:GS], tg[:], op=ALU.mult)
        nc.vector.tensor_tensor(c_new[:], t_fc[:], t_ig[:], op=ALU.add)
        tc2 = sbw.tile([B, GS], dt.float32, tag=f"tc2{tag}")
        nc.scalar.activation(tc2[:], c_new[:], AFT.Tanh)
        h = sbw.tile([B, GS], dt.bfloat16, tag=f"h{tag}")
        nc.vector.tensor_tensor(h[:], so[:], tc2[:], op=ALU.mult)
        return h

    for s in range(S_EFF):
        h1t_prev = h1t_pp[s % 2][:]
        h0t_prev = h0t_pp[s % 2]

        # --- hidW for all batches: (B, H) = h1.T(T) @ W1.T + attn_b ---
        ps_hw = psA.tile([B, H], dt.float32, tag="psA")
        for half in range(2):
            hs = slice(half * 512, (half + 1) * 512)
            nc.tensor.matmul(
                ps_hw[:, hs], ones_bf[:, :B], attn_b_sb[:, hs], start=True, stop=False
            )
            for k in range(KH):
                nc.tensor.matmul(
                    ps_hw[:, hs],
                    h1t_prev[:, k, :],
                    w1t_sb[:, k, hs],
                    start=False,
                    stop=(k == KH - 1),
                )
        hw_all = sbw.tile([B, H], dt.bfloat16, tag="hw_all", bufs=1)
        nc.vector.tensor_copy(hw_all[:], ps_hw[:])
        # --- select + transpose in one matmul per h-tile:
        #     hwt[:, k, :] = hw_all[:, k-slice].T @ sel  (128 h x 4 own batches)
        hwt = sbw.tile([128, KH, BPC], dt.float32, tag="hwt")
        for k in range(KH):
            ps_hk = psC.tile([128, 128], dt.float32, tag="ps_tr", name="ps_hk")
            nc.tensor.matmul(
                ps_hk[:, :BPC], hw_all[:, k * 128:(k + 1) * 128], sel_sb[:],
                start=True, stop=True,
            )
            nc.vector.tensor_copy(hwt[:, k, :], ps_hk[:, :BPC])

        # --- attention per local batch ---
        awt = sbw.tile([128, KT, BPC], dt.bfloat16, tag="awt", bufs=3)
        for b in range(BPC):
            ps_sc = psB.tile([1, T], dt.float32, tag="psB")
            for k in range(KH):
                en = sbw.tile([128, T], dt.bfloat16, tag="energy", bufs=4)
                if "notanh" not in ABL:
                    nc.scalar.activation(
                        en[:], encw2[:, b, k, :], AFT.Tanh, bias=hwt[:, k, b:b + 1]
                    )
                nc.tensor.matmul(
                    ps_sc[:], vvec_sb[:, k, :], en[:],
                    start=(k == 0), stop=(k == KH - 1),
                )
            awr = sbw.tile([1, T], dt.float32, tag=f"awr{b}", name=f"awr{b}", bufs=1)
            den = sbw.tile([1, 1], dt.float32, tag=f"den{b}", name=f"den{b}", bufs=1)
            nc.scalar.activation(
                awr[:], ps_sc[:], AFT.Exp, accum_out=den[:, 0:1]
            )
            rec = sbw.tile([1, 1], dt.float32, tag=f"rec{b}", name=f"rec{b}", bufs=1)
            nc.vector.reciprocal(rec[:], den[:])
            awn = sbw.tile([1, T], dt.bfloat16, tag=f"awn{b}", name=f"awn{b}", bufs=1)
            nc.vector.tensor_scalar(awn[:], awr[:], rec[:, 0:1], None, ALU.mult)
            for t in range(KT):
                pst = psC.tile([128, 128], dt.bfloat16, tag="ps_tr")
                nc.tensor.transpose(
                    pst[:, :1], awn[:, t * 128:(t + 1) * 128], id_bf[:1, :1]
                )
                nc.vector.tensor_copy(awt[:, t, b:b + 1], pst[:, :1])

        # --- context rows then transpose to feature-major ---
        ctx_rows = sbw.tile([BPC, H], dt.bfloat16, tag="ctx_rows", bufs=1)
        cxw = sbw.tile([1, BPC * H], dt.bfloat16, tag="cxw", bufs=1)
        for b in ([] if "noctx" in ABL else range(BPC)):
            ps_cx = psA.tile([1, H], dt.float32, tag="psA")
            for half in range(2):
                hs = slice(half * 512, (half + 1) * 512)
                for t in range(KT):
                    nc.tensor.matmul(
                        ps_cx[:, hs],
                        awt[:, t, b:b + 1],
                        enc_nat_sb[:, b, t, hs],
                        start=(t == 0),
                        stop=(t == KT - 1),
                    )
            nc.any.tensor_copy(cxw[:, b * H:(b + 1) * H], ps_cx[:])
        for b in range(BPC):
            nc.sync.dma_start(
                ctx_rows[b:b + 1, :], cxw[:, b * H:(b + 1) * H]
            )
        ctxt = sbw.tile([128, KH, BPC], dt.bfloat16, tag="ctxt")
        for k in range(KH):
            transpose_to(
                ctxt[:, k, :], ctx_rows[:, k * 128:(k + 1) * 128], BPC, 128, id_bf
            )
        bx_in = dram.tile([H, BPC], dt.bfloat16, tag="bx_in", bufs=3)
        nc.sync.dma_start(
            bx_in[:].rearrange("(k p) b -> p k b", p=128), ctxt[:]
        )
        bx_out = dram.tile(
            [NCORES * H, BPC], dt.bfloat16, tag="bx_out", bufs=3, addr_space=SHARED
        )
        _allgather(nc, bx_in[:], bx_out[:])
        xt_ctx = sbw.tile([128, KH, NCORES, BPC], dt.bfloat16, tag="xt_ctx", bufs=3)
        for _k in range(KH):
            nc.sync.dma_start(
                xt_ctx[:, _k, :, :],
                bx_out[:].rearrange("(c k p) b -> k p c b", p=128, c=NCORES)[_k],
            )

        # --- LSTM layer 0 (tensor-parallel gates) ---
        ps_g0 = psB.tile([B, NG], dt.float32, tag="psB")
        nc.tensor.matmul(ps_g0[:], ones_bf[:, :B], bias_g0_sb[:], start=True, stop=False)
        for k in range(KE):
            nc.tensor.matmul(
                ps_g0[:], emb_t[:, k, s * B:(s + 1) * B], wih0e_sb[:, k, :],
                start=False, stop=False,
            )
        for k in range(KH):
            nc.tensor.matmul(
                ps_g0[:], h0t_prev[:, k, :], whh0_sb[:, k, :],
                start=False, stop=False,
            )
        for k in range(KH):
            nc.tensor.matmul(
                ps_g0[:], xt_ctx[:, k, :, :], wih0c_sb[:, k, :],
                start=False, stop=(k == KH - 1),
            )
        c_l0_new = sbw.tile([B, GS], dt.float32, tag="c_l0n", bufs=2)
        h0n = lstm_gates(ps_g0, c_l0, c_l0_new, "l0")
        c_l0 = c_l0_new
        h0ts = sbw.tile([128, B], dt.bfloat16, tag="h0ts")
        transpose_to(h0ts[:], h0n[:], B, 128, id_bf)
        bh0_in = dram.tile([GS, B], dt.bfloat16, tag="bh0_in", bufs=3)
        nc.sync.dma_start(bh0_in[:], h0ts[:])
        bh0_out = dram.tile(
            [NCORES * GS, B], dt.bfloat16, tag="bh0_out", bufs=3, addr_space=SHARED
        )
        _allgather(nc, bh0_in[:], bh0_out[:])
        h0t_new = h0t_pp[(s + 1) % 2]
        nc.sync.dma_start(
            h0t_new[:], bh0_out[:].rearrange("(k p) b -> p k b", p=128)
        )

        # --- LSTM layer 1 ---
        ps_g1 = psB.tile([B, NG], dt.float32, tag="psB")
        nc.tensor.matmul(ps_g1[:], ones_bf[:, :B], bias_g1_sb[:], start=True, stop=False)
        for k in range(KH):
            nc.tensor.matmul(
                ps_g1[:], h1t_prev[:, k, :], whh1_sb[:, k, :],
                start=False, stop=False,
            )
        for k in range(KH):
            nc.tensor.matmul(
                ps_g1[:], h0t_new[:, k, :], wih1_sb[:, k, :],
                start=False, stop=(k == KH - 1),
            )
        c_l1_new = sbw.tile([B, GS], dt.float32, tag="c_l1n", bufs=2)
        h1n = lstm_gates(ps_g1, c_l1, c_l1_new, "l1")
        c_l1 = c_l1_new
        h1ts = sbw.tile([128, B], dt.bfloat16, tag="h1ts")
        transpose_to(h1ts[:], h1n[:], B, 128, id_bf)
        bh1_in = dram.tile([GS, B], dt.bfloat16, tag="bh1_in", bufs=3)
        nc.sync.dma_start(bh1_in[:], h1ts[:])
        bh1_out = hall_steps[s][:]
        _allgather(nc, bh1_in[:], bh1_out)
        nc.sync.dma_start(
            h1t_pp[(s + 1) % 2][:],
            bh1_out.rearrange("(k p) b -> p k b", p=128),
        )

    # close loop pools
    psC_cm.__exit__(None, None, None)
    psB_cm.__exit__(None, None, None)
    psA_cm.__exit__(None, None, None)
    sbw_cm.__exit__(None, None, None)
    loopers_cm.__exit__(None, None, None)

    # ---------------- phase 4: vocab projection + exp-sums ----------------
    p4_cm = tc.tile_pool(name="p4", bufs=3)
    p4 = p4_cm.__enter__()
    p4c_cm = tc.tile_pool(name="p4c", bufs=1)
    p4c = p4c_cm.__enter__()
    with tc.tile_pool(name="p4ps", bufs=4, space="PSUM") as p4ps:
        genb_sb = p4c.tile([1, VPC], dt.bfloat16, name="genb_sb")
        nc.sync.dma_start(genb_sb[:], di["genb_v"].ap())
        hhs = []
        for (m0, mr) in MTILES:
            s0 = m0 // B
            ns = mr // B
            hh = p4.tile([128, KH, 4, B], dt.bfloat16, tag=f"hh{m0}",
                         name=f"hh{m0}", bufs=1)
            for sl in range(ns):
                nc.sync.dma_start(
                    hh[:, :, sl, :],
                    hall_steps[s0 + sl][:].rearrange("(k p) b -> p k b", p=128),
                )
            hhs.append(hh)
        sparts_all = [
            p4.tile([128, len(VCH)], dt.float32, tag=f"sp{m0}",
                    name=f"sp{m0}", bufs=1)
            for (m0, mr) in MTILES
        ]
        logits_sb = [
            p4c.tile([128, VPC], dt.bfloat16, tag=f"lgs{m0}",
                     name=f"lgs{m0}", bufs=1)
            for (m0, mr) in MTILES
        ]
        for n, cw in enumerate(VCH):
            gw = p4.tile([128, KH, 512], dt.bfloat16, tag="gw")
            nc.sync.dma_start(
                gw[:, :, :cw],
                di["genw_t"].ap()[:, VOFF[n]:VOFF[n] + cw].rearrange(
                    "(k p) v -> p k v", p=128
                ),
            )
            for mi, (m0, mr) in enumerate(MTILES):
                ns = mr // B
                hh = hhs[mi]
                ps = p4ps.tile([128, 512], dt.float32, tag="p4p")
                nc.tensor.matmul(
                    ps[:mr, :cw], ones_bf[:, :mr],
                    genb_sb[:, VOFF[n]:VOFF[n] + cw], start=True, stop=False,
                )
                for k in range(KH):
                    nc.tensor.matmul(
                        ps[:mr, :cw],
                        hh[:, k, :ns, :],
                        gw[:, k, :cw],
                        start=False, stop=(k == KH - 1),
                    )
                scr = p4.tile([128, 512], dt.bfloat16, tag="scr")
                nc.scalar.activation(
                    scr[:mr, :cw], ps[:mr, :cw], AFT.Exp,
                    accum_out=sparts_all[mi][:mr, n:n + 1],
                )
                nc.vector.tensor_copy(
                    logits_sb[mi][:mr, VOFF[n]:VOFF[n] + cw], ps[:mr, :cw]
                )
        for mi, (m0, mr) in enumerate(MTILES):
            nc.vector.tensor_reduce(
                sume_all[:mr, mi:mi + 1], sparts_all[mi][:mr, :],
                axis=mybir.AxisListType.X, op=ALU.add,
            )

    # ---------------- phase 5: global logsumexp + subtract ----------------
    nm = len(MTILES)
    blse_in = dram.tile([128, nm], dt.float32, name="blse_in")
    blse_out = dram.tile([128, nm], dt.float32, name="blse_out", addr_space=SHARED)
    nc.sync.dma_start(blse_in[:], sume_all[:])
    _allreduce(nc, blse_in[:], blse_out[:])
    with tc.tile_pool(name="p5", bufs=2) as p5:
        sume_g = p5.tile([128, nm], dt.float32, name="sume_g", bufs=1)
        nc.sync.dma_start(sume_g[:], blse_out[:])
        lse = p5.tile([128, nm], dt.float32, name="lse", bufs=1)
        nc.scalar.activation(lse[:], sume_g[:], AFT.Ln)
        for mi, (m0, mr) in enumerate(MTILES):
            lpo = p5.tile([128, VPC], dt.float32, tag="lpo")
            nc.vector.tensor_scalar(
                lpo[:mr, :], logits_sb[mi][:mr, :], lse[:mr, mi:mi + 1],
                None, ALU.subtract,
            )
            nc.sync.dma_start(logp.ap()[m0:m0 + mr, :], lpo[:mr, :])
    p4c_cm.__exit__(None, None, None)
    p4_cm.__exit__(None, None, None)

    dram_cm.__exit__(None, None, None)
    glob_cm.__exit__(None, None, None)


def _prep_inputs(inputs):
    """Host-side sharding/layout prep. Returns list of per-core input dicts."""
    f32 = np.float32
    enc_out = np.asarray(inputs["enc_out"], f32)
    enc_h = np.asarray(inputs["enc_h"], f32)
    enc_c = np.asarray(inputs["enc_c"], f32)
    emb = np.asarray(inputs["embedding"], f32)
    attn_W = np.asarray(inputs["attn_W"], f32)
    attn_b = np.asarray(inputs["attn_b"], f32)
    vv = np.asarray(inputs["v"], f32)
    Wih0 = np.asarray(inputs["Wih0"], f32)
    Whh0 = np.asarray(inputs["Whh0"], f32)
    bih0 = np.asarray(inputs["bih0"], f32)
    bhh0 = np.asarray(inputs["bhh0"], f32)
    Wih1 = np.asarray(inputs["Wih1"], f32)
    Whh1 = np.asarray(inputs["Whh1"], f32)
    bih1 = np.asarray(inputs["bih1"], f32)
    bhh1 = np.asarray(inputs["bhh1"], f32)
    genW = np.asarray(inputs["genW"], f32)
    genb = np.asarray(inputs["genb"], f32)
    q = np.asarray(inputs["question"]).astype(np.int64)

    W1 = attn_W[:, :H]
    W2 = attn_W[:, H:]
    h0 = np.concatenate([enc_h[0], enc_h[1]], 1)  # (B, H) layer 0
    h1 = np.concatenate([enc_h[2], enc_h[3]], 1)  # layer 1
    c0 = np.concatenate([enc_c[0], enc_c[1]], 1)
    c1 = np.concatenate([enc_c[2], enc_c[3]], 1)
    qflat = q.T.reshape(NSAMP, 1).astype(np.int32)  # (s,b) order

    def bf(x):
        return np.ascontiguousarray(x).astype(BF)

    shared = {
        "w2t": bf(W2.T),
        "w1t": bf(W1.T),
        "attn_bias": bf(attn_b.reshape(1, H)),
        "vvec": bf(vv.reshape(H, 1)),
        "emb_tab": np.ascontiguousarray(emb),
        "qidx": qflat,
        "h0t_init": bf(h0.T),
        "h1t_init": bf(h1.T),
    }
    maps = []
    for c in range(NCORES):
        bs = slice(c * BPC, (c + 1) * BPC)
        rows = np.concatenate(
            [np.arange(g * H + c * GS, g * H + (c + 1) * GS) for g in range(4)]
        )
        wih0_s = Wih0[rows]  # (NG, E+H)
        wih0e = np.zeros((EP, NG), f32)
        wih0e[:E] = wih0_s[:, :E].T
        sel = np.zeros((B, BPC), f32)
        for j in range(BPC):
            sel[c * BPC + j, j] = 1.0
        vrows = slice(c * VPC, (c + 1) * VPC)
        m = dict(shared)
        m.update({
            "enc_nat": bf(enc_out[bs]),
            "enc_tr": bf(enc_out[bs].transpose(0, 2, 1)),
            "wih0e": bf(wih0e),
            "wih0c": bf(wih0_s[:, E:].T),
            "whh0": bf(Whh0[rows].T),
            "wih1": bf(Wih1[rows].T),
            "whh1": bf(Whh1[rows].T),
            "bias_g0": bf((bih0 + bhh0)[rows].reshape(1, NG)),
            "bias_g1": bf((bih1 + bhh1)[rows].reshape(1, NG)),
            "sel": bf(sel),
            "c0_l0": np.ascontiguousarray(c0[:, c * GS:(c + 1) * GS]),
            "c0_l1": np.ascontiguousarray(c1[:, c * GS:(c + 1) * GS]),
            "genw_t": bf(genW[vrows].T),
            "genb_v": bf(genb[vrows].reshape(1, VPC)),
        })
        maps.append(m)
    return maps


_CACHED = {}


def _get_compiled():
    if "nc" not in _CACHED:
        nc = bacc.Bacc(
            "TRN2", target_bir_lowering=False, debug=False,
            num_devices=1 if SIM1 else NCORES,
        )
        build(nc)
        nc.compile()
        _CACHED["nc"] = nc
    return _CACHED["nc"]


def run_cores(in_maps, **kw):
    nc = _get_compiled()
    return bass_utils.run_bass_kernel_spmd(nc, in_maps, list(range(NCORES)), **kw)


def kernel(**inputs):
    in_maps = _prep_inputs(inputs)
    res = run_cores(in_maps)
    parts = [res.results[c]["logp"] for c in range(NCORES)]
    full = np.concatenate(parts, axis=1)  # (NSAMP, V)
    out = full.reshape(S, B, V).transpose(1, 0, 2)
    return np.ascontiguousarray(out.astype(np.float32))



# revision 34
# speedup vs baseline: 1.9921x; 1.9921x over previous
"""Trainium2 Bass kernel for the attention-LSTM decoder (nn_Decoder).

Strategy (8 NeuronCores), v2 — restructured for the TRN2 cost model
(matmul cost ~ output free size; Act/DVE cost ~ free size; DVE 4x for
bf16 SBUF tensor_scalar):
  - Attention batch-sharded: each core owns B/8 = 4 batches. Energies are
    computed feature-major: DVE adds the per-step hidden bias (4x mode),
    Act does tanh in 2 big instructions per batch. Scores/softmax are
    transpose-free (ones-matmul partition reductions, unnormalized exp
    weights with context post-scaling).
  - LSTM tensor-parallel over gate rows (512/core, gate order i|f|o|g),
    everything feature-major so gate matmuls have N=16 and the cell state
    lives as (128, B) tiles. Batches advance in 2 waves of 16 columns to
    pipeline the 3 per-wave exchanges under the Act-bound tanh.
  - Vocab projection tensor-parallel over V (4000/core) in fp8 with
    DoubleRow (2 k-tiles per matmul, 0.5 cyc/row), interleaved into the
    recurrent loop per 128-sample mtile; per-mtile exp-sums, logsumexp
    AllReduce, subtract, and f32 output DMA all stream during the loop.
Dtypes: bf16 compute everywhere, fp32 PSUM + cell state, fp8e4m3 for the
ctx encoder operand and the vocab projection (genW and the h1 history).
"""
import os
import sys

sys.path.insert(0, "/opt/trn_rl_repo")

import numpy as np
import ml_dtypes

import concourse.bass as bass
import concourse.bacc as bacc
import concourse.mybir as mybir
import concourse.tile as tile
from concourse import bass_utils
from concourse.masks import make_identity

BF = ml_dtypes.bfloat16
F8 = ml_dtypes.float8_e4m3
dt = mybir.dt
AFT = mybir.ActivationFunctionType
ALU = mybir.AluOpType
PM = mybir.MatmulPerfMode

B, T, H, E, V, S = 32, 512, 1024, 300, 32000, 50
NCORES = 8
BPC = B // NCORES      # 4 batches per core
GS = H // NCORES       # 128-wide hidden slice per core
NG = 4 * GS            # 512 gate rows per core (i|f|o|g blocks of 128)
VPC = V // NCORES      # 4000 vocab rows per core
EP = 384               # padded embedding feature dim (3 k-tiles)
KE = EP // 128         # 3
KH = H // 128          # 8
KT = T // 128          # 4
KP = KH // 2           # 4 k-pairs for fp8 DoubleRow
NSAMP = S * B          # 1600
NW = 2                 # batch waves per step
WB = B // NW           # 16 step-columns per wave
S_EFF = int(os.environ.get("DECODER_STEPS", str(S)))
SIM1 = os.environ.get("DECODER_SIM", "0") == "1"
RG = [list(range(NCORES))]
SHARED = "Local" if SIM1 else "Shared"

# phase-4 sample tiles: 12 x 128 + 1 x 64
MTILES = [(m * 128, min(128, NSAMP - m * 128)) for m in range((NSAMP + 127) // 128)]
VC_N, VC_W = 16, 250     # vocab chunks for the projection psum
OC_N, OC_W = 8, 500      # output chunks for subtract + DMA


def _allgather(nc, eng, src_sbuf_ap, stage_tile, out_ap):
    """AllGather src (sbuf, (128, n)) into out (dram, (8*128, n)).
    SIM1: one fan-out DMA straight from SBUF (cost proxy for the real
    store+collective). Real: stage to dram, then collective.
    `eng` picks the DMA issue queue (SP / Pool / DVE)."""
    if SIM1:
        rows, cols = src_sbuf_ap.shape[0], src_sbuf_ap.shape[1]
        eng.dma_start(
            out_ap.rearrange("(c r) k -> r c k", c=NCORES),
            src_sbuf_ap.unsqueeze(1).broadcast_to((rows, NCORES, cols)),
        )
    else:
        eng.dma_start(stage_tile[:], src_sbuf_ap)
        nc.gpsimd.collective_compute(
            "AllGather", mybir.AluOpType.bypass, replica_groups=RG,
            ins=[stage_tile[:].opt()], outs=[out_ap.opt()],
        )


def _allreduce(nc, in_ap, out_ap):
    if SIM1:
        nc.gpsimd.dma_start(out_ap, in_ap)
    else:
        nc.gpsimd.collective_compute(
            "AllReduce", mybir.AluOpType.add, replica_groups=RG,
            ins=[in_ap.opt()], outs=[out_ap.opt()],
        )


def build(nc):
    di = {}

    def inp(name, shape, dtype):
        di[name] = nc.dram_tensor(name, list(shape), dtype, kind="ExternalInput")
        return di[name]

    inp("enc_tr", (BPC, H, T), dt.bfloat16)       # feature-major enc (p1b rhs)
    inp("enc_f8", (BPC, T, H), dt.float8e4)       # time-major enc (ctx lhsT)
    inp("w2t", (H, H), dt.bfloat16)
    inp("w1t", (H, H), dt.bfloat16)
    inp("attn_bias", (1, H), dt.bfloat16)
    inp("vvec", (H, 1), dt.bfloat16)
    inp("emb_tab", (V, E), dt.float32)
    inp("qidx", (NSAMP, 1), dt.int32)
    inp("wih0e", (EP, NG), dt.bfloat16)
    inp("bias_g0c", (GS, 4), dt.float32)
    inp("wih0c", (H, NG), dt.bfloat16)
    inp("whh0", (H, NG), dt.bfloat16)
    inp("wih1", (H, NG), dt.bfloat16)
    inp("whh1", (H, NG), dt.bfloat16)
    inp("bias_g1", (1, NG), dt.bfloat16)
    inp("sel_own", (WB, 2), dt.bfloat16)
    inp("h0t_init", (H, B), dt.bfloat16)
    inp("h1t_init", (H, B), dt.bfloat16)
    inp("c0_l0", (GS, B), dt.float32)
    inp("c0_l1", (GS, B), dt.float32)
    inp("genw_kp", (128, KP, 2, VPC), dt.float8e4)
    inp("genb_v", (1, VPC), dt.bfloat16)
    logp = nc.dram_tensor("logp", [NSAMP, VPC], dt.float32, kind="ExternalOutput")

    with tile.TileContext(nc) as tc:
        _body(nc, tc, di, logp)
    return di


def _body(nc, tc, di, logp):
    glob_cm = tc.tile_pool(name="glob", bufs=1)
    glob = glob_cm.__enter__()
    dram_cm = tc.tile_pool(name="dram", bufs=1, space="DRAM")
    dram = dram_cm.__enter__()

    # ---- global constants ----
    id_bf = glob.tile([128, 128], dt.bfloat16, name="id_bf")
    id_f32 = glob.tile([128, 128], dt.float32, name="id_f32")
    make_identity(nc, id_bf[:])
    make_identity(nc, id_f32[:])
    ones_bf = glob.tile([1, 512], dt.bfloat16, name="ones_bf")
    nc.gpsimd.memset(ones_bf[:], 1.0)
    ones_col = glob.tile([128, 1], dt.bfloat16, name="ones_col")
    nc.gpsimd.memset(ones_col[:], 1.0)
    ones_f32 = glob.tile([1, 128], dt.float32, name="ones_f32")
    nc.gpsimd.memset(ones_f32[:], 1.0)
    sume = glob.tile([128, len(MTILES)], dt.float32, name="sume")
    sume8 = glob.tile([128, OC_N], dt.float32, name="sume8")

    # ---------------- persistent loop tensors ----------------
    loopers_cm = tc.tile_pool(name="loopers", bufs=1)
    loopers = loopers_cm.__enter__()

    w1t_sb = loopers.tile([128, KH, H], dt.bfloat16, name="w1t_sb")
    nc.sync.dma_start(w1t_sb[:], di["w1t"].ap().rearrange("(k p) h -> p k h", p=128))
    vvec_sb = loopers.tile([128, KH, 1], dt.bfloat16, name="vvec_sb")
    nc.sync.dma_start(vvec_sb[:], di["vvec"].ap().rearrange("(k p) o -> p k o", p=128))
    attn_b_sb = loopers.tile([1, H], dt.bfloat16, name="attn_b_sb")
    nc.sync.dma_start(attn_b_sb[:], di["attn_bias"].ap())
    wih0c_sb = loopers.tile([128, KH, NG], dt.bfloat16, name="wih0c_sb")
    nc.sync.dma_start(wih0c_sb[:], di["wih0c"].ap().rearrange("(k p) g -> p k g", p=128))
    whh0_sb = loopers.tile([128, KH, NG], dt.bfloat16, name="whh0_sb")
    nc.sync.dma_start(whh0_sb[:], di["whh0"].ap().rearrange("(k p) g -> p k g", p=128))
    wih1_sb = loopers.tile([128, KH, NG], dt.bfloat16, name="wih1_sb")
    nc.sync.dma_start(wih1_sb[:], di["wih1"].ap().rearrange("(k p) g -> p k g", p=128))
    whh1_sb = loopers.tile([128, KH, NG], dt.bfloat16, name="whh1_sb")
    nc.sync.dma_start(whh1_sb[:], di["whh1"].ap().rearrange("(k p) g -> p k g", p=128))
    bias_g1_sb = loopers.tile([1, NG], dt.bfloat16, name="bias_g1_sb")
    nc.sync.dma_start(bias_g1_sb[:], di["bias_g1"].ap())
    enc_f8_sb = loopers.tile([128, BPC, KT, H], dt.float8e4, name="enc_f8_sb")
    for _b in range(BPC):
        nc.sync.dma_start(
            enc_f8_sb[:, _b, :, :],
            di["enc_f8"].ap()[_b].rearrange("(k p) h -> p k h", p=128),
        )
    # hidden state ping-pong, factored (p, k, w, c, i)
    h0t_pp = [
        loopers.tile([128, KH, NW, NCORES, 2], dt.bfloat16, name=f"h0t_pp{i}")
        for i in range(2)
    ]
    h1t_pp = [
        loopers.tile([128, KH, NW, NCORES, 2], dt.bfloat16, name=f"h1t_pp{i}")
        for i in range(2)
    ]
    nc.sync.dma_start(
        h0t_pp[0][:],
        di["h0t_init"].ap().rearrange("(k p) (w c i) -> p k w c i", p=128, w=NW, c=NCORES),
    )
    nc.sync.dma_start(
        h1t_pp[0][:],
        di["h1t_init"].ap().rearrange("(k p) (w c i) -> p k w c i", p=128, w=NW, c=NCORES),
    )
    c_l0 = loopers.tile([128, B], dt.float32, name="c_l0")
    nc.sync.dma_start(c_l0[:], di["c0_l0"].ap())
    c_l1 = loopers.tile([128, B], dt.float32, name="c_l1")
    nc.sync.dma_start(c_l1[:], di["c0_l1"].ap())
    genw_sb = loopers.tile([128, KP, 2, VPC], dt.float8e4, name="genw_sb")
    nc.sync.dma_start(genw_sb[:], di["genw_kp"].ap())
    genb_sb = loopers.tile([1, VPC], dt.bfloat16, name="genb_sb")
    nc.sync.dma_start(genb_sb[:], di["genb_v"].ap())
    bias_g0c_sb = loopers.tile([128, 4], dt.float32, name="bias_g0c_sb")
    nc.sync.dma_start(bias_g0c_sb[:], di["bias_g0c"].ap())
    sel_own_sb = loopers.tile([WB, 2], dt.bfloat16, name="sel_own_sb")
    nc.sync.dma_start(sel_own_sb[:], di["sel_own"].ap())

    encw2 = loopers.tile([128, BPC, KH, T], dt.bfloat16, name="encw2")
    g_emb = loopers.tile([128, 4, NSAMP], dt.bfloat16, name="g_emb")
    hcat = loopers.tile([128, KP, 2, NSAMP], dt.float8e4, name="hcat")

    # ---- phase 1: embedding gather/transpose, encW2, emb-gate precompute ----
    with tc.tile_pool(name="p1emb", bufs=1) as p1emb:
        emb_t = p1emb.tile([128, KE, NSAMP], dt.bfloat16, name="emb_t")
        nc.gpsimd.memset(emb_t[:], 0.0)
        wih0e_sb = p1emb.tile([128, KE, NG], dt.bfloat16, name="wih0e_sb")
        nc.sync.dma_start(
            wih0e_sb[:], di["wih0e"].ap().rearrange("(k p) g -> p k g", p=128)
        )

        # 1a: gather + transpose to feature-major
        with tc.tile_pool(name="p1e", bufs=3) as p1e, \
             tc.tile_pool(name="p1eps", bufs=3, space="PSUM") as p1eps:
            for (m0, mr) in MTILES:
                idx = p1e.tile([128, 1], dt.int32, tag="idx")
                nc.sync.dma_start(idx[:mr, :], di["qidx"].ap()[m0:m0 + mr, :])
                gath = p1e.tile([128, E], dt.float32, tag="gath")
                nc.gpsimd.indirect_dma_start(
                    out=gath[:mr, :],
                    out_offset=None,
                    in_=di["emb_tab"].ap(),
                    in_offset=bass.IndirectOffsetOnAxis(ap=idx[:mr, 0:1], axis=0),
                )
                for k in range(KE):
                    cw = min(128, E - k * 128)
                    ps = p1eps.tile([128, 128], dt.float32, tag="ps")
                    nc.tensor.transpose(
                        ps[:cw, :mr], gath[:mr, k * 128:k * 128 + cw],
                        id_f32[:mr, :mr]
                    )
                    nc.vector.tensor_copy(emb_t[:cw, k, m0:m0 + mr], ps[:cw, :mr])

        # 1b: encW2[b] feature-major = W2 @ enc[b].T
        with tc.tile_pool(name="p1w", bufs=1) as p1w, \
             tc.tile_pool(name="p1s", bufs=3) as p1s, \
             tc.tile_pool(name="p1ps", bufs=1, space="PSUM") as p1ps:
            w2t_sb = p1w.tile([128, KH, H], dt.bfloat16, name="w2t_sb")
            nc.sync.dma_start(
                w2t_sb[:], di["w2t"].ap().rearrange("(k p) h -> p k h", p=128)
            )
            for b in range(BPC):
                pss = [
                    p1ps.tile([128, T], dt.float32, tag=f"p1p{m}", name=f"p1p{b}_{m}")
                    for m in range(KH)
                ]
                for k in range(KH):
                    rhs = p1s.tile([128, T], dt.bfloat16, tag="rhs")
                    nc.sync.dma_start(
                        rhs[:], di["enc_tr"].ap()[b, k * 128:(k + 1) * 128, :]
                    )
                    for m in range(KH):
                        nc.tensor.matmul(
                            pss[m][:],
                            w2t_sb[:, k, m * 128:(m + 1) * 128],
                            rhs[:],
                            start=(k == 0),
                            stop=(k == KH - 1),
                        )
                for m in range(KH):
                    if m % 2 == 0:
                        nc.vector.tensor_copy(encw2[:, b, m, :], pss[m][:])
                    else:
                        nc.scalar.activation(encw2[:, b, m, :], pss[m][:], AFT.Copy)

        # 1c: embedding gate contributions (bias folded on the copy)
        with tc.tile_pool(name="p1gps", bufs=3, space="PSUM") as p1gps:
            for gt in range(4):
                for ch in range(4):
                    c0 = ch * 400
                    ps = p1gps.tile([128, 400], dt.float32, tag="gps")
                    for ke in range(KE):
                        nc.tensor.matmul(
                            ps[:],
                            wih0e_sb[:, ke, gt * 128:(gt + 1) * 128],
                            emb_t[:, ke, c0:c0 + 400],
                            start=(ke == 0),
                            stop=(ke == KE - 1),
                        )
                    nc.vector.tensor_scalar(
                        g_emb[:, gt, c0:c0 + 400], ps[:],
                        bias_g0c_sb[:, gt:gt + 1], None, ALU.add,
                    )

    # ---------------- phase 2: the recurrent loop ----------------
    sbw_cm = tc.tile_pool(name="sbw", bufs=2)
    sbw = sbw_cm.__enter__()
    psL_cm = tc.tile_pool(name="psL", bufs=1, space="PSUM")
    psL = psL_cm.__enter__()
    p4ps_cm = tc.tile_pool(name="p4ps", bufs=1, space="PSUM")
    p4ps = p4ps_cm.__enter__()
    p4c_cm = tc.tile_pool(name="p4c", bufs=1)
    p4c = p4c_cm.__enter__()

    def cell(gps, c_ap, tag):
        """gates i|f|o|g (each (128, WB) psum slice). Updates c_ap in place,
        returns h (128, WB) bf16. Elementwise runs on GpSimd (SBUF-only ops)
        to keep the DVE queue free for the energy bias-adds."""
        ifo_r = sbw.tile([128, 3, WB], dt.float32, tag=f"ifor{tag}")
        nc.scalar.activation(ifo_r[:], gps[:, 0:3, :], AFT.Tanh, scale=0.5)
        tg = sbw.tile([128, WB], dt.float32, tag=f"tg{tag}")
        nc.scalar.activation(tg[:], gps[:, 3, :], AFT.Tanh)
        ifo = sbw.tile([128, 3, WB], dt.float32, tag=f"ifo{tag}")
        nc.gpsimd.tensor_scalar(ifo[:], ifo_r[:], 0.5, 0.5, ALU.mult, ALU.add)
        t_fc = sbw.tile([128, WB], dt.float32, tag=f"tfc{tag}")
        nc.gpsimd.tensor_tensor(t_fc[:], ifo[:, 1, :], c_ap, op=ALU.mult)
        t_ig = sbw.tile([128, WB], dt.float32, tag=f"tig{tag}")
        nc.gpsimd.tensor_tensor(t_ig[:], ifo[:, 0, :], tg[:], op=ALU.mult)
        nc.gpsimd.tensor_tensor(c_ap, t_fc[:], t_ig[:], op=ALU.add)
        tc2 = sbw.tile([128, WB], dt.float32, tag=f"tc2{tag}")
        nc.scalar.activation(tc2[:], c_ap, AFT.Tanh)
        h = sbw.tile([128, WB], dt.bfloat16, tag=f"h{tag}")
        nc.gpsimd.tensor_tensor(h[:], ifo[:, 2, :], tc2[:], op=ALU.mult)
        return h

    def p4_burst(m, m0, mr):
        lg = p4c.tile([128, VPC], dt.bfloat16, tag="lgits", bufs=1)
        for vc in range(VC_N):
            v0 = vc * VC_W
            ps = p4ps.tile([128, 256], dt.float32, tag="p4p")
            nc.tensor.matmul(
                ps[:mr, :VC_W], ones_bf[0:1, :mr], genb_sb[0:1, v0:v0 + VC_W],
                start=True, stop=False,
            )
            for kp in range(KP):
                nc.tensor.matmul(
                    ps[:mr, :VC_W],
                    hcat[:, kp, :, m0:m0 + mr],
                    genw_sb[:, kp, :, v0:v0 + VC_W],
                    start=False, stop=(kp == KP - 1),
                    perf_mode=PM.DoubleRow,
                )
            nc.vector.tensor_copy(lg[:mr, v0:v0 + VC_W], ps[:mr, :VC_W])
        for ec in range(OC_N):
            e0 = ec * OC_W
            tmp = p4c.tile([128, OC_W], dt.float32, tag="lpo", bufs=2)
            nc.scalar.activation(
                tmp[:mr], lg[:mr, e0:e0 + OC_W], AFT.Exp,
                accum_out=sume8[:mr, ec:ec + 1],
            )
        nc.vector.tensor_reduce(
            sume[:mr, m:m + 1], sume8[:mr, :], axis=mybir.AxisListType.X,
            op=ALU.add,
        )
        bar_in = dram.tile([128, 1], dt.float32, tag="bar_in", bufs=2)
        nc.gpsimd.dma_start(bar_in[:mr], sume[:mr, m:m + 1])
        bar_out = dram.tile([128, 1], dt.float32, tag="bar_out", bufs=2,
                            addr_space=SHARED)
        _allreduce(nc, bar_in[:], bar_out[:])
        sg = p4c.tile([128, 1], dt.float32, tag="sg", bufs=2)
        nc.gpsimd.dma_start(sg[:], bar_out[:])
        lse = p4c.tile([128, 1], dt.float32, tag="lse", bufs=2)
        nc.scalar.activation(lse[:mr], sg[:mr], AFT.Ln)
        for oc in range(OC_N):
            o0 = oc * OC_W
            lpo = p4c.tile([128, OC_W], dt.float32, tag="lpo", bufs=2)
            nc.vector.tensor_scalar(
                lpo[:mr], lg[:mr, o0:o0 + OC_W], lse[:mr, 0:1], None, ALU.subtract
            )
            nc.gpsimd.dma_start(logp.ap()[m0:m0 + mr, o0:o0 + OC_W], lpo[:mr])

    for s in range(S_EFF):
        h1t_prev = h1t_pp[s % 2]
        h0t_prev = h0t_pp[s % 2]
        h1t_next = h1t_pp[(s + 1) % 2]
        h0t_next = h0t_pp[(s + 1) % 2]

        for w in range(NW):
            wc = slice(w * WB, (w + 1) * WB)
            # DMA issue queue for this wave's exchange chains: SP for wave 0,
            # GpSimd (SWDGE) for wave 1 — avoids cross-chain head-of-line
            # blocking on one sequencer.
            dq = nc.sync if w == 0 else nc.gpsimd

            # --- hw = W1 h1 + attn_b for this wave's 16 cols, then pick own
            #     2 cols via the per-core sel matrix (SPMD-safe selection) ---
            ps_hw = psL.tile([128, KH, WB], dt.float32, tag="ps_hwx", bufs=2)
            for m in range(KH):
                nc.tensor.matmul(
                    ps_hw[:, m, :],
                    attn_b_sb[0:1, m * 128:(m + 1) * 128],
                    ones_bf[0:1, 0:WB],
                    start=True, stop=False,
                )
                for k in range(KH):
                    nc.tensor.matmul(
                        ps_hw[:, m, :],
                        w1t_sb[:, k, m * 128:(m + 1) * 128],
                        h1t_prev[:, k, w],
                        start=False, stop=(k == KH - 1),
                    )
            hwf = sbw.tile([128, KH, WB], dt.bfloat16, tag="hwf", bufs=2)
            nc.vector.tensor_copy(hwf[:], ps_hw[:])
            ps_t = psL.tile([WB, KH, 128], dt.bfloat16, tag="ps_hwx", bufs=2)
            for m in range(KH):
                nc.tensor.transpose(ps_t[:WB, m, :], hwf[:, m, :], id_bf[:, :])
            hwT = sbw.tile([WB, KH, 128], dt.bfloat16, tag="hwT", bufs=2)
            nc.vector.tensor_copy(hwT[:], ps_t[:WB, :, :])
            ps_own = psL.tile([128, KH, 2], dt.float32, tag="ps_hwx", bufs=2)
            for m in range(KH):
                nc.tensor.matmul(
                    ps_own[:, m, :], hwT[:WB, m, :], sel_own_sb[:],
                    start=True, stop=True,
                )
            hwt = sbw.tile([128, KH, 2], dt.float32, tag="hwt", bufs=2)
            nc.vector.tensor_copy(hwt[:], ps_own[:])
            # --- attention for wave's 2 local batches ---
            ps_sc = psL.tile([128, KT, 2], dt.float32, tag="ps_sc", bufs=1)
            for i in range(2):
                lb = 2 * w + i
                for half in range(2):
                    k0 = half * 4
                    en = sbw.tile([128, 4, T], dt.bfloat16, tag="en", bufs=2)
                    for kk in range(4):
                        nc.vector.tensor_scalar(
                            en[:, kk, :], encw2[:, lb, k0 + kk, :],
                            hwt[:, k0 + kk, i:i + 1], None, ALU.add,
                        )
                    ent = sbw.tile([128, 4, T], dt.bfloat16, tag="ent", bufs=2)
                    nc.scalar.activation(ent[:], en[:], AFT.Tanh)
                    for tk in range(KT):
                        for kk in range(4):
                            nc.tensor.matmul(
                                ps_sc[:, tk, i:i + 1],
                                ent[:, kk, tk * 128:(tk + 1) * 128],
                                vvec_sb[:, k0 + kk, :],
                                start=(k0 + kk == 0), stop=(k0 + kk == KH - 1),
                            )
            # --- softmax (unnormalized weights + reciprocal for ctx scale) ---
            exps = sbw.tile([128, KT, 2], dt.bfloat16, tag="exps", bufs=2)
            nc.scalar.activation(exps[:], ps_sc[:], AFT.Exp)
            ps_den = psL.tile([128, 2], dt.float32, tag="ps_small", bufs=1)
            for tk in range(KT):
                nc.tensor.matmul(
                    ps_den[0:1, :], ones_col[:, :], exps[:, tk, :],
                    start=(tk == 0), stop=(tk == KT - 1),
                )
            den = sbw.tile([1, 2], dt.float32, tag="den", bufs=2)
            nc.vector.tensor_copy(den[:], ps_den[0:1, :])
            rec = sbw.tile([1, 2], dt.float32, tag="rec", bufs=2)
            nc.vector.reciprocal(rec[:], den[:])
            ps_rcb = psL.tile([128, 2], dt.float32, tag="ps_small", bufs=1)
            nc.tensor.matmul(
                ps_rcb[:, :], ones_f32[:, :], rec[0:1, :], start=True, stop=True
            )
            recb = sbw.tile([128, 2], dt.float32, tag="recb", bufs=2)
            nc.vector.tensor_copy(recb[:], ps_rcb[:])
            # --- context (feature-major, scaled by 1/den on copy) ---
            ctxw = sbw.tile([128, KH, 2], dt.bfloat16, tag="ctxw", bufs=2)
            for i in range(2):
                lb = 2 * w + i
                ps_cx = psL.tile([128, KH], dt.float32, tag="ps_cx", bufs=1)
                for hk in range(KH):
                    for tk in range(KT):
                        nc.tensor.matmul(
                            ps_cx[:, hk:hk + 1],
                            enc_f8_sb[:, lb, tk, hk * 128:(hk + 1) * 128],
                            exps[:, tk, i:i + 1],
                            start=(tk == 0), stop=(tk == KT - 1),
                        )
                nc.vector.tensor_scalar(
                    ctxw[:, :, i], ps_cx[:], recb[:, i:i + 1], None, ALU.mult
                )
            # --- exchange ctx (AllGather over cores) ---
            bx_in = dram.tile([128, KH * 2], dt.bfloat16, tag="bx_in", bufs=3)
            bx_out = dram.tile([NCORES * 128, KH * 2], dt.bfloat16, tag="bx_out",
                               bufs=3, addr_space=SHARED)
            _allgather(nc, dq, ctxw[:].rearrange("p k i -> p (k i)"),
                       bx_in, bx_out[:])
            xt = sbw.tile([128, NCORES, KH, 2], dt.bfloat16, tag="xt", bufs=2)
            dq.dma_start(
                xt[:], bx_out[:].rearrange("(c p) (k i) -> p c k i", p=128, k=KH)
            )
            # --- LSTM layer 0 gates (N=16) ---
            ps_g0 = psL.tile([128, 4, WB], dt.float32, tag="ps_g", bufs=2)
            for gt in range(4):
                gsl = slice(gt * 128, (gt + 1) * 128)
                nc.tensor.matmul(
                    ps_g0[:, gt, :], id_bf[:],
                    g_emb[:, gt, s * B + w * WB:s * B + (w + 1) * WB],
                    start=True, stop=False,
                )
                for k in range(KH):
                    nc.tensor.matmul(
                        ps_g0[:, gt, :], whh0_sb[:, k, gsl],
                        h0t_prev[:, k, w], start=False, stop=False,
                    )
                for k in range(KH):
                    nc.tensor.matmul(
                        ps_g0[:, gt, :], wih0c_sb[:, k, gsl],
                        xt[:, :, k, :], start=False, stop=(k == KH - 1),
                    )
            h0n = cell(ps_g0, c_l0[:, wc], "l0")
            bh0_in = dram.tile([128, WB], dt.bfloat16, tag="bh0_in", bufs=3)
            bh0_out = dram.tile([NCORES * 128, WB], dt.bfloat16, tag="bh0_out",
                                bufs=3, addr_space=SHARED)
            _allgather(nc, dq, h0n[:], bh0_in, bh0_out[:])
            dq.dma_start(
                h0t_next[:, :, w],
                bh0_out[:].rearrange("(g p) (c i) -> p g c i", p=128, c=NCORES),
            )
            # --- LSTM layer 1 gates ---
            ps_g1 = psL.tile([128, 4, WB], dt.float32, tag="ps_g", bufs=2)
            for gt in range(4):
                gsl = slice(gt * 128, (gt + 1) * 128)
                nc.tensor.matmul(
                    ps_g1[:, gt, :], bias_g1_sb[0:1, gsl], ones_bf[0:1, :WB],
                    start=True, stop=False,
                )
                for k in range(KH):
                    nc.tensor.matmul(
                        ps_g1[:, gt, :], whh1_sb[:, k, gsl],
                        h1t_prev[:, k, w], start=False, stop=False,
                    )
                for k in range(KH):
                    nc.tensor.matmul(
                        ps_g1[:, gt, :], wih1_sb[:, k, gsl],
                        h0t_next[:, k, w], start=False, stop=(k == KH - 1),
                    )
            h1n = cell(ps_g1, c_l1[:, wc], "l1")
            bh1_in = dram.tile([128, WB], dt.bfloat16, tag="bh1_in", bufs=3)
            bh1_out = dram.tile([NCORES * 128, WB], dt.bfloat16, tag="bh1_out",
                                bufs=3, addr_space=SHARED)
            _allgather(nc, dq, h1n[:], bh1_in, bh1_out[:])
            dq.dma_start(
                h1t_next[:, :, w],
                bh1_out[:].rearrange("(g p) (c i) -> p g c i", p=128, c=NCORES),
            )
        # --- h1 history for the vocab projection (fp8, k-pair layout) ---
        for k in range(KH):
            nc.vector.tensor_copy(
                hcat[:, k // 2, k % 2, s * B:(s + 1) * B],
                h1t_next[:, k].rearrange("p w c i -> p (w c i)"),
            )
        # --- interleaved vocab projection bursts ---
        if (s + 1) % 4 == 0:
            m = (s + 1) // 4 - 1
            p4_burst(m, MTILES[m][0], MTILES[m][1])
        elif s == S_EFF - 1:
            m = ((s + 1) * B) // 128
            p4_burst(m, m * 128, (s + 1) * B - m * 128)

    # close loop pools
    p4c_cm.__exit__(None, None, None)
    p4ps_cm.__exit__(None, None, None)
    psL_cm.__exit__(None, None, None)
    sbw_cm.__exit__(None, None, None)
    loopers_cm.__exit__(None, None, None)
    dram_cm.__exit__(None, None, None)
    glob_cm.__exit__(None, None, None)


def _prep_inputs(inputs):
    """Host-side sharding/layout prep. Returns list of per-core input dicts."""
    f32 = np.float32
    enc_out = np.asarray(inputs["enc_out"], f32)
    enc_h = np.asarray(inputs["enc_h"], f32)
    enc_c = np.asarray(inputs["enc_c"], f32)
    emb = np.asarray(inputs["embedding"], f32)
    attn_W = np.asarray(inputs["attn_W"], f32)
    attn_b = np.asarray(inputs["attn_b"], f32)
    vv = np.asarray(inputs["v"], f32)
    Wih0 = np.asarray(inputs["Wih0"], f32)
    Whh0 = np.asarray(inputs["Whh0"], f32)
    bih0 = np.asarray(inputs["bih0"], f32)
    bhh0 = np.asarray(inputs["bhh0"], f32)
    Wih1 = np.asarray(inputs["Wih1"], f32)
    Whh1 = np.asarray(inputs["Whh1"], f32)
    bih1 = np.asarray(inputs["bih1"], f32)
    bhh1 = np.asarray(inputs["bhh1"], f32)
    genW = np.asarray(inputs["genW"], f32)
    genb = np.asarray(inputs["genb"], f32)
    q = np.asarray(inputs["question"]).astype(np.int64)

    W1 = attn_W[:, :H]
    W2 = attn_W[:, H:]
    h0 = np.concatenate([enc_h[0], enc_h[1]], 1)  # (B, H) layer 0
    h1 = np.concatenate([enc_h[2], enc_h[3]], 1)  # layer 1
    c0 = np.concatenate([enc_c[0], enc_c[1]], 1)
    c1 = np.concatenate([enc_c[2], enc_c[3]], 1)

    # step-column order: col = w*16 + c2*2 + i  <->  global batch 4*c2 + 2*w + i
    col2gb = np.array(
        [4 * c2 + 2 * w + i for w in range(NW) for c2 in range(NCORES)
         for i in range(2)], dtype=np.int64)
    qperm = q[col2gb, :].T.reshape(NSAMP, 1).astype(np.int32)  # (s, col) order

    def bf(x):
        return np.ascontiguousarray(x).astype(BF)

    def f8(x):
        return np.ascontiguousarray(x).astype(F8)

    shared = {
        "w2t": bf(W2.T),
        "w1t": bf(W1.T),
        "attn_bias": bf(attn_b.reshape(1, H)),
        "vvec": bf(vv.reshape(H, 1)),
        "emb_tab": np.ascontiguousarray(emb),
        "qidx": qperm,
        "h0t_init": bf(h0[col2gb].T),
        "h1t_init": bf(h1[col2gb].T),
    }
    maps = []
    for c in range(NCORES):
        # local batches (in (w, i) order) = global ids for this core's slots
        my_gb = [4 * c + 2 * w + i for w in range(NW) for i in range(2)]
        sel_own = np.zeros((WB, 2), f32)
        for i in range(2):
            sel_own[c * 2 + i, i] = 1.0
        # gate rows: order i|f|o|g (torch order is i,f,g,o -> pick blocks 0,1,3,2)
        gorder = [0, 1, 3, 2]
        rows = np.concatenate(
            [np.arange(g * H + c * GS, g * H + (c + 1) * GS) for g in gorder]
        )
        wih0_s = Wih0[rows]  # (NG, E+H)
        wih0e = np.zeros((EP, NG), f32)
        wih0e[:E] = wih0_s[:, :E].T
        bias0 = (bih0 + bhh0)[rows]  # (NG,)
        bias_g0c = bias0.reshape(4, GS).T  # (GS, 4)
        vrows = slice(c * VPC, (c + 1) * VPC)
        genw_t = genW[vrows].T  # (H, VPC)
        genw_kp = np.ascontiguousarray(
            genw_t.reshape(KP, 2, 128, VPC).transpose(2, 0, 1, 3)
        )
        m = dict(shared)
        m.update({
            "enc_tr": bf(enc_out[my_gb].transpose(0, 2, 1)),
            "enc_f8": f8(enc_out[my_gb]),
            "wih0e": bf(wih0e),
            "sel_own": bf(sel_own),
            "bias_g0c": np.ascontiguousarray(bias_g0c),
            "wih0c": bf(wih0_s[:, E:].T),
            "whh0": bf(Whh0[rows].T),
            "wih1": bf(Wih1[rows].T),
            "whh1": bf(Whh1[rows].T),
            "bias_g1": bf((bih1 + bhh1)[rows].reshape(1, NG)),
            "c0_l0": np.ascontiguousarray(c0[col2gb, c * GS:(c + 1) * GS].T),
            "c0_l1": np.ascontiguousarray(c1[col2gb, c * GS:(c + 1) * GS].T),
            "genw_kp": f8(genw_kp),
            "genb_v": bf(genb[vrows].reshape(1, VPC)),
        })
        maps.append(m)
    return maps


_CACHED = {}


def _get_compiled():
    if "nc" not in _CACHED:
        nc = bacc.Bacc(
            "TRN2", target_bir_lowering=False, debug=False,
            num_devices=1 if SIM1 else NCORES,
        )
        build(nc)
        nc.compile()
        _CACHED["nc"] = nc
    return _CACHED["nc"]


def run_cores(in_maps, **kw):
    nc = _get_compiled()
    return bass_utils.run_bass_kernel_spmd(nc, in_maps, list(range(NCORES)), **kw)


def kernel(**inputs):
    in_maps = _prep_inputs(inputs)
    res = run_cores(in_maps)
    parts = [res.results[c]["logp"] for c in range(NCORES)]
    full = np.concatenate(parts, axis=1)  # (NSAMP, V) in (s, col) order
    col2gb = np.array(
        [4 * c2 + 2 * w + i for w in range(NW) for c2 in range(NCORES)
         for i in range(2)], dtype=np.int64)
    full = full.reshape(S, B, V)
    out = np.empty((B, S, V), np.float32)
    out[col2gb, :, :] = full.transpose(1, 0, 2)
    return np.ascontiguousarray(out)


# revision 46
# speedup vs baseline: 2.6033x; 1.3068x over previous
"""Trainium2 Bass kernel for the attention-LSTM decoder (nn_Decoder).

Strategy (8 NeuronCores), v2 — restructured for the TRN2 cost model
(matmul cost ~ output free size; Act/DVE cost ~ free size; DVE 4x for
bf16 SBUF tensor_scalar):
  - Attention batch-sharded: each core owns B/8 = 4 batches. Energies are
    computed feature-major: DVE adds the per-step hidden bias (4x mode),
    Act does tanh in 2 big instructions per batch. Scores/softmax are
    transpose-free (ones-matmul partition reductions, unnormalized exp
    weights with context post-scaling).
  - LSTM tensor-parallel over gate rows (512/core, gate order i|f|o|g),
    everything feature-major so gate matmuls have N=16 and the cell state
    lives as (128, B) tiles. Batches advance in 2 waves of 16 columns to
    pipeline the 3 per-wave exchanges under the Act-bound tanh.
  - Vocab projection tensor-parallel over V (4000/core) in fp8 with
    DoubleRow (2 k-tiles per matmul, 0.5 cyc/row), interleaved into the
    recurrent loop per 128-sample mtile; per-mtile exp-sums, logsumexp
    AllReduce, subtract, and f32 output DMA all stream during the loop.
Dtypes: bf16 compute everywhere, fp32 PSUM + cell state, fp8e4m3 for the
ctx encoder operand and the vocab projection (genW and the h1 history).
"""
import os
import sys

sys.path.insert(0, "/opt/trn_rl_repo")

import numpy as np
import ml_dtypes

import concourse.bass as bass
import concourse.bacc as bacc
import concourse.mybir as mybir
import concourse.tile as tile
from concourse import bass_utils
from concourse.masks import make_identity

BF = ml_dtypes.bfloat16
F8 = ml_dtypes.float8_e4m3
dt = mybir.dt
AFT = mybir.ActivationFunctionType
ALU = mybir.AluOpType
PM = mybir.MatmulPerfMode

B, T, H, E, V, S = 32, 512, 1024, 300, 32000, 50
NCORES = 8
BPC = B // NCORES      # 4 batches per core
GS = H // NCORES       # 128-wide hidden slice per core
NG = 4 * GS            # 512 gate rows per core (i|f|o|g blocks of 128)
VPC = V // NCORES      # 4000 vocab rows per core
EP = 384               # padded embedding feature dim (3 k-tiles)
KE = EP // 128         # 3
KH = H // 128          # 8
KT = T // 128          # 4
KP = KH // 2           # 4 k-pairs for fp8 DoubleRow
NSAMP = S * B          # 1600
NW = int(os.environ.get("DECODER_NW", "4"))   # batch waves per step
WB = B // NW           # step-columns per wave
NBW = BPC // NW        # local batches per wave
S_EFF = int(os.environ.get("DECODER_STEPS", str(S)))
SIM1 = os.environ.get("DECODER_SIM", "0") == "1"
RG = [list(range(NCORES))]
SHARED = "Local" if SIM1 else "Shared"

# phase-4 sample tiles: 12 x 128 + 1 x 64
MTILES = [(m * 128, min(128, NSAMP - m * 128)) for m in range((NSAMP + 127) // 128)]
VC_N, VC_W = 16, 250     # vocab chunks for the projection psum
OC_N, OC_W = 8, 500      # output chunks for subtract + DMA


def _exchange(nc, eng, src_flat_ap, dst_bc_ap, stage_tile, shared_tile,
              reload_out_ap, reload_in_ap):
    """AllGather src (sbuf, (128, n)) into a consumer sbuf tile holding all
    8 cores' slices. SIM1 cost proxy: ONE fan-out DMA straight into the
    destination SBUF tile — the cost a remote-DMA-broadcast implementation
    would pay per exchange (same bytes x 8 destinations). Real build:
    stage to dram -> AllGather collective -> reload (collectives need dram).
    `eng` picks the DMA issue queue (SP / Pool)."""
    if SIM1:
        rows, cols = src_flat_ap.shape[0], src_flat_ap.shape[1]
        eng.dma_start(
            dst_bc_ap,
            src_flat_ap.unsqueeze(1).broadcast_to((rows, NCORES, cols)),
        )
    else:
        eng.dma_start(stage_tile[:], src_flat_ap)
        nc.gpsimd.collective_compute(
            "AllGather", mybir.AluOpType.bypass, replica_groups=RG,
            ins=[stage_tile[:].opt()], outs=[shared_tile[:].opt()],
        )
        eng.dma_start(reload_out_ap, reload_in_ap)


def _allreduce(nc, in_ap, out_ap):
    if SIM1:
        nc.gpsimd.dma_start(out_ap, in_ap)
    else:
        nc.gpsimd.collective_compute(
            "AllReduce", mybir.AluOpType.add, replica_groups=RG,
            ins=[in_ap.opt()], outs=[out_ap.opt()],
        )


def build(nc):
    di = {}

    def inp(name, shape, dtype):
        di[name] = nc.dram_tensor(name, list(shape), dtype, kind="ExternalInput")
        return di[name]

    inp("enc_tr", (BPC, H, T), dt.bfloat16)       # feature-major enc (p1b rhs)
    inp("enc_f8", (BPC, T, H), dt.float8e4)       # time-major enc (ctx lhsT)
    inp("w2t", (H, H), dt.bfloat16)
    inp("w1t", (H, H), dt.bfloat16)
    inp("attn_bias", (1, H), dt.bfloat16)
    inp("vvec", (H, 1), dt.bfloat16)
    inp("emb_tab", (V, E), dt.float32)
    inp("qidx", (NSAMP, 1), dt.int32)
    inp("wih0e", (EP, NG), dt.bfloat16)
    inp("bias_g0c", (GS, 4), dt.float32)
    inp("wih0c", (H, NG), dt.bfloat16)
    inp("whh0", (H, NG), dt.bfloat16)
    inp("wih1", (H, NG), dt.bfloat16)
    inp("whh1", (H, NG), dt.bfloat16)
    inp("bias_g1", (1, NG), dt.bfloat16)
    inp("sel_own", (WB, NBW), dt.bfloat16)
    inp("h0t_init", (H, B), dt.bfloat16)
    inp("h1t_init", (H, B), dt.bfloat16)
    inp("c0_l0", (GS, B), dt.float32)
    inp("c0_l1", (GS, B), dt.float32)
    inp("genw_kp", (128, KP, 2, VPC), dt.float8e4)
    inp("genb_v", (1, VPC), dt.bfloat16)
    logp = nc.dram_tensor("logp", [NSAMP, VPC], dt.float32, kind="ExternalOutput")

    with tile.TileContext(nc) as tc:
        _body(nc, tc, di, logp)
    return di


def _body(nc, tc, di, logp):
    glob_cm = tc.tile_pool(name="glob", bufs=1)
    glob = glob_cm.__enter__()
    dram_cm = tc.tile_pool(name="dram", bufs=1, space="DRAM")
    dram = dram_cm.__enter__()

    # ---- global constants ----
    id_bf = glob.tile([128, 128], dt.bfloat16, name="id_bf")
    id_f32 = glob.tile([128, 128], dt.float32, name="id_f32")
    make_identity(nc, id_bf[:])
    make_identity(nc, id_f32[:])
    ones_bf = glob.tile([1, 512], dt.bfloat16, name="ones_bf")
    nc.gpsimd.memset(ones_bf[:], 1.0)
    ones_col = glob.tile([128, 1], dt.bfloat16, name="ones_col")
    nc.gpsimd.memset(ones_col[:], 1.0)
    ones_f32 = glob.tile([1, 128], dt.float32, name="ones_f32")
    nc.gpsimd.memset(ones_f32[:], 1.0)
    sume = glob.tile([128, len(MTILES)], dt.float32, name="sume")
    sume8 = glob.tile([128, OC_N], dt.float32, name="sume8")

    # ---------------- persistent loop tensors ----------------
    loopers_cm = tc.tile_pool(name="loopers", bufs=1)
    loopers = loopers_cm.__enter__()

    w1t_sb = loopers.tile([128, KH, H], dt.bfloat16, name="w1t_sb")
    nc.sync.dma_start(w1t_sb[:], di["w1t"].ap().rearrange("(k p) h -> p k h", p=128))
    vvec_sb = loopers.tile([128, KH, 1], dt.bfloat16, name="vvec_sb")
    nc.sync.dma_start(vvec_sb[:], di["vvec"].ap().rearrange("(k p) o -> p k o", p=128))
    attn_b_sb = loopers.tile([1, H], dt.bfloat16, name="attn_b_sb")
    nc.sync.dma_start(attn_b_sb[:], di["attn_bias"].ap())
    wih0c_sb = loopers.tile([128, KH, NG], dt.bfloat16, name="wih0c_sb")
    nc.sync.dma_start(wih0c_sb[:], di["wih0c"].ap().rearrange("(k p) g -> p k g", p=128))
    whh0_sb = loopers.tile([128, KH, NG], dt.bfloat16, name="whh0_sb")
    nc.sync.dma_start(whh0_sb[:], di["whh0"].ap().rearrange("(k p) g -> p k g", p=128))
    wih1_sb = loopers.tile([128, KH, NG], dt.bfloat16, name="wih1_sb")
    nc.sync.dma_start(wih1_sb[:], di["wih1"].ap().rearrange("(k p) g -> p k g", p=128))
    whh1_sb = loopers.tile([128, KH, NG], dt.bfloat16, name="whh1_sb")
    nc.sync.dma_start(whh1_sb[:], di["whh1"].ap().rearrange("(k p) g -> p k g", p=128))
    bias_g1_sb = loopers.tile([1, NG], dt.bfloat16, name="bias_g1_sb")
    nc.sync.dma_start(bias_g1_sb[:], di["bias_g1"].ap())
    enc_f8_sb = loopers.tile([128, BPC, KT, H], dt.float8e4, name="enc_f8_sb")
    for _b in range(BPC):
        nc.sync.dma_start(
            enc_f8_sb[:, _b, :, :],
            di["enc_f8"].ap()[_b].rearrange("(k p) h -> p k h", p=128),
        )
    # hidden state ping-pong, factored (p, k, w, c, i)
    h0t_pp = [
        loopers.tile([128, KH, NW, NCORES, NBW], dt.bfloat16, name=f"h0t_pp{i}")
        for i in range(2)
    ]
    h1t_pp = [
        loopers.tile([128, KH, NW, NCORES, NBW], dt.bfloat16, name=f"h1t_pp{i}")
        for i in range(2)
    ]
    nc.sync.dma_start(
        h0t_pp[0][:],
        di["h0t_init"].ap().rearrange("(k p) (w c i) -> p k w c i", p=128, w=NW, c=NCORES),
    )
    nc.sync.dma_start(
        h1t_pp[0][:],
        di["h1t_init"].ap().rearrange("(k p) (w c i) -> p k w c i", p=128, w=NW, c=NCORES),
    )
    c_l0 = loopers.tile([128, B], dt.float32, name="c_l0")
    nc.sync.dma_start(c_l0[:], di["c0_l0"].ap())
    c_l1 = loopers.tile([128, B], dt.float32, name="c_l1")
    nc.sync.dma_start(c_l1[:], di["c0_l1"].ap())
    genw_sb = loopers.tile([128, KP, 2, VPC], dt.float8e4, name="genw_sb")
    nc.sync.dma_start(genw_sb[:], di["genw_kp"].ap())
    genb_sb = loopers.tile([1, VPC], dt.bfloat16, name="genb_sb")
    nc.sync.dma_start(genb_sb[:], di["genb_v"].ap())
    bias_g0c_sb = loopers.tile([128, 4], dt.float32, name="bias_g0c_sb")
    nc.sync.dma_start(bias_g0c_sb[:], di["bias_g0c"].ap())
    sel_own_sb = loopers.tile([WB, NBW], dt.bfloat16, name="sel_own_sb")
    nc.sync.dma_start(sel_own_sb[:], di["sel_own"].ap())

    encw2 = loopers.tile([128, BPC, KH, T], dt.bfloat16, name="encw2")
    g_emb = loopers.tile([128, 4, NSAMP], dt.bfloat16, name="g_emb")
    hcat = loopers.tile([128, KP, 2, NSAMP], dt.float8e4, name="hcat")

    # ---- phase 1: embedding gather/transpose, encW2, emb-gate precompute ----
    with tc.tile_pool(name="p1emb", bufs=1) as p1emb:
        emb_t = p1emb.tile([128, KE, NSAMP], dt.bfloat16, name="emb_t")
        nc.gpsimd.memset(emb_t[:], 0.0)
        wih0e_sb = p1emb.tile([128, KE, NG], dt.bfloat16, name="wih0e_sb")
        nc.sync.dma_start(
            wih0e_sb[:], di["wih0e"].ap().rearrange("(k p) g -> p k g", p=128)
        )

        # 1a: gather + transpose to feature-major
        with tc.tile_pool(name="p1e", bufs=3) as p1e, \
             tc.tile_pool(name="p1eps", bufs=3, space="PSUM") as p1eps:
            for (m0, mr) in MTILES:
                idx = p1e.tile([128, 1], dt.int32, tag="idx")
                nc.sync.dma_start(idx[:mr, :], di["qidx"].ap()[m0:m0 + mr, :])
                gath = p1e.tile([128, E], dt.float32, tag="gath")
                nc.gpsimd.indirect_dma_start(
                    out=gath[:mr, :],
                    out_offset=None,
                    in_=di["emb_tab"].ap(),
                    in_offset=bass.IndirectOffsetOnAxis(ap=idx[:mr, 0:1], axis=0),
                )
                for k in range(KE):
                    cw = min(128, E - k * 128)
                    ps = p1eps.tile([128, 128], dt.float32, tag="ps")
                    nc.tensor.transpose(
                        ps[:cw, :mr], gath[:mr, k * 128:k * 128 + cw],
                        id_f32[:mr, :mr]
                    )
                    nc.vector.tensor_copy(emb_t[:cw, k, m0:m0 + mr], ps[:cw, :mr])

        # 1b: encW2[b] feature-major = W2 @ enc[b].T
        with tc.tile_pool(name="p1w", bufs=1) as p1w, \
             tc.tile_pool(name="p1s", bufs=3) as p1s, \
             tc.tile_pool(name="p1ps", bufs=1, space="PSUM") as p1ps:
            w2t_sb = p1w.tile([128, KH, H], dt.bfloat16, name="w2t_sb")
            nc.sync.dma_start(
                w2t_sb[:], di["w2t"].ap().rearrange("(k p) h -> p k h", p=128)
            )
            for b in range(BPC):
                pss = [
                    p1ps.tile([128, T], dt.float32, tag=f"p1p{m}", name=f"p1p{b}_{m}")
                    for m in range(KH)
                ]
                for k in range(KH):
                    rhs = p1s.tile([128, T], dt.bfloat16, tag="rhs")
                    nc.sync.dma_start(
                        rhs[:], di["enc_tr"].ap()[b, k * 128:(k + 1) * 128, :]
                    )
                    for m in range(KH):
                        nc.tensor.matmul(
                            pss[m][:],
                            w2t_sb[:, k, m * 128:(m + 1) * 128],
                            rhs[:],
                            start=(k == 0),
                            stop=(k == KH - 1),
                        )
                for m in range(KH):
                    if m % 2 == 0:
                        nc.vector.tensor_copy(encw2[:, b, m, :], pss[m][:])
                    else:
                        nc.scalar.activation(encw2[:, b, m, :], pss[m][:], AFT.Copy)

        # 1c: embedding gate contributions (bias folded on the copy)
        with tc.tile_pool(name="p1gps", bufs=3, space="PSUM") as p1gps:
            for gt in range(4):
                for ch in range(4):
                    c0 = ch * 400
                    ps = p1gps.tile([128, 400], dt.float32, tag="gps")
                    for ke in range(KE):
                        nc.tensor.matmul(
                            ps[:],
                            wih0e_sb[:, ke, gt * 128:(gt + 1) * 128],
                            emb_t[:, ke, c0:c0 + 400],
                            start=(ke == 0),
                            stop=(ke == KE - 1),
                        )
                    nc.vector.tensor_scalar(
                        g_emb[:, gt, c0:c0 + 400], ps[:],
                        bias_g0c_sb[:, gt:gt + 1], None, ALU.add,
                    )

    # ---------------- phase 2: the recurrent loop ----------------
    sbw_cm = tc.tile_pool(name="sbw", bufs=2)
    sbw = sbw_cm.__enter__()
    psL_cm = tc.tile_pool(name="psL", bufs=1, space="PSUM")
    psL = psL_cm.__enter__()
    p4ps_cm = tc.tile_pool(name="p4ps", bufs=1, space="PSUM")
    p4ps = p4ps_cm.__enter__()
    p4c_cm = tc.tile_pool(name="p4c", bufs=1)
    p4c = p4c_cm.__enter__()

    def cell(gps, c_ap, tag):
        """gates i|f|o|g (each (128, WB) psum slice). Updates c_ap in place,
        returns h (128, WB) bf16. Elementwise runs on GpSimd (SBUF-only ops)
        to keep the DVE queue free for the energy bias-adds."""
        ifo_r = sbw.tile([128, 3, WB], dt.float32, tag=f"ifor{tag}")
        nc.scalar.activation(ifo_r[:], gps[:, 0:3, :], AFT.Tanh, scale=0.5)
        tg = sbw.tile([128, WB], dt.float32, tag=f"tg{tag}")
        nc.scalar.activation(tg[:], gps[:, 3, :], AFT.Tanh)
        ifo = sbw.tile([128, 3, WB], dt.float32, tag=f"ifo{tag}")
        nc.gpsimd.tensor_scalar(ifo[:], ifo_r[:], 0.5, 0.5, ALU.mult, ALU.add)
        t_fc = sbw.tile([128, WB], dt.float32, tag=f"tfc{tag}")
        nc.gpsimd.tensor_tensor(t_fc[:], ifo[:, 1, :], c_ap, op=ALU.mult)
        t_ig = sbw.tile([128, WB], dt.float32, tag=f"tig{tag}")
        nc.gpsimd.tensor_tensor(t_ig[:], ifo[:, 0, :], tg[:], op=ALU.mult)
        nc.gpsimd.tensor_tensor(c_ap, t_fc[:], t_ig[:], op=ALU.add)
        tc2 = sbw.tile([128, WB], dt.float32, tag=f"tc2{tag}")
        nc.scalar.activation(tc2[:], c_ap, AFT.Tanh)
        h = sbw.tile([128, WB], dt.bfloat16, tag=f"h{tag}")
        nc.gpsimd.tensor_tensor(h[:], ifo[:, 2, :], tc2[:], op=ALU.mult)
        return h

    def p4_burst(m, m0, mr):
        lg = p4c.tile([128, VPC], dt.bfloat16, tag="lgits", bufs=1)
        for vc in range(VC_N):
            v0 = vc * VC_W
            ps = p4ps.tile([128, 256], dt.float32, tag="p4p")
            nc.tensor.matmul(
                ps[:mr, :VC_W], ones_bf[0:1, :mr], genb_sb[0:1, v0:v0 + VC_W],
                start=True, stop=False,
            )
            for kp in range(KP):
                nc.tensor.matmul(
                    ps[:mr, :VC_W],
                    hcat[:, kp, :, m0:m0 + mr],
                    genw_sb[:, kp, :, v0:v0 + VC_W],
                    start=False, stop=(kp == KP - 1),
                    perf_mode=PM.DoubleRow,
                )
            nc.vector.tensor_copy(lg[:mr, v0:v0 + VC_W], ps[:mr, :VC_W])
        for ec in range(OC_N):
            e0 = ec * OC_W
            tmp = p4c.tile([128, OC_W], dt.float32, tag="lpo", bufs=2)
            nc.scalar.activation(
                tmp[:mr], lg[:mr, e0:e0 + OC_W], AFT.Exp,
                accum_out=sume8[:mr, ec:ec + 1],
            )
        nc.vector.tensor_reduce(
            sume[:mr, m:m + 1], sume8[:mr, :], axis=mybir.AxisListType.X,
            op=ALU.add,
        )
        bar_in = dram.tile([128, 1], dt.float32, tag="bar_in", bufs=2)
        nc.gpsimd.dma_start(bar_in[:mr], sume[:mr, m:m + 1])
        bar_out = dram.tile([128, 1], dt.float32, tag="bar_out", bufs=2,
                            addr_space=SHARED)
        _allreduce(nc, bar_in[:], bar_out[:])
        sg = p4c.tile([128, 1], dt.float32, tag="sg", bufs=2)
        nc.gpsimd.dma_start(sg[:], bar_out[:])
        lse = p4c.tile([128, 1], dt.float32, tag="lse", bufs=2)
        nc.scalar.activation(lse[:mr], sg[:mr], AFT.Ln)
        for oc in range(OC_N):
            o0 = oc * OC_W
            lpo = p4c.tile([128, OC_W], dt.float32, tag="lpo", bufs=2)
            nc.vector.tensor_scalar(
                lpo[:mr], lg[:mr, o0:o0 + OC_W], lse[:mr, 0:1], None, ALU.subtract
            )
            nc.gpsimd.dma_start(logp.ap()[m0:m0 + mr, o0:o0 + OC_W], lpo[:mr])

    for s in range(S_EFF):
        h1t_prev = h1t_pp[s % 2]
        h0t_prev = h0t_pp[s % 2]
        h1t_next = h1t_pp[(s + 1) % 2]
        h0t_next = h0t_pp[(s + 1) % 2]

        for w in range(NW):
            wc = slice(w * WB, (w + 1) * WB)
            # DMA issue queue for this wave's exchange chains: SP for wave 0,
            # GpSimd (SWDGE) for wave 1 — avoids cross-chain head-of-line
            # blocking on one sequencer.
            dq = nc.sync if w % 2 == 0 else nc.gpsimd

            # --- hw = W1 h1 + attn_b for this wave's 16 cols, then pick own
            #     2 cols via the per-core sel matrix (SPMD-safe selection) ---
            ps_hw = psL.tile([128, KH, WB], dt.float32, tag="ps_hwx", bufs=2)
            for m in range(KH):
                nc.tensor.matmul(
                    ps_hw[:, m, :],
                    attn_b_sb[0:1, m * 128:(m + 1) * 128],
                    ones_bf[0:1, 0:WB],
                    start=True, stop=False,
                )
                for k in range(KH):
                    nc.tensor.matmul(
                        ps_hw[:, m, :],
                        w1t_sb[:, k, m * 128:(m + 1) * 128],
                        h1t_prev[:, k, w],
                        start=False, stop=(k == KH - 1),
                    )
            hwf = sbw.tile([128, KH, WB], dt.bfloat16, tag="hwf", bufs=2)
            nc.vector.tensor_copy(hwf[:], ps_hw[:])
            ps_t = psL.tile([WB, KH, 128], dt.bfloat16, tag="ps_hwx", bufs=2)
            for m in range(KH):
                nc.tensor.transpose(ps_t[:WB, m, :], hwf[:, m, :], id_bf[:, :])
            hwT = sbw.tile([WB, KH, 128], dt.bfloat16, tag="hwT", bufs=2)
            nc.vector.tensor_copy(hwT[:], ps_t[:WB, :, :])
            ps_own = psL.tile([128, KH, NBW], dt.float32, tag="ps_hwx", bufs=2)
            for m in range(KH):
                nc.tensor.matmul(
                    ps_own[:, m, :], hwT[:WB, m, :], sel_own_sb[:],
                    start=True, stop=True,
                )
            hwt = sbw.tile([128, KH, NBW], dt.float32, tag="hwt", bufs=2)
            nc.vector.tensor_copy(hwt[:], ps_own[:])
            # --- attention for wave's 2 local batches ---
            ps_sc = psL.tile([128, KT, NBW], dt.float32, tag="ps_sc", bufs=1)
            for i in range(NBW):
                lb = NBW * w + i
                for half in range(2):
                    k0 = half * 4
                    en = sbw.tile([128, 4, T], dt.bfloat16, tag="en", bufs=3)
                    for kk in range(4):
                        nc.vector.tensor_scalar(
                            en[:, kk, :], encw2[:, lb, k0 + kk, :],
                            hwt[:, k0 + kk, i:i + 1], None, ALU.add,
                        )
                    ent = sbw.tile([128, 4, T], dt.bfloat16, tag="ent", bufs=2)
                    nc.scalar.activation(ent[:], en[:], AFT.Tanh)
                    for tk in range(KT):
                        for kk in range(4):
                            nc.tensor.matmul(
                                ps_sc[:, tk, i:i + 1],
                                ent[:, kk, tk * 128:(tk + 1) * 128],
                                vvec_sb[:, k0 + kk, :],
                                start=(k0 + kk == 0), stop=(k0 + kk == KH - 1),
                            )
            # --- softmax (unnormalized weights + reciprocal for ctx scale) ---
            exps = sbw.tile([128, KT, NBW], dt.bfloat16, tag="exps", bufs=2)
            nc.scalar.activation(exps[:], ps_sc[:], AFT.Exp)
            ps_den = psL.tile([128, NBW], dt.float32, tag="ps_small", bufs=1)
            for tk in range(KT):
                nc.tensor.matmul(
                    ps_den[0:1, :], ones_col[:, :], exps[:, tk, :],
                    start=(tk == 0), stop=(tk == KT - 1),
                )
            rec = sbw.tile([1, NBW], dt.float32, tag="rec", bufs=2)
            nc.vector.reciprocal(rec[:], ps_den[0:1, :])
            ps_rcb = psL.tile([128, NBW], dt.float32, tag="ps_small", bufs=1)
            nc.tensor.matmul(
                ps_rcb[:, :], ones_f32[:, :], rec[0:1, :], start=True, stop=True
            )
            recb = sbw.tile([128, NBW], dt.float32, tag="recb", bufs=2)
            nc.vector.tensor_copy(recb[:], ps_rcb[:])
            # --- context (feature-major, scaled by 1/den on copy) ---
            ctxw = sbw.tile([128, KH, NBW], dt.bfloat16, tag="ctxw", bufs=2)
            for i in range(NBW):
                lb = NBW * w + i
                ps_cx = psL.tile([128, KH], dt.float32, tag="ps_cx", bufs=1)
                for hk in range(KH):
                    for tk in range(KT):
                        nc.tensor.matmul(
                            ps_cx[:, hk:hk + 1],
                            enc_f8_sb[:, lb, tk, hk * 128:(hk + 1) * 128],
                            exps[:, tk, i:i + 1],
                            start=(tk == 0), stop=(tk == KT - 1),
                        )
                nc.vector.tensor_scalar(
                    ctxw[:, :, i], ps_cx[:], recb[:, i:i + 1], None, ALU.mult
                )
            # --- exchange ctx (AllGather over cores) ---
            bx_in = dram.tile([128, KH * NBW], dt.bfloat16, tag="bx_in", bufs=3)
            bx_out = dram.tile([NCORES * 128, KH * NBW], dt.bfloat16, tag="bx_out",
                               bufs=3, addr_space=SHARED)
            xt = sbw.tile([128, NCORES, KH, NBW], dt.bfloat16, tag="xt", bufs=2)
            _exchange(
                nc, dq, ctxw[:].rearrange("p k i -> p (k i)"),
                xt[:].rearrange("p c k i -> p c (k i)"), bx_in, bx_out,
                xt[:],
                bx_out[:].rearrange("(c p) (k i) -> p c k i", p=128, k=KH),
            )
            # --- LSTM layer 0 gates (N=16) ---
            ps_g0 = psL.tile([128, 4, WB], dt.float32, tag="ps_g", bufs=2)
            for gt in range(4):
                gsl = slice(gt * 128, (gt + 1) * 128)
                nc.tensor.matmul(
                    ps_g0[:, gt, :], id_bf[:],
                    g_emb[:, gt, s * B + w * WB:s * B + (w + 1) * WB],
                    start=True, stop=False,
                )
                for k in range(KH):
                    nc.tensor.matmul(
                        ps_g0[:, gt, :], whh0_sb[:, k, gsl],
                        h0t_prev[:, k, w], start=False, stop=False,
                    )
                for k in range(KH):
                    nc.tensor.matmul(
                        ps_g0[:, gt, :], wih0c_sb[:, k, gsl],
                        xt[:, :, k, :], start=False, stop=(k == KH - 1),
                    )
            h0n = cell(ps_g0, c_l0[:, wc], "l0")
            bh0_in = dram.tile([128, WB], dt.bfloat16, tag="bh0_in", bufs=3)
            bh0_out = dram.tile([NCORES * 128, WB], dt.bfloat16, tag="bh0_out",
                                bufs=3, addr_space=SHARED)
            _exchange(
                nc, dq, h0n[:],
                h0t_next[:, :, w].rearrange("p g c i -> p g (c i)"),
                bh0_in, bh0_out,
                h0t_next[:, :, w],
                bh0_out[:].rearrange("(g p) (c i) -> p g c i", p=128, c=NCORES),
            )
            # --- LSTM layer 1 gates ---
            ps_g1 = psL.tile([128, 4, WB], dt.float32, tag="ps_g", bufs=2)
            for gt in range(4):
                gsl = slice(gt * 128, (gt + 1) * 128)
                nc.tensor.matmul(
                    ps_g1[:, gt, :], bias_g1_sb[0:1, gsl], ones_bf[0:1, :WB],
                    start=True, stop=False,
                )
                for k in range(KH):
                    nc.tensor.matmul(
                        ps_g1[:, gt, :], whh1_sb[:, k, gsl],
                        h1t_prev[:, k, w], start=False, stop=False,
                    )
                for k in range(KH):
                    nc.tensor.matmul(
                        ps_g1[:, gt, :], wih1_sb[:, k, gsl],
                        h0t_next[:, k, w], start=False, stop=(k == KH - 1),
                    )
            h1n = cell(ps_g1, c_l1[:, wc], "l1")
            bh1_in = dram.tile([128, WB], dt.bfloat16, tag="bh1_in", bufs=3)
            bh1_out = dram.tile([NCORES * 128, WB], dt.bfloat16, tag="bh1_out",
                                bufs=3, addr_space=SHARED)
            _exchange(
                nc, dq, h1n[:],
                h1t_next[:, :, w].rearrange("p g c i -> p g (c i)"),
                bh1_in, bh1_out,
                h1t_next[:, :, w],
                bh1_out[:].rearrange("(g p) (c i) -> p g c i", p=128, c=NCORES),
            )
        # --- h1 history for the vocab projection (fp8, k-pair layout) ---
        for k in range(KH):
            nc.vector.tensor_copy(
                hcat[:, k // 2, k % 2, s * B:(s + 1) * B],
                h1t_next[:, k].rearrange("p w c i -> p (w c i)"),
            )
        # --- interleaved vocab projection bursts ---
        if (s + 1) % 4 == 0:
            m = (s + 1) // 4 - 1
            p4_burst(m, MTILES[m][0], MTILES[m][1])
        elif s == S_EFF - 1:
            m = ((s + 1) * B) // 128
            p4_burst(m, m * 128, (s + 1) * B - m * 128)

    # close loop pools
    p4c_cm.__exit__(None, None, None)
    p4ps_cm.__exit__(None, None, None)
    psL_cm.__exit__(None, None, None)
    sbw_cm.__exit__(None, None, None)
    loopers_cm.__exit__(None, None, None)
    dram_cm.__exit__(None, None, None)
    glob_cm.__exit__(None, None, None)


def _prep_inputs(inputs):
    """Host-side sharding/layout prep. Returns list of per-core input dicts."""
    f32 = np.float32
    enc_out = np.asarray(inputs["enc_out"], f32)
    enc_h = np.asarray(inputs["enc_h"], f32)
    enc_c = np.asarray(inputs["enc_c"], f32)
    emb = np.asarray(inputs["embedding"], f32)
    attn_W = np.asarray(inputs["attn_W"], f32)
    attn_b = np.asarray(inputs["attn_b"], f32)
    vv = np.asarray(inputs["v"], f32)
    Wih0 = np.asarray(inputs["Wih0"], f32)
    Whh0 = np.asarray(inputs["Whh0"], f32)
    bih0 = np.asarray(inputs["bih0"], f32)
    bhh0 = np.asarray(inputs["bhh0"], f32)
    Wih1 = np.asarray(inputs["Wih1"], f32)
    Whh1 = np.asarray(inputs["Whh1"], f32)
    bih1 = np.asarray(inputs["bih1"], f32)
    bhh1 = np.asarray(inputs["bhh1"], f32)
    genW = np.asarray(inputs["genW"], f32)
    genb = np.asarray(inputs["genb"], f32)
    q = np.asarray(inputs["question"]).astype(np.int64)

    W1 = attn_W[:, :H]
    W2 = attn_W[:, H:]
    h0 = np.concatenate([enc_h[0], enc_h[1]], 1)  # (B, H) layer 0
    h1 = np.concatenate([enc_h[2], enc_h[3]], 1)  # layer 1
    c0 = np.concatenate([enc_c[0], enc_c[1]], 1)
    c1 = np.concatenate([enc_c[2], enc_c[3]], 1)

    # step-column order: col = w*16 + c2*2 + i  <->  global batch 4*c2 + 2*w + i
    col2gb = np.array(
        [4 * c2 + NBW * w + i for w in range(NW) for c2 in range(NCORES)
         for i in range(NBW)], dtype=np.int64)
    qperm = q[col2gb, :].T.reshape(NSAMP, 1).astype(np.int32)  # (s, col) order

    def bf(x):
        return np.ascontiguousarray(x).astype(BF)

    def f8(x):
        return np.ascontiguousarray(x).astype(F8)

    shared = {
        "w2t": bf(W2.T),
        "w1t": bf(W1.T),
        "attn_bias": bf(attn_b.reshape(1, H)),
        "vvec": bf(vv.reshape(H, 1)),
        "emb_tab": np.ascontiguousarray(emb),
        "qidx": qperm,
        "h0t_init": bf(h0[col2gb].T),
        "h1t_init": bf(h1[col2gb].T),
    }
    maps = []
    for c in range(NCORES):
        # local batches (in (w, i) order) = global ids for this core's slots
        my_gb = [4 * c + NBW * w + i for w in range(NW) for i in range(NBW)]
        sel_own = np.zeros((WB, NBW), f32)
        for i in range(NBW):
            sel_own[c * NBW + i, i] = 1.0
        # gate rows: order i|f|o|g (torch order is i,f,g,o -> pick blocks 0,1,3,2)
        gorder = [0, 1, 3, 2]
        rows = np.concatenate(
            [np.arange(g * H + c * GS, g * H + (c + 1) * GS) for g in gorder]
        )
        wih0_s = Wih0[rows]  # (NG, E+H)
        wih0e = np.zeros((EP, NG), f32)
        wih0e[:E] = wih0_s[:, :E].T
        bias0 = (bih0 + bhh0)[rows]  # (NG,)
        bias_g0c = bias0.reshape(4, GS).T  # (GS, 4)
        vrows = slice(c * VPC, (c + 1) * VPC)
        genw_t = genW[vrows].T  # (H, VPC)
        genw_kp = np.ascontiguousarray(
            genw_t.reshape(KP, 2, 128, VPC).transpose(2, 0, 1, 3)
        )
        m = dict(shared)
        m.update({
            "enc_tr": bf(enc_out[my_gb].transpose(0, 2, 1)),
            "enc_f8": f8(enc_out[my_gb]),
            "wih0e": bf(wih0e),
            "sel_own": bf(sel_own),
            "bias_g0c": np.ascontiguousarray(bias_g0c),
            "wih0c": bf(wih0_s[:, E:].T),
            "whh0": bf(Whh0[rows].T),
            "wih1": bf(Wih1[rows].T),
            "whh1": bf(Whh1[rows].T),
            "bias_g1": bf((bih1 + bhh1)[rows].reshape(1, NG)),
            "c0_l0": np.ascontiguousarray(c0[col2gb, c * GS:(c + 1) * GS].T),
            "c0_l1": np.ascontiguousarray(c1[col2gb, c * GS:(c + 1) * GS].T),
            "genw_kp": f8(genw_kp),
            "genb_v": bf(genb[vrows].reshape(1, VPC)),
        })
        maps.append(m)
    return maps


_CACHED = {}


def _get_compiled():
    if "nc" not in _CACHED:
        nc = bacc.Bacc(
            "TRN2", target_bir_lowering=False, debug=False,
            num_devices=1 if SIM1 else NCORES,
        )
        build(nc)
        nc.compile()
        _CACHED["nc"] = nc
    return _CACHED["nc"]


def run_cores(in_maps, **kw):
    nc = _get_compiled()
    return bass_utils.run_bass_kernel_spmd(nc, in_maps, list(range(NCORES)), **kw)


def kernel(**inputs):
    in_maps = _prep_inputs(inputs)
    res = run_cores(in_maps)
    parts = [res.results[c]["logp"] for c in range(NCORES)]
    full = np.concatenate(parts, axis=1)  # (NSAMP, V) in (s, col) order
    col2gb = np.array(
        [4 * c2 + NBW * w + i for w in range(NW) for c2 in range(NCORES)
         for i in range(NBW)], dtype=np.int64)
    full = full.reshape(S, B, V)
    out = np.empty((B, S, V), np.float32)
    out[col2gb, :, :] = full.transpose(1, 0, 2)
    return np.ascontiguousarray(out)


# revision 50
# speedup vs baseline: 2.8556x; 1.0969x over previous
"""Trainium2 Bass kernel for the attention-LSTM decoder (nn_Decoder).

Strategy (8 NeuronCores), v2 — restructured for the TRN2 cost model
(matmul cost ~ output free size; Act/DVE cost ~ free size; DVE 4x for
bf16 SBUF tensor_scalar):
  - Attention batch-sharded: each core owns B/8 = 4 batches. Energies are
    computed feature-major: DVE adds the per-step hidden bias (4x mode),
    Act does tanh in 2 big instructions per batch. Scores/softmax are
    transpose-free (ones-matmul partition reductions, unnormalized exp
    weights with context post-scaling).
  - LSTM tensor-parallel over gate rows (512/core, gate order i|f|o|g),
    everything feature-major so gate matmuls have small-N outputs and the
    cell state lives as (128, B) tiles. Batches advance in NW=4 waves of 8
    columns, giving 4 independent per-step pipelines whose exchange chains
    (issued alternately on the SP and GpSimd DMA queues) overlap the
    Act-bound tanh work; cell elementwise runs on GpSimd.
  - Vocab projection tensor-parallel over V (4000/core) in fp8 with
    DoubleRow (2 k-tiles per matmul, 0.5 cyc/row), interleaved into the
    recurrent loop per 128-sample mtile; per-mtile exp-sums, logsumexp
    AllReduce, subtract, and f32 output DMA all stream during the loop.
Dtypes: bf16 compute everywhere, fp32 PSUM + cell state, fp8e4m3 for the
ctx encoder operand and the vocab projection (genW and the h1 history).
"""
import os
import sys

sys.path.insert(0, "/opt/trn_rl_repo")

import numpy as np
import ml_dtypes

import concourse.bass as bass
import concourse.bacc as bacc
import concourse.mybir as mybir
import concourse.tile as tile
from concourse import bass_utils
from concourse.masks import make_identity

BF = ml_dtypes.bfloat16
F8 = ml_dtypes.float8_e4m3
dt = mybir.dt
AFT = mybir.ActivationFunctionType
ALU = mybir.AluOpType
PM = mybir.MatmulPerfMode

B, T, H, E, V, S = 32, 512, 1024, 300, 32000, 50
NCORES = 8
BPC = B // NCORES      # 4 batches per core
GS = H // NCORES       # 128-wide hidden slice per core
NG = 4 * GS            # 512 gate rows per core (i|f|o|g blocks of 128)
VPC = V // NCORES      # 4000 vocab rows per core
EP = 384               # padded embedding feature dim (3 k-tiles)
KE = EP // 128         # 3
KH = H // 128          # 8
KT = T // 128          # 4
KP = KH // 2           # 4 k-pairs for fp8 DoubleRow
NSAMP = S * B          # 1600
NW = int(os.environ.get("DECODER_NW", "4"))   # batch waves per step
WB = B // NW           # step-columns per wave
NBW = BPC // NW        # local batches per wave
S_EFF = int(os.environ.get("DECODER_STEPS", str(S)))
SIM1 = os.environ.get("DECODER_SIM", "0") == "1"
RG = [list(range(NCORES))]
SHARED = "Local" if SIM1 else "Shared"

# phase-4 sample tiles: 12 x 128 + 1 x 64
MTILES = [(m * 128, min(128, NSAMP - m * 128)) for m in range((NSAMP + 127) // 128)]
VC_N, VC_W = 16, 250     # vocab chunks for the projection psum
OC_N, OC_W = 8, 500      # output chunks for subtract + DMA


def _exchange(nc, eng, src_flat_ap, dst_bc_ap, stage_tile, shared_tile,
              reload_out_ap, reload_in_ap):
    """AllGather src (sbuf, (128, n)) into a consumer sbuf tile holding all
    8 cores' slices. SIM1 cost proxy: ONE fan-out DMA straight into the
    destination SBUF tile — the cost a remote-DMA-broadcast implementation
    would pay per exchange (same bytes x 8 destinations). Real build:
    stage to dram -> AllGather collective -> reload (collectives need dram).
    `eng` picks the DMA issue queue (SP / Pool)."""
    if SIM1:
        rows, cols = src_flat_ap.shape[0], src_flat_ap.shape[1]
        eng.dma_start(
            dst_bc_ap,
            src_flat_ap.unsqueeze(1).broadcast_to((rows, NCORES, cols)),
        )
    else:
        eng.dma_start(stage_tile[:], src_flat_ap)
        nc.gpsimd.collective_compute(
            "AllGather", mybir.AluOpType.bypass, replica_groups=RG,
            ins=[stage_tile[:].opt()], outs=[shared_tile[:].opt()],
        )
        eng.dma_start(reload_out_ap, reload_in_ap)


def _allreduce(nc, in_ap, out_ap):
    if SIM1:
        nc.gpsimd.dma_start(out_ap, in_ap)
    else:
        nc.gpsimd.collective_compute(
            "AllReduce", mybir.AluOpType.add, replica_groups=RG,
            ins=[in_ap.opt()], outs=[out_ap.opt()],
        )


def build(nc):
    di = {}

    def inp(name, shape, dtype):
        di[name] = nc.dram_tensor(name, list(shape), dtype, kind="ExternalInput")
        return di[name]

    inp("enc_tr", (BPC, H, T), dt.bfloat16)       # feature-major enc (p1b rhs)
    inp("enc_f8", (BPC, T, H), dt.float8e4)       # time-major enc (ctx lhsT)
    inp("w2t", (H, H), dt.bfloat16)
    inp("w1t", (H, H), dt.bfloat16)
    inp("attn_bias", (1, H), dt.bfloat16)
    inp("vvec", (H, 1), dt.bfloat16)
    inp("emb_tab", (V, E), dt.float32)
    inp("qidx", (NSAMP, 1), dt.int32)
    inp("wih0e", (EP, NG), dt.bfloat16)
    inp("bias_g0c", (GS, 4), dt.float32)
    inp("wih0c", (H, NG), dt.bfloat16)
    inp("whh0", (H, NG), dt.bfloat16)
    inp("wih1", (H, NG), dt.bfloat16)
    inp("whh1", (H, NG), dt.bfloat16)
    inp("bias_g1", (1, NG), dt.bfloat16)
    inp("sel_own", (WB, NBW), dt.bfloat16)
    inp("h0t_init", (H, B), dt.bfloat16)
    inp("h1t_init", (H, B), dt.bfloat16)
    inp("c0_l0", (GS, B), dt.float32)
    inp("c0_l1", (GS, B), dt.float32)
    inp("genw_kp", (128, KP, 2, VPC), dt.float8e4)
    inp("genb_v", (1, VPC), dt.bfloat16)
    logp = nc.dram_tensor("logp", [NSAMP, VPC], dt.float32, kind="ExternalOutput")

    with tile.TileContext(nc) as tc:
        _body(nc, tc, di, logp)
    return di


def _body(nc, tc, di, logp):
    glob_cm = tc.tile_pool(name="glob", bufs=1)
    glob = glob_cm.__enter__()
    dram_cm = tc.tile_pool(name="dram", bufs=1, space="DRAM")
    dram = dram_cm.__enter__()

    # ---- global constants ----
    id_bf = glob.tile([128, 128], dt.bfloat16, name="id_bf")
    id_f32 = glob.tile([128, 128], dt.float32, name="id_f32")
    make_identity(nc, id_bf[:])
    make_identity(nc, id_f32[:])
    ones_bf = glob.tile([1, 512], dt.bfloat16, name="ones_bf")
    nc.gpsimd.memset(ones_bf[:], 1.0)
    ones_col = glob.tile([128, 1], dt.bfloat16, name="ones_col")
    nc.gpsimd.memset(ones_col[:], 1.0)
    ones_f32 = glob.tile([1, 128], dt.float32, name="ones_f32")
    nc.gpsimd.memset(ones_f32[:], 1.0)
    sume = glob.tile([128, len(MTILES)], dt.float32, name="sume")
    sume8 = glob.tile([128, OC_N], dt.float32, name="sume8")

    # ---------------- persistent loop tensors ----------------
    loopers_cm = tc.tile_pool(name="loopers", bufs=1)
    loopers = loopers_cm.__enter__()

    w1t_sb = loopers.tile([128, KH, H], dt.bfloat16, name="w1t_sb")
    nc.sync.dma_start(w1t_sb[:], di["w1t"].ap().rearrange("(k p) h -> p k h", p=128))
    vvec_sb = loopers.tile([128, KH, 1], dt.bfloat16, name="vvec_sb")
    nc.sync.dma_start(vvec_sb[:], di["vvec"].ap().rearrange("(k p) o -> p k o", p=128))
    attn_b_sb = loopers.tile([1, H], dt.bfloat16, name="attn_b_sb")
    nc.sync.dma_start(attn_b_sb[:], di["attn_bias"].ap())
    wih0c_sb = loopers.tile([128, KH, NG], dt.bfloat16, name="wih0c_sb")
    nc.sync.dma_start(wih0c_sb[:], di["wih0c"].ap().rearrange("(k p) g -> p k g", p=128))
    whh0_sb = loopers.tile([128, KH, NG], dt.bfloat16, name="whh0_sb")
    nc.sync.dma_start(whh0_sb[:], di["whh0"].ap().rearrange("(k p) g -> p k g", p=128))
    wih1_sb = loopers.tile([128, KH, NG], dt.bfloat16, name="wih1_sb")
    nc.sync.dma_start(wih1_sb[:], di["wih1"].ap().rearrange("(k p) g -> p k g", p=128))
    whh1_sb = loopers.tile([128, KH, NG], dt.bfloat16, name="whh1_sb")
    nc.sync.dma_start(whh1_sb[:], di["whh1"].ap().rearrange("(k p) g -> p k g", p=128))
    bias_g1_sb = loopers.tile([1, NG], dt.bfloat16, name="bias_g1_sb")
    nc.sync.dma_start(bias_g1_sb[:], di["bias_g1"].ap())
    enc_f8_sb = loopers.tile([128, BPC, KT, H], dt.float8e4, name="enc_f8_sb")
    for _b in range(BPC):
        nc.sync.dma_start(
            enc_f8_sb[:, _b, :, :],
            di["enc_f8"].ap()[_b].rearrange("(k p) h -> p k h", p=128),
        )
    # hidden state ping-pong, factored (p, k, w, c, i)
    h0t_pp = [
        loopers.tile([128, KH, NW, NCORES, NBW], dt.bfloat16, name=f"h0t_pp{i}")
        for i in range(2)
    ]
    h1t_pp = [
        loopers.tile([128, KH, NW, NCORES, NBW], dt.bfloat16, name=f"h1t_pp{i}")
        for i in range(2)
    ]
    nc.sync.dma_start(
        h0t_pp[0][:],
        di["h0t_init"].ap().rearrange("(k p) (w c i) -> p k w c i", p=128, w=NW, c=NCORES),
    )
    nc.sync.dma_start(
        h1t_pp[0][:],
        di["h1t_init"].ap().rearrange("(k p) (w c i) -> p k w c i", p=128, w=NW, c=NCORES),
    )
    c_l0 = loopers.tile([128, B], dt.float32, name="c_l0")
    nc.sync.dma_start(c_l0[:], di["c0_l0"].ap())
    c_l1 = loopers.tile([128, B], dt.float32, name="c_l1")
    nc.sync.dma_start(c_l1[:], di["c0_l1"].ap())
    genw_sb = loopers.tile([128, KP, 2, VPC], dt.float8e4, name="genw_sb")
    nc.sync.dma_start(genw_sb[:], di["genw_kp"].ap())
    genb_sb = loopers.tile([1, VPC], dt.bfloat16, name="genb_sb")
    nc.sync.dma_start(genb_sb[:], di["genb_v"].ap())
    bias_g0c_sb = loopers.tile([128, 4], dt.float32, name="bias_g0c_sb")
    nc.sync.dma_start(bias_g0c_sb[:], di["bias_g0c"].ap())
    sel_own_sb = loopers.tile([WB, NBW], dt.bfloat16, name="sel_own_sb")
    nc.sync.dma_start(sel_own_sb[:], di["sel_own"].ap())

    encw2 = loopers.tile([128, BPC, KH, T], dt.bfloat16, name="encw2")
    g_emb = loopers.tile([128, 4, NSAMP], dt.bfloat16, name="g_emb")
    hcat = loopers.tile([128, KP, 2, NSAMP], dt.float8e4, name="hcat")

    # ---- phase 1: embedding gather/transpose, encW2, emb-gate precompute ----
    with tc.tile_pool(name="p1emb", bufs=1) as p1emb:
        emb_t = p1emb.tile([128, KE, NSAMP], dt.bfloat16, name="emb_t")
        nc.gpsimd.memset(emb_t[:], 0.0)
        wih0e_sb = p1emb.tile([128, KE, NG], dt.bfloat16, name="wih0e_sb")
        nc.sync.dma_start(
            wih0e_sb[:], di["wih0e"].ap().rearrange("(k p) g -> p k g", p=128)
        )

        # 1a: gather + transpose to feature-major
        with tc.tile_pool(name="p1e", bufs=3) as p1e, \
             tc.tile_pool(name="p1eps", bufs=3, space="PSUM") as p1eps:
            for (m0, mr) in MTILES:
                idx = p1e.tile([128, 1], dt.int32, tag="idx")
                nc.sync.dma_start(idx[:mr, :], di["qidx"].ap()[m0:m0 + mr, :])
                gath = p1e.tile([128, E], dt.float32, tag="gath")
                nc.gpsimd.indirect_dma_start(
                    out=gath[:mr, :],
                    out_offset=None,
                    in_=di["emb_tab"].ap(),
                    in_offset=bass.IndirectOffsetOnAxis(ap=idx[:mr, 0:1], axis=0),
                )
                for k in range(KE):
                    cw = min(128, E - k * 128)
                    ps = p1eps.tile([128, 128], dt.float32, tag="ps")
                    nc.tensor.transpose(
                        ps[:cw, :mr], gath[:mr, k * 128:k * 128 + cw],
                        id_f32[:mr, :mr]
                    )
                    nc.vector.tensor_copy(emb_t[:cw, k, m0:m0 + mr], ps[:cw, :mr])

        # 1b: encW2[b] feature-major = W2 @ enc[b].T
        with tc.tile_pool(name="p1w", bufs=1) as p1w, \
             tc.tile_pool(name="p1s", bufs=3) as p1s, \
             tc.tile_pool(name="p1ps", bufs=1, space="PSUM") as p1ps:
            w2t_sb = p1w.tile([128, KH, H], dt.bfloat16, name="w2t_sb")
            nc.sync.dma_start(
                w2t_sb[:], di["w2t"].ap().rearrange("(k p) h -> p k h", p=128)
            )
            for b in range(BPC):
                pss = [
                    p1ps.tile([128, T], dt.float32, tag=f"p1p{m}", name=f"p1p{b}_{m}")
                    for m in range(KH)
                ]
                for k in range(KH):
                    rhs = p1s.tile([128, T], dt.bfloat16, tag="rhs")
                    nc.sync.dma_start(
                        rhs[:], di["enc_tr"].ap()[b, k * 128:(k + 1) * 128, :]
                    )
                    for m in range(KH):
                        nc.tensor.matmul(
                            pss[m][:],
                            w2t_sb[:, k, m * 128:(m + 1) * 128],
                            rhs[:],
                            start=(k == 0),
                            stop=(k == KH - 1),
                        )
                for m in range(KH):
                    if m % 2 == 0:
                        nc.vector.tensor_copy(encw2[:, b, m, :], pss[m][:])
                    else:
                        nc.scalar.activation(encw2[:, b, m, :], pss[m][:], AFT.Copy)

        # 1c: embedding gate contributions (bias folded on the copy)
        with tc.tile_pool(name="p1gps", bufs=3, space="PSUM") as p1gps:
            for gt in range(4):
                for ch in range(4):
                    c0 = ch * 400
                    ps = p1gps.tile([128, 400], dt.float32, tag="gps")
                    for ke in range(KE):
                        nc.tensor.matmul(
                            ps[:],
                            wih0e_sb[:, ke, gt * 128:(gt + 1) * 128],
                            emb_t[:, ke, c0:c0 + 400],
                            start=(ke == 0),
                            stop=(ke == KE - 1),
                        )
                    nc.vector.tensor_scalar(
                        g_emb[:, gt, c0:c0 + 400], ps[:],
                        bias_g0c_sb[:, gt:gt + 1], None, ALU.add,
                    )

    # ---------------- phase 2: the recurrent loop ----------------
    sbw_cm = tc.tile_pool(name="sbw", bufs=2)
    sbw = sbw_cm.__enter__()
    psL_cm = tc.tile_pool(name="psL", bufs=1, space="PSUM")
    psL = psL_cm.__enter__()
    p4ps_cm = tc.tile_pool(name="p4ps", bufs=1, space="PSUM")
    p4ps = p4ps_cm.__enter__()
    p4c_cm = tc.tile_pool(name="p4c", bufs=1)
    p4c = p4c_cm.__enter__()

    def cell(gps, c_ap, tag):
        """gates i|f|o|g, i/f/o pre-scaled by 0.5 on the host so one tanh
        covers all four (sigmoid(x) = tanh(x/2)/2 + 0.5). Updates c_ap in place,
        returns h (128, WB) bf16. Elementwise runs on GpSimd (SBUF-only ops)
        to keep the DVE queue free for the energy bias-adds."""
        ifog = sbw.tile([128, 4, WB], dt.float32, tag=f"ifog{tag}")
        nc.scalar.activation(ifog[:], gps[:, :, :], AFT.Tanh)
        ifo = sbw.tile([128, 3, WB], dt.float32, tag=f"ifo{tag}")
        nc.gpsimd.tensor_scalar(ifo[:], ifog[:, 0:3, :], 0.5, 0.5, ALU.mult, ALU.add)
        t_fc = sbw.tile([128, WB], dt.float32, tag=f"tfc{tag}")
        nc.gpsimd.tensor_tensor(t_fc[:], ifo[:, 1, :], c_ap, op=ALU.mult)
        t_ig = sbw.tile([128, WB], dt.float32, tag=f"tig{tag}")
        nc.gpsimd.tensor_tensor(t_ig[:], ifo[:, 0, :], ifog[:, 3, :], op=ALU.mult)
        nc.gpsimd.tensor_tensor(c_ap, t_fc[:], t_ig[:], op=ALU.add)
        tc2 = sbw.tile([128, WB], dt.float32, tag=f"tc2{tag}")
        nc.scalar.activation(tc2[:], c_ap, AFT.Tanh)
        h = sbw.tile([128, WB], dt.bfloat16, tag=f"h{tag}")
        nc.gpsimd.tensor_tensor(h[:], ifo[:, 2, :], tc2[:], op=ALU.mult)
        return h

    def p4_burst(m, m0, mr):
        lg = p4c.tile([128, VPC], dt.bfloat16, tag="lgits", bufs=1)
        for vc in range(VC_N):
            v0 = vc * VC_W
            ps = p4ps.tile([128, 256], dt.float32, tag="p4p")
            nc.tensor.matmul(
                ps[:mr, :VC_W], ones_bf[0:1, :mr], genb_sb[0:1, v0:v0 + VC_W],
                start=True, stop=False,
            )
            for kp in range(KP):
                nc.tensor.matmul(
                    ps[:mr, :VC_W],
                    hcat[:, kp, :, m0:m0 + mr],
                    genw_sb[:, kp, :, v0:v0 + VC_W],
                    start=False, stop=(kp == KP - 1),
                    perf_mode=PM.DoubleRow,
                )
            nc.vector.tensor_copy(lg[:mr, v0:v0 + VC_W], ps[:mr, :VC_W])
        for ec in range(OC_N):
            e0 = ec * OC_W
            tmp = p4c.tile([128, OC_W], dt.float32, tag="lpo", bufs=2)
            nc.scalar.activation(
                tmp[:mr], lg[:mr, e0:e0 + OC_W], AFT.Exp,
                accum_out=sume8[:mr, ec:ec + 1],
            )
        nc.vector.tensor_reduce(
            sume[:mr, m:m + 1], sume8[:mr, :], axis=mybir.AxisListType.X,
            op=ALU.add,
        )
        bar_in = dram.tile([128, 1], dt.float32, tag="bar_in", bufs=2)
        nc.gpsimd.dma_start(bar_in[:mr], sume[:mr, m:m + 1])
        bar_out = dram.tile([128, 1], dt.float32, tag="bar_out", bufs=2,
                            addr_space=SHARED)
        _allreduce(nc, bar_in[:], bar_out[:])
        sg = p4c.tile([128, 1], dt.float32, tag="sg", bufs=2)
        nc.gpsimd.dma_start(sg[:], bar_out[:])
        lse = p4c.tile([128, 1], dt.float32, tag="lse", bufs=2)
        nc.scalar.activation(lse[:mr], sg[:mr], AFT.Ln)
        for oc in range(OC_N):
            o0 = oc * OC_W
            lpo = p4c.tile([128, OC_W], dt.float32, tag="lpo", bufs=2)
            nc.vector.tensor_scalar(
                lpo[:mr], lg[:mr, o0:o0 + OC_W], lse[:mr, 0:1], None, ALU.subtract
            )
            nc.gpsimd.dma_start(logp.ap()[m0:m0 + mr, o0:o0 + OC_W], lpo[:mr])

    for s in range(S_EFF):
        h1t_prev = h1t_pp[s % 2]
        h0t_prev = h0t_pp[s % 2]
        h1t_next = h1t_pp[(s + 1) % 2]
        h0t_next = h0t_pp[(s + 1) % 2]

        for w in range(NW):
            wc = slice(w * WB, (w + 1) * WB)
            # DMA issue queue for this wave's exchange chains: SP for wave 0,
            # GpSimd (SWDGE) for wave 1 — avoids cross-chain head-of-line
            # blocking on one sequencer.
            dq = nc.sync if w % 2 == 0 else nc.gpsimd

            # --- hw = W1 h1 + attn_b for this wave's 16 cols, then pick own
            #     2 cols via the per-core sel matrix (SPMD-safe selection) ---
            ps_hw = psL.tile([128, KH, WB], dt.float32, tag="ps_hwx", bufs=2)
            for m in range(KH):
                nc.tensor.matmul(
                    ps_hw[:, m, :],
                    attn_b_sb[0:1, m * 128:(m + 1) * 128],
                    ones_bf[0:1, 0:WB],
                    start=True, stop=False,
                )
                for k in range(KH):
                    nc.tensor.matmul(
                        ps_hw[:, m, :],
                        w1t_sb[:, k, m * 128:(m + 1) * 128],
                        h1t_prev[:, k, w],
                        start=False, stop=(k == KH - 1),
                    )
            hwf = sbw.tile([128, KH, WB], dt.bfloat16, tag="hwf", bufs=2)
            nc.vector.tensor_copy(hwf[:], ps_hw[:])
            ps_t = psL.tile([WB, KH, 128], dt.bfloat16, tag="ps_hwx", bufs=2)
            for m in range(KH):
                nc.tensor.transpose(ps_t[:WB, m, :], hwf[:, m, :], id_bf[:, :])
            hwT = sbw.tile([WB, KH, 128], dt.bfloat16, tag="hwT", bufs=2)
            nc.vector.tensor_copy(hwT[:], ps_t[:WB, :, :])
            ps_own = psL.tile([128, KH, NBW], dt.float32, tag="ps_hwx", bufs=2)
            for m in range(KH):
                nc.tensor.matmul(
                    ps_own[:, m, :], hwT[:WB, m, :], sel_own_sb[:],
                    start=True, stop=True,
                )
            hwt = sbw.tile([128, KH, NBW], dt.float32, tag="hwt", bufs=2)
            nc.vector.tensor_copy(hwt[:], ps_own[:])
            # --- attention for wave's 2 local batches ---
            ps_sc = psL.tile([128, KT, NBW], dt.float32, tag="ps_hwx", bufs=2)
            for i in range(NBW):
                lb = NBW * w + i
                for half in range(2):
                    k0 = half * 4
                    en = sbw.tile([128, 4, T], dt.bfloat16, tag="en", bufs=3)
                    for kk in range(4):
                        nc.vector.tensor_scalar(
                            en[:, kk, :], encw2[:, lb, k0 + kk, :],
                            hwt[:, k0 + kk, i:i + 1], None, ALU.add,
                        )
                    ent = sbw.tile([128, 4, T], dt.bfloat16, tag="ent", bufs=2)
                    nc.scalar.activation(ent[:], en[:], AFT.Tanh)
                    for tk in range(KT):
                        for kk in range(4):
                            nc.tensor.matmul(
                                ps_sc[:, tk, i:i + 1],
                                ent[:, kk, tk * 128:(tk + 1) * 128],
                                vvec_sb[:, k0 + kk, :],
                                start=(k0 + kk == 0), stop=(k0 + kk == KH - 1),
                            )
            # --- softmax (unnormalized weights + reciprocal for ctx scale) ---
            exps = sbw.tile([128, KT, NBW], dt.bfloat16, tag="exps", bufs=2)
            nc.scalar.activation(exps[:], ps_sc[:], AFT.Exp)
            ps_den = psL.tile([128, NBW], dt.float32, tag="ps_small", bufs=1)
            for tk in range(KT):
                nc.tensor.matmul(
                    ps_den[0:1, :], ones_col[:, :], exps[:, tk, :],
                    start=(tk == 0), stop=(tk == KT - 1),
                )
            rec = sbw.tile([1, NBW], dt.float32, tag="rec", bufs=2)
            nc.vector.reciprocal(rec[:], ps_den[0:1, :])
            ps_rcb = psL.tile([128, NBW], dt.float32, tag="ps_small", bufs=1)
            nc.tensor.matmul(
                ps_rcb[:, :], ones_f32[:, :], rec[0:1, :], start=True, stop=True
            )
            recb = sbw.tile([128, NBW], dt.float32, tag="recb", bufs=2)
            nc.vector.tensor_copy(recb[:], ps_rcb[:])
            # --- context (feature-major, scaled by 1/den on copy) ---
            ctxw = sbw.tile([128, KH, NBW], dt.bfloat16, tag="ctxw", bufs=2)
            for i in range(NBW):
                lb = NBW * w + i
                ps_cx = psL.tile([128, KH], dt.float32, tag="ps_cx", bufs=1)
                for hk in range(KH):
                    for tk in range(KT):
                        nc.tensor.matmul(
                            ps_cx[:, hk:hk + 1],
                            enc_f8_sb[:, lb, tk, hk * 128:(hk + 1) * 128],
                            exps[:, tk, i:i + 1],
                            start=(tk == 0), stop=(tk == KT - 1),
                        )
                nc.vector.tensor_scalar(
                    ctxw[:, :, i], ps_cx[:], recb[:, i:i + 1], None, ALU.mult
                )
            # --- exchange ctx (AllGather over cores) ---
            bx_in = dram.tile([128, KH * NBW], dt.bfloat16, tag="bx_in", bufs=3)
            bx_out = dram.tile([NCORES * 128, KH * NBW], dt.bfloat16, tag="bx_out",
                               bufs=3, addr_space=SHARED)
            xt = sbw.tile([128, NCORES, KH, NBW], dt.bfloat16, tag="xt", bufs=2)
            _exchange(
                nc, dq, ctxw[:].rearrange("p k i -> p (k i)"),
                xt[:].rearrange("p c k i -> p c (k i)"), bx_in, bx_out,
                xt[:],
                bx_out[:].rearrange("(c p) (k i) -> p c k i", p=128, k=KH),
            )
            # --- LSTM layer 0 gates (N=16) ---
            ps_g0 = psL.tile([128, 4, WB], dt.float32, tag="ps_g", bufs=3)
            for gt in range(4):
                gsl = slice(gt * 128, (gt + 1) * 128)
                nc.tensor.matmul(
                    ps_g0[:, gt, :], id_bf[:],
                    g_emb[:, gt, s * B + w * WB:s * B + (w + 1) * WB],
                    start=True, stop=False,
                )
                for k in range(KH):
                    nc.tensor.matmul(
                        ps_g0[:, gt, :], whh0_sb[:, k, gsl],
                        h0t_prev[:, k, w], start=False, stop=False,
                    )
                for k in range(KH):
                    nc.tensor.matmul(
                        ps_g0[:, gt, :], wih0c_sb[:, k, gsl],
                        xt[:, :, k, :], start=False, stop=(k == KH - 1),
                    )
            h0n = cell(ps_g0, c_l0[:, wc], "l0")
            bh0_in = dram.tile([128, WB], dt.bfloat16, tag="bh0_in", bufs=3)
            bh0_out = dram.tile([NCORES * 128, WB], dt.bfloat16, tag="bh0_out",
                                bufs=3, addr_space=SHARED)
            _exchange(
                nc, dq, h0n[:],
                h0t_next[:, :, w].rearrange("p g c i -> p g (c i)"),
                bh0_in, bh0_out,
                h0t_next[:, :, w],
                bh0_out[:].rearrange("(g p) (c i) -> p g c i", p=128, c=NCORES),
            )
            # --- LSTM layer 1 gates ---
            ps_g1 = psL.tile([128, 4, WB], dt.float32, tag="ps_g", bufs=3)
            for gt in range(4):
                gsl = slice(gt * 128, (gt + 1) * 128)
                nc.tensor.matmul(
                    ps_g1[:, gt, :], bias_g1_sb[0:1, gsl], ones_bf[0:1, :WB],
                    start=True, stop=False,
                )
                for k in range(KH):
                    nc.tensor.matmul(
                        ps_g1[:, gt, :], whh1_sb[:, k, gsl],
                        h1t_prev[:, k, w], start=False, stop=False,
                    )
                for k in range(KH):
                    nc.tensor.matmul(
                        ps_g1[:, gt, :], wih1_sb[:, k, gsl],
                        h0t_next[:, k, w], start=False, stop=(k == KH - 1),
                    )
            h1n = cell(ps_g1, c_l1[:, wc], "l1")
            bh1_in = dram.tile([128, WB], dt.bfloat16, tag="bh1_in", bufs=3)
            bh1_out = dram.tile([NCORES * 128, WB], dt.bfloat16, tag="bh1_out",
                                bufs=3, addr_space=SHARED)
            _exchange(
                nc, dq, h1n[:],
                h1t_next[:, :, w].rearrange("p g c i -> p g (c i)"),
                bh1_in, bh1_out,
                h1t_next[:, :, w],
                bh1_out[:].rearrange("(g p) (c i) -> p g c i", p=128, c=NCORES),
            )
        # --- h1 history for the vocab projection (fp8, k-pair layout) ---
        for k in range(KH):
            nc.vector.tensor_copy(
                hcat[:, k // 2, k % 2, s * B:(s + 1) * B],
                h1t_next[:, k].rearrange("p w c i -> p (w c i)"),
            )
        # --- interleaved vocab projection bursts ---
        if (s + 1) % 4 == 0:
            m = (s + 1) // 4 - 1
            p4_burst(m, MTILES[m][0], MTILES[m][1])
        elif s == S_EFF - 1:
            m = ((s + 1) * B) // 128
            p4_burst(m, m * 128, (s + 1) * B - m * 128)

    # close loop pools
    p4c_cm.__exit__(None, None, None)
    p4ps_cm.__exit__(None, None, None)
    psL_cm.__exit__(None, None, None)
    sbw_cm.__exit__(None, None, None)
    loopers_cm.__exit__(None, None, None)
    dram_cm.__exit__(None, None, None)
    glob_cm.__exit__(None, None, None)


def _prep_inputs(inputs):
    """Host-side sharding/layout prep. Returns list of per-core input dicts."""
    f32 = np.float32
    enc_out = np.asarray(inputs["enc_out"], f32)
    enc_h = np.asarray(inputs["enc_h"], f32)
    enc_c = np.asarray(inputs["enc_c"], f32)
    emb = np.asarray(inputs["embedding"], f32)
    attn_W = np.asarray(inputs["attn_W"], f32)
    attn_b = np.asarray(inputs["attn_b"], f32)
    vv = np.asarray(inputs["v"], f32)
    Wih0 = np.asarray(inputs["Wih0"], f32)
    Whh0 = np.asarray(inputs["Whh0"], f32)
    bih0 = np.asarray(inputs["bih0"], f32)
    bhh0 = np.asarray(inputs["bhh0"], f32)
    Wih1 = np.asarray(inputs["Wih1"], f32)
    Whh1 = np.asarray(inputs["Whh1"], f32)
    bih1 = np.asarray(inputs["bih1"], f32)
    bhh1 = np.asarray(inputs["bhh1"], f32)
    genW = np.asarray(inputs["genW"], f32)
    genb = np.asarray(inputs["genb"], f32)
    q = np.asarray(inputs["question"]).astype(np.int64)

    W1 = attn_W[:, :H]
    W2 = attn_W[:, H:]
    h0 = np.concatenate([enc_h[0], enc_h[1]], 1)  # (B, H) layer 0
    h1 = np.concatenate([enc_h[2], enc_h[3]], 1)  # layer 1
    c0 = np.concatenate([enc_c[0], enc_c[1]], 1)
    c1 = np.concatenate([enc_c[2], enc_c[3]], 1)

    # step-column order: col = w*16 + c2*2 + i  <->  global batch 4*c2 + 2*w + i
    col2gb = np.array(
        [4 * c2 + NBW * w + i for w in range(NW) for c2 in range(NCORES)
         for i in range(NBW)], dtype=np.int64)
    qperm = q[col2gb, :].T.reshape(NSAMP, 1).astype(np.int32)  # (s, col) order

    def bf(x):
        return np.ascontiguousarray(x).astype(BF)

    def f8(x):
        return np.ascontiguousarray(x).astype(F8)

    shared = {
        "w2t": bf(W2.T),
        "w1t": bf(W1.T),
        "attn_bias": bf(attn_b.reshape(1, H)),
        "vvec": bf(vv.reshape(H, 1)),
        "emb_tab": np.ascontiguousarray(emb),
        "qidx": qperm,
        "h0t_init": bf(h0[col2gb].T),
        "h1t_init": bf(h1[col2gb].T),
    }
    maps = []
    for c in range(NCORES):
        # local batches (in (w, i) order) = global ids for this core's slots
        my_gb = [4 * c + NBW * w + i for w in range(NW) for i in range(NBW)]
        sel_own = np.zeros((WB, NBW), f32)
        for i in range(NBW):
            sel_own[c * NBW + i, i] = 1.0
        # gate rows: order i|f|o|g (torch order is i,f,g,o -> pick blocks 0,1,3,2)
        gorder = [0, 1, 3, 2]
        rows = np.concatenate(
            [np.arange(g * H + c * GS, g * H + (c + 1) * GS) for g in gorder]
        )
        gsc = np.repeat([0.5, 0.5, 0.5, 1.0], GS)[:, None].astype(f32)
        wih0_s = Wih0[rows] * gsc  # (NG, E+H); i/f/o halved for 1-tanh cells
        wih0e = np.zeros((EP, NG), f32)
        wih0e[:E] = wih0_s[:, :E].T
        bias0 = (bih0 + bhh0)[rows] * gsc[:, 0]  # (NG,)
        bias_g0c = bias0.reshape(4, GS).T  # (GS, 4)
        vrows = slice(c * VPC, (c + 1) * VPC)
        genw_t = genW[vrows].T  # (H, VPC)
        genw_kp = np.ascontiguousarray(
            genw_t.reshape(KP, 2, 128, VPC).transpose(2, 0, 1, 3)
        )
        m = dict(shared)
        m.update({
            "enc_tr": bf(enc_out[my_gb].transpose(0, 2, 1)),
            "enc_f8": f8(enc_out[my_gb]),
            "wih0e": bf(wih0e),
            "sel_own": bf(sel_own),
            "bias_g0c": np.ascontiguousarray(bias_g0c),
            "wih0c": bf(wih0_s[:, E:].T),
            "whh0": bf((Whh0[rows] * gsc).T),
            "wih1": bf((Wih1[rows] * gsc).T),
            "whh1": bf((Whh1[rows] * gsc).T),
            "bias_g1": bf(((bih1 + bhh1)[rows] * gsc[:, 0]).reshape(1, NG)),
            "c0_l0": np.ascontiguousarray(c0[col2gb, c * GS:(c + 1) * GS].T),
            "c0_l1": np.ascontiguousarray(c1[col2gb, c * GS:(c + 1) * GS].T),
            "genw_kp": f8(genw_kp),
            "genb_v": bf(genb[vrows].reshape(1, VPC)),
        })
        maps.append(m)
    return maps


_CACHED = {}


def _get_compiled():
    if "nc" not in _CACHED:
        nc = bacc.Bacc(
            "TRN2", target_bir_lowering=False, debug=False,
            num_devices=1 if SIM1 else NCORES,
        )
        build(nc)
        nc.compile()
        _CACHED["nc"] = nc
    return _CACHED["nc"]


def run_cores(in_maps, **kw):
    nc = _get_compiled()
    return bass_utils.run_bass_kernel_spmd(nc, in_maps, list(range(NCORES)), **kw)


def kernel(**inputs):
    in_maps = _prep_inputs(inputs)
    res = run_cores(in_maps)
    parts = [res.results[c]["logp"] for c in range(NCORES)]
    full = np.concatenate(parts, axis=1)  # (NSAMP, V) in (s, col) order
    col2gb = np.array(
        [4 * c2 + NBW * w + i for w in range(NW) for c2 in range(NCORES)
         for i in range(NBW)], dtype=np.int64)
    full = full.reshape(S, B, V)
    out = np.empty((B, S, V), np.float32)
    out[col2gb, :, :] = full.transpose(1, 0, 2)
    return np.ascontiguousarray(out)


# revision 51
# speedup vs baseline: 2.8900x; 1.0120x over previous
"""Trainium2 Bass kernel for the attention-LSTM decoder (nn_Decoder).

Strategy (8 NeuronCores), v2 — restructured for the TRN2 cost model
(matmul cost ~ output free size; Act/DVE cost ~ free size; DVE 4x for
bf16 SBUF tensor_scalar):
  - Attention batch-sharded: each core owns B/8 = 4 batches. Energies are
    computed feature-major: DVE adds the per-step hidden bias (4x mode),
    Act does tanh in 2 big instructions per batch. Scores/softmax are
    transpose-free (ones-matmul partition reductions, unnormalized exp
    weights with context post-scaling).
  - LSTM tensor-parallel over gate rows (512/core, gate order i|f|o|g),
    everything feature-major so gate matmuls have small-N outputs and the
    cell state lives as (128, B) tiles. Batches advance in NW=4 waves of 8
    columns, giving 4 independent per-step pipelines whose exchange chains
    (issued alternately on the SP and GpSimd DMA queues) overlap the
    Act-bound tanh work; cell elementwise runs on GpSimd.
  - Vocab projection tensor-parallel over V (4000/core) in fp8 with
    DoubleRow (2 k-tiles per matmul, 0.5 cyc/row), interleaved into the
    recurrent loop per 128-sample mtile; per-mtile exp-sums, logsumexp
    AllReduce, subtract, and f32 output DMA all stream during the loop.
Dtypes: bf16 compute everywhere, fp32 PSUM + cell state, fp8e4m3 for the
ctx encoder operand and the vocab projection (genW and the h1 history).
"""
import os
import sys

sys.path.insert(0, "/opt/trn_rl_repo")

import numpy as np
import ml_dtypes

import concourse.bass as bass
import concourse.bacc as bacc
import concourse.mybir as mybir
import concourse.tile as tile
from concourse import bass_utils
from concourse.masks import make_identity

BF = ml_dtypes.bfloat16
F8 = ml_dtypes.float8_e4m3
dt = mybir.dt
AFT = mybir.ActivationFunctionType
ALU = mybir.AluOpType
PM = mybir.MatmulPerfMode

B, T, H, E, V, S = 32, 512, 1024, 300, 32000, 50
NCORES = 8
BPC = B // NCORES      # 4 batches per core
GS = H // NCORES       # 128-wide hidden slice per core
NG = 4 * GS            # 512 gate rows per core (i|f|o|g blocks of 128)
VPC = V // NCORES      # 4000 vocab rows per core
EP = 384               # padded embedding feature dim (3 k-tiles)
KE = EP // 128         # 3
KH = H // 128          # 8
KT = T // 128          # 4
KP = KH // 2           # 4 k-pairs for fp8 DoubleRow
NSAMP = S * B          # 1600
NW = int(os.environ.get("DECODER_NW", "4"))   # batch waves per step
WB = B // NW           # step-columns per wave
NBW = BPC // NW        # local batches per wave
S_EFF = int(os.environ.get("DECODER_STEPS", str(S)))
SIM1 = os.environ.get("DECODER_SIM", "0") == "1"
RG = [list(range(NCORES))]
SHARED = "Local" if SIM1 else "Shared"

# phase-4 sample tiles: 12 x 128 + 1 x 64
MTILES = [(m * 128, min(128, NSAMP - m * 128)) for m in range((NSAMP + 127) // 128)]
VC_N, VC_W = 16, 250     # vocab chunks for the projection psum
OC_N, OC_W = 8, 500      # output chunks for subtract + DMA


def _exchange(nc, eng, src_flat_ap, dst_bc_ap, stage_tile, shared_tile,
              reload_out_ap, reload_in_ap):
    """AllGather src (sbuf, (128, n)) into a consumer sbuf tile holding all
    8 cores' slices. SIM1 cost proxy: ONE fan-out DMA straight into the
    destination SBUF tile — the cost a remote-DMA-broadcast implementation
    would pay per exchange (same bytes x 8 destinations). Real build:
    stage to dram -> AllGather collective -> reload (collectives need dram).
    `eng` picks the DMA issue queue (SP / Pool)."""
    if SIM1:
        rows, cols = src_flat_ap.shape[0], src_flat_ap.shape[1]
        eng.dma_start(
            dst_bc_ap,
            src_flat_ap.unsqueeze(1).broadcast_to((rows, NCORES, cols)),
        )
    else:
        eng.dma_start(stage_tile[:], src_flat_ap)
        nc.gpsimd.collective_compute(
            "AllGather", mybir.AluOpType.bypass, replica_groups=RG,
            ins=[stage_tile[:].opt()], outs=[shared_tile[:].opt()],
        )
        eng.dma_start(reload_out_ap, reload_in_ap)


def _allreduce(nc, in_ap, out_ap):
    if SIM1:
        nc.gpsimd.dma_start(out_ap, in_ap)
    else:
        nc.gpsimd.collective_compute(
            "AllReduce", mybir.AluOpType.add, replica_groups=RG,
            ins=[in_ap.opt()], outs=[out_ap.opt()],
        )


def build(nc):
    di = {}

    def inp(name, shape, dtype):
        di[name] = nc.dram_tensor(name, list(shape), dtype, kind="ExternalInput")
        return di[name]

    inp("enc_tr", (BPC, H, T), dt.bfloat16)       # feature-major enc (p1b rhs)
    inp("enc_f8", (BPC, T, H), dt.float8e4)       # time-major enc (ctx lhsT)
    inp("w2t", (H, H), dt.bfloat16)
    inp("w1t", (H, H), dt.bfloat16)
    inp("attn_bias", (1, H), dt.bfloat16)
    inp("vvec", (H, 1), dt.bfloat16)
    inp("emb_tab", (V, E), dt.float32)
    inp("qidx", (NSAMP, 1), dt.int32)
    inp("wih0e", (EP, NG), dt.bfloat16)
    inp("bias_g0c", (GS, 4), dt.float32)
    inp("wih0c", (H, NG), dt.bfloat16)
    inp("whh0", (H, NG), dt.bfloat16)
    inp("wih1", (H, NG), dt.bfloat16)
    inp("whh1", (H, NG), dt.bfloat16)
    inp("bias_g1", (1, NG), dt.bfloat16)
    inp("sel_own", (WB, NBW), dt.bfloat16)
    inp("h0t_init", (H, B), dt.bfloat16)
    inp("h1t_init", (H, B), dt.bfloat16)
    inp("c0_l0", (GS, B), dt.float32)
    inp("c0_l1", (GS, B), dt.float32)
    inp("genw_kp", (128, KP, 2, VPC), dt.float8e4)
    inp("genb_v", (1, VPC), dt.bfloat16)
    logp = nc.dram_tensor("logp", [NSAMP, VPC], dt.float32, kind="ExternalOutput")

    with tile.TileContext(nc) as tc:
        _body(nc, tc, di, logp)
    return di


def _body(nc, tc, di, logp):
    glob_cm = tc.tile_pool(name="glob", bufs=1)
    glob = glob_cm.__enter__()
    dram_cm = tc.tile_pool(name="dram", bufs=1, space="DRAM")
    dram = dram_cm.__enter__()

    # ---- global constants ----
    id_bf = glob.tile([128, 128], dt.bfloat16, name="id_bf")
    id_f32 = glob.tile([128, 128], dt.float32, name="id_f32")
    make_identity(nc, id_bf[:])
    make_identity(nc, id_f32[:])
    ones_bf = glob.tile([1, 512], dt.bfloat16, name="ones_bf")
    nc.gpsimd.memset(ones_bf[:], 1.0)
    ones_col = glob.tile([128, 1], dt.bfloat16, name="ones_col")
    nc.gpsimd.memset(ones_col[:], 1.0)
    ones_f32 = glob.tile([1, 128], dt.float32, name="ones_f32")
    nc.gpsimd.memset(ones_f32[:], 1.0)
    sume = glob.tile([128, len(MTILES)], dt.float32, name="sume")
    sume8 = glob.tile([128, OC_N], dt.float32, name="sume8")

    # ---------------- persistent loop tensors ----------------
    loopers_cm = tc.tile_pool(name="loopers", bufs=1)
    loopers = loopers_cm.__enter__()

    w1t_sb = loopers.tile([128, KH, H], dt.bfloat16, name="w1t_sb")
    nc.sync.dma_start(w1t_sb[:], di["w1t"].ap().rearrange("(k p) h -> p k h", p=128))
    vvec_sb = loopers.tile([128, KH, 1], dt.bfloat16, name="vvec_sb")
    nc.sync.dma_start(vvec_sb[:], di["vvec"].ap().rearrange("(k p) o -> p k o", p=128))
    attn_b_sb = loopers.tile([1, H], dt.bfloat16, name="attn_b_sb")
    nc.sync.dma_start(attn_b_sb[:], di["attn_bias"].ap())
    wih0c_sb = loopers.tile([128, KH, NG], dt.bfloat16, name="wih0c_sb")
    nc.sync.dma_start(wih0c_sb[:], di["wih0c"].ap().rearrange("(k p) g -> p k g", p=128))
    whh0_sb = loopers.tile([128, KH, NG], dt.bfloat16, name="whh0_sb")
    nc.sync.dma_start(whh0_sb[:], di["whh0"].ap().rearrange("(k p) g -> p k g", p=128))
    wih1_sb = loopers.tile([128, KH, NG], dt.bfloat16, name="wih1_sb")
    nc.sync.dma_start(wih1_sb[:], di["wih1"].ap().rearrange("(k p) g -> p k g", p=128))
    whh1_sb = loopers.tile([128, KH, NG], dt.bfloat16, name="whh1_sb")
    nc.sync.dma_start(whh1_sb[:], di["whh1"].ap().rearrange("(k p) g -> p k g", p=128))
    bias_g1_sb = loopers.tile([1, NG], dt.bfloat16, name="bias_g1_sb")
    nc.sync.dma_start(bias_g1_sb[:], di["bias_g1"].ap())
    enc_f8_sb = loopers.tile([128, BPC, KT, H], dt.float8e4, name="enc_f8_sb")
    for _b in range(BPC):
        nc.sync.dma_start(
            enc_f8_sb[:, _b, :, :],
            di["enc_f8"].ap()[_b].rearrange("(k p) h -> p k h", p=128),
        )
    # hidden state ping-pong, factored (p, k, w, c, i)
    h0t_pp = [
        loopers.tile([128, KH, NW, NCORES, NBW], dt.bfloat16, name=f"h0t_pp{i}")
        for i in range(2)
    ]
    h1t_pp = [
        loopers.tile([128, KH, NW, NCORES, NBW], dt.bfloat16, name=f"h1t_pp{i}")
        for i in range(2)
    ]
    nc.sync.dma_start(
        h0t_pp[0][:],
        di["h0t_init"].ap().rearrange("(k p) (w c i) -> p k w c i", p=128, w=NW, c=NCORES),
    )
    nc.sync.dma_start(
        h1t_pp[0][:],
        di["h1t_init"].ap().rearrange("(k p) (w c i) -> p k w c i", p=128, w=NW, c=NCORES),
    )
    c_l0 = loopers.tile([128, B], dt.float32, name="c_l0")
    nc.sync.dma_start(c_l0[:], di["c0_l0"].ap())
    c_l1 = loopers.tile([128, B], dt.float32, name="c_l1")
    nc.sync.dma_start(c_l1[:], di["c0_l1"].ap())
    genw_sb = loopers.tile([128, KP, 2, VPC], dt.float8e4, name="genw_sb")
    nc.sync.dma_start(genw_sb[:], di["genw_kp"].ap())
    genb_sb = loopers.tile([1, VPC], dt.bfloat16, name="genb_sb")
    nc.sync.dma_start(genb_sb[:], di["genb_v"].ap())
    bias_g0c_sb = loopers.tile([128, 4], dt.float32, name="bias_g0c_sb")
    nc.sync.dma_start(bias_g0c_sb[:], di["bias_g0c"].ap())
    sel_own_sb = loopers.tile([WB, NBW], dt.bfloat16, name="sel_own_sb")
    nc.sync.dma_start(sel_own_sb[:], di["sel_own"].ap())

    encw2 = loopers.tile([128, BPC, KH, T], dt.bfloat16, name="encw2")
    g_emb = loopers.tile([128, 4, NSAMP], dt.bfloat16, name="g_emb")
    hcat = loopers.tile([128, KP, 2, NSAMP], dt.float8e4, name="hcat")

    # ---- phase 1: embedding gather/transpose, encW2, emb-gate precompute ----
    with tc.tile_pool(name="p1emb", bufs=1) as p1emb:
        emb_t = p1emb.tile([128, KE, NSAMP], dt.bfloat16, name="emb_t")
        nc.gpsimd.memset(emb_t[:], 0.0)
        wih0e_sb = p1emb.tile([128, KE, NG], dt.bfloat16, name="wih0e_sb")
        nc.sync.dma_start(
            wih0e_sb[:], di["wih0e"].ap().rearrange("(k p) g -> p k g", p=128)
        )

        # 1a: gather + transpose to feature-major
        with tc.tile_pool(name="p1e", bufs=3) as p1e, \
             tc.tile_pool(name="p1eps", bufs=3, space="PSUM") as p1eps:
            for (m0, mr) in MTILES:
                idx = p1e.tile([128, 1], dt.int32, tag="idx")
                nc.sync.dma_start(idx[:mr, :], di["qidx"].ap()[m0:m0 + mr, :])
                gath = p1e.tile([128, E], dt.float32, tag="gath")
                nc.gpsimd.indirect_dma_start(
                    out=gath[:mr, :],
                    out_offset=None,
                    in_=di["emb_tab"].ap(),
                    in_offset=bass.IndirectOffsetOnAxis(ap=idx[:mr, 0:1], axis=0),
                )
                for k in range(KE):
                    cw = min(128, E - k * 128)
                    ps = p1eps.tile([128, 128], dt.float32, tag="ps")
                    nc.tensor.transpose(
                        ps[:cw, :mr], gath[:mr, k * 128:k * 128 + cw],
                        id_f32[:mr, :mr]
                    )
                    nc.vector.tensor_copy(emb_t[:cw, k, m0:m0 + mr], ps[:cw, :mr])

        # 1b: encW2[b] feature-major = W2 @ enc[b].T
        with tc.tile_pool(name="p1w", bufs=1) as p1w, \
             tc.tile_pool(name="p1s", bufs=3) as p1s, \
             tc.tile_pool(name="p1ps", bufs=1, space="PSUM") as p1ps:
            w2t_sb = p1w.tile([128, KH, H], dt.bfloat16, name="w2t_sb")
            nc.sync.dma_start(
                w2t_sb[:], di["w2t"].ap().rearrange("(k p) h -> p k h", p=128)
            )
            for b in range(BPC):
                pss = [
                    p1ps.tile([128, T], dt.float32, tag=f"p1p{m}", name=f"p1p{b}_{m}")
                    for m in range(KH)
                ]
                for k in range(KH):
                    rhs = p1s.tile([128, T], dt.bfloat16, tag="rhs")
                    nc.sync.dma_start(
                        rhs[:], di["enc_tr"].ap()[b, k * 128:(k + 1) * 128, :]
                    )
                    for m in range(KH):
                        nc.tensor.matmul(
                            pss[m][:],
                            w2t_sb[:, k, m * 128:(m + 1) * 128],
                            rhs[:],
                            start=(k == 0),
                            stop=(k == KH - 1),
                        )
                for m in range(KH):
                    if m % 2 == 0:
                        nc.vector.tensor_copy(encw2[:, b, m, :], pss[m][:])
                    else:
                        nc.scalar.activation(encw2[:, b, m, :], pss[m][:], AFT.Copy)

        # 1c: embedding gate contributions (bias folded on the copy)
        with tc.tile_pool(name="p1gps", bufs=3, space="PSUM") as p1gps:
            for gt in range(4):
                for ch in range(4):
                    c0 = ch * 400
                    ps = p1gps.tile([128, 400], dt.float32, tag="gps")
                    for ke in range(KE):
                        nc.tensor.matmul(
                            ps[:],
                            wih0e_sb[:, ke, gt * 128:(gt + 1) * 128],
                            emb_t[:, ke, c0:c0 + 400],
                            start=(ke == 0),
                            stop=(ke == KE - 1),
                        )
                    nc.vector.tensor_scalar(
                        g_emb[:, gt, c0:c0 + 400], ps[:],
                        bias_g0c_sb[:, gt:gt + 1], None, ALU.add,
                    )

    # ---------------- phase 2: the recurrent loop ----------------
    sbw_cm = tc.tile_pool(name="sbw", bufs=2)
    sbw = sbw_cm.__enter__()
    psL_cm = tc.tile_pool(name="psL", bufs=1, space="PSUM")
    psL = psL_cm.__enter__()
    p4ps_cm = tc.tile_pool(name="p4ps", bufs=1, space="PSUM")
    p4ps = p4ps_cm.__enter__()
    p4c_cm = tc.tile_pool(name="p4c", bufs=1)
    p4c = p4c_cm.__enter__()

    def cell(gps, c_ap, tag):
        """gates i|f|o|g, i/f/o pre-scaled by 0.5 on the host so one tanh
        covers all four (sigmoid(x) = tanh(x/2)/2 + 0.5). Updates c_ap in place,
        returns h (128, WB) bf16. Elementwise runs on GpSimd (SBUF-only ops)
        to keep the DVE queue free for the energy bias-adds."""
        ifog = sbw.tile([128, 4, WB], dt.float32, tag=f"ifog{tag}")
        nc.scalar.activation(ifog[:], gps[:, :, :], AFT.Tanh)
        ifo = sbw.tile([128, 3, WB], dt.float32, tag=f"ifo{tag}")
        nc.gpsimd.tensor_scalar(ifo[:], ifog[:, 0:3, :], 0.5, 0.5, ALU.mult, ALU.add)
        t_fc = sbw.tile([128, WB], dt.float32, tag=f"tfc{tag}")
        nc.gpsimd.tensor_tensor(t_fc[:], ifo[:, 1, :], c_ap, op=ALU.mult)
        t_ig = sbw.tile([128, WB], dt.float32, tag=f"tig{tag}")
        nc.gpsimd.tensor_tensor(t_ig[:], ifo[:, 0, :], ifog[:, 3, :], op=ALU.mult)
        nc.gpsimd.tensor_tensor(c_ap, t_fc[:], t_ig[:], op=ALU.add)
        tc2 = sbw.tile([128, WB], dt.float32, tag=f"tc2{tag}")
        nc.scalar.activation(tc2[:], c_ap, AFT.Tanh)
        h = sbw.tile([128, WB], dt.bfloat16, tag=f"h{tag}")
        nc.gpsimd.tensor_tensor(h[:], ifo[:, 2, :], tc2[:], op=ALU.mult)
        return h

    def p4_burst(m, m0, mr):
        lg = p4c.tile([128, VPC], dt.bfloat16, tag="lgits", bufs=1)
        for vc in range(VC_N):
            v0 = vc * VC_W
            ps = p4ps.tile([128, 256], dt.float32, tag="p4p")
            nc.tensor.matmul(
                ps[:mr, :VC_W], ones_bf[0:1, :mr], genb_sb[0:1, v0:v0 + VC_W],
                start=True, stop=False,
            )
            for kp in range(KP):
                nc.tensor.matmul(
                    ps[:mr, :VC_W],
                    hcat[:, kp, :, m0:m0 + mr],
                    genw_sb[:, kp, :, v0:v0 + VC_W],
                    start=False, stop=(kp == KP - 1),
                    perf_mode=PM.DoubleRow,
                )
            nc.vector.tensor_copy(lg[:mr, v0:v0 + VC_W], ps[:mr, :VC_W])
        for ec in range(4):
            e0 = ec * 1000
            tmp = p4c.tile([128, 1000], dt.bfloat16, tag="etmp", bufs=2)
            nc.scalar.activation(
                tmp[:mr], lg[:mr, e0:e0 + 1000], AFT.Exp,
                accum_out=sume8[:mr, ec:ec + 1],
            )
        nc.vector.tensor_reduce(
            sume[:mr, m:m + 1], sume8[:mr, 0:4], axis=mybir.AxisListType.X,
            op=ALU.add,
        )
        bar_in = dram.tile([128, 1], dt.float32, tag="bar_in", bufs=2)
        nc.gpsimd.dma_start(bar_in[:mr], sume[:mr, m:m + 1])
        bar_out = dram.tile([128, 1], dt.float32, tag="bar_out", bufs=2,
                            addr_space=SHARED)
        _allreduce(nc, bar_in[:], bar_out[:])
        sg = p4c.tile([128, 1], dt.float32, tag="sg", bufs=2)
        nc.gpsimd.dma_start(sg[:], bar_out[:])
        lse = p4c.tile([128, 1], dt.float32, tag="lse", bufs=2)
        nc.scalar.activation(lse[:mr], sg[:mr], AFT.Ln)
        for oc in range(OC_N):
            o0 = oc * OC_W
            lpo = p4c.tile([128, OC_W], dt.float32, tag="lpo", bufs=2)
            nc.vector.tensor_scalar(
                lpo[:mr], lg[:mr, o0:o0 + OC_W], lse[:mr, 0:1], None, ALU.subtract
            )
            nc.gpsimd.dma_start(logp.ap()[m0:m0 + mr, o0:o0 + OC_W], lpo[:mr])

    for s in range(S_EFF):
        h1t_prev = h1t_pp[s % 2]
        h0t_prev = h0t_pp[s % 2]
        h1t_next = h1t_pp[(s + 1) % 2]
        h0t_next = h0t_pp[(s + 1) % 2]

        for w in range(NW):
            wc = slice(w * WB, (w + 1) * WB)
            # DMA issue queue for this wave's exchange chains: SP for wave 0,
            # GpSimd (SWDGE) for wave 1 — avoids cross-chain head-of-line
            # blocking on one sequencer.
            dq = nc.sync if w % 2 == 0 else nc.gpsimd

            # --- hw = W1 h1 + attn_b for this wave's 16 cols, then pick own
            #     2 cols via the per-core sel matrix (SPMD-safe selection) ---
            ps_hw = psL.tile([128, KH, WB], dt.float32, tag="ps_hwx", bufs=2)
            for m in range(KH):
                nc.tensor.matmul(
                    ps_hw[:, m, :],
                    attn_b_sb[0:1, m * 128:(m + 1) * 128],
                    ones_bf[0:1, 0:WB],
                    start=True, stop=False,
                )
                for k in range(KH):
                    nc.tensor.matmul(
                        ps_hw[:, m, :],
                        w1t_sb[:, k, m * 128:(m + 1) * 128],
                        h1t_prev[:, k, w],
                        start=False, stop=(k == KH - 1),
                    )
            hwf = sbw.tile([128, KH, WB], dt.bfloat16, tag="hwf", bufs=2)
            nc.vector.tensor_copy(hwf[:], ps_hw[:])
            ps_t = psL.tile([WB, KH, 128], dt.bfloat16, tag="ps_hwx", bufs=2)
            for m in range(KH):
                nc.tensor.transpose(ps_t[:WB, m, :], hwf[:, m, :], id_bf[:, :])
            hwT = sbw.tile([WB, KH, 128], dt.bfloat16, tag="hwT", bufs=2)
            nc.vector.tensor_copy(hwT[:], ps_t[:WB, :, :])
            ps_own = psL.tile([128, KH, NBW], dt.float32, tag="ps_hwx", bufs=2)
            for m in range(KH):
                nc.tensor.matmul(
                    ps_own[:, m, :], hwT[:WB, m, :], sel_own_sb[:],
                    start=True, stop=True,
                )
            hwt = sbw.tile([128, KH, NBW], dt.float32, tag="hwt", bufs=2)
            nc.vector.tensor_copy(hwt[:], ps_own[:])
            # --- attention for wave's 2 local batches ---
            ps_sc = psL.tile([128, KT, NBW], dt.float32, tag="ps_hwx", bufs=2)
            for i in range(NBW):
                lb = NBW * w + i
                for half in range(2):
                    k0 = half * 4
                    en = sbw.tile([128, 4, T], dt.bfloat16, tag="en", bufs=4)
                    for kk in range(4):
                        nc.vector.tensor_scalar(
                            en[:, kk, :], encw2[:, lb, k0 + kk, :],
                            hwt[:, k0 + kk, i:i + 1], None, ALU.add,
                        )
                    nc.scalar.activation(en[:], en[:], AFT.Tanh)
                    for tk in range(KT):
                        for kk in range(4):
                            nc.tensor.matmul(
                                ps_sc[:, tk, i:i + 1],
                                en[:, kk, tk * 128:(tk + 1) * 128],
                                vvec_sb[:, k0 + kk, :],
                                start=(k0 + kk == 0), stop=(k0 + kk == KH - 1),
                            )
            # --- softmax (unnormalized weights + reciprocal for ctx scale) ---
            exps = sbw.tile([128, KT, NBW], dt.bfloat16, tag="exps", bufs=2)
            nc.scalar.activation(exps[:], ps_sc[:], AFT.Exp)
            ps_den = psL.tile([128, NBW], dt.float32, tag="ps_small", bufs=1)
            for tk in range(KT):
                nc.tensor.matmul(
                    ps_den[0:1, :], ones_col[:, :], exps[:, tk, :],
                    start=(tk == 0), stop=(tk == KT - 1),
                )
            rec = sbw.tile([1, NBW], dt.float32, tag="rec", bufs=2)
            nc.vector.reciprocal(rec[:], ps_den[0:1, :])
            ps_rcb = psL.tile([128, NBW], dt.float32, tag="ps_small", bufs=1)
            nc.tensor.matmul(
                ps_rcb[:, :], ones_f32[:, :], rec[0:1, :], start=True, stop=True
            )
            recb = sbw.tile([128, NBW], dt.float32, tag="recb", bufs=2)
            nc.vector.tensor_copy(recb[:], ps_rcb[:])
            # --- context (feature-major, scaled by 1/den on copy) ---
            ctxw = sbw.tile([128, KH, NBW], dt.bfloat16, tag="ctxw", bufs=2)
            for i in range(NBW):
                lb = NBW * w + i
                ps_cx = psL.tile([128, KH], dt.float32, tag="ps_cx", bufs=1)
                for hk in range(KH):
                    for tk in range(KT):
                        nc.tensor.matmul(
                            ps_cx[:, hk:hk + 1],
                            enc_f8_sb[:, lb, tk, hk * 128:(hk + 1) * 128],
                            exps[:, tk, i:i + 1],
                            start=(tk == 0), stop=(tk == KT - 1),
                        )
                nc.vector.tensor_scalar(
                    ctxw[:, :, i], ps_cx[:], recb[:, i:i + 1], None, ALU.mult
                )
            # --- exchange ctx (AllGather over cores) ---
            bx_in = dram.tile([128, KH * NBW], dt.bfloat16, tag="bx_in", bufs=3)
            bx_out = dram.tile([NCORES * 128, KH * NBW], dt.bfloat16, tag="bx_out",
                               bufs=3, addr_space=SHARED)
            xt = sbw.tile([128, NCORES, KH, NBW], dt.bfloat16, tag="xt", bufs=2)
            _exchange(
                nc, dq, ctxw[:].rearrange("p k i -> p (k i)"),
                xt[:].rearrange("p c k i -> p c (k i)"), bx_in, bx_out,
                xt[:],
                bx_out[:].rearrange("(c p) (k i) -> p c k i", p=128, k=KH),
            )
            # --- LSTM layer 0 gates (N=16) ---
            ps_g0 = psL.tile([128, 4, WB], dt.float32, tag="ps_g", bufs=3)
            for gt in range(4):
                gsl = slice(gt * 128, (gt + 1) * 128)
                nc.tensor.matmul(
                    ps_g0[:, gt, :], id_bf[:],
                    g_emb[:, gt, s * B + w * WB:s * B + (w + 1) * WB],
                    start=True, stop=False,
                )
                for k in range(KH):
                    nc.tensor.matmul(
                        ps_g0[:, gt, :], whh0_sb[:, k, gsl],
                        h0t_prev[:, k, w], start=False, stop=False,
                    )
                for k in range(KH):
                    nc.tensor.matmul(
                        ps_g0[:, gt, :], wih0c_sb[:, k, gsl],
                        xt[:, :, k, :], start=False, stop=(k == KH - 1),
                    )
            h0n = cell(ps_g0, c_l0[:, wc], "l0")
            bh0_in = dram.tile([128, WB], dt.bfloat16, tag="bh0_in", bufs=3)
            bh0_out = dram.tile([NCORES * 128, WB], dt.bfloat16, tag="bh0_out",
                                bufs=3, addr_space=SHARED)
            _exchange(
                nc, dq, h0n[:],
                h0t_next[:, :, w].rearrange("p g c i -> p g (c i)"),
                bh0_in, bh0_out,
                h0t_next[:, :, w],
                bh0_out[:].rearrange("(g p) (c i) -> p g c i", p=128, c=NCORES),
            )
            # --- LSTM layer 1 gates ---
            ps_g1 = psL.tile([128, 4, WB], dt.float32, tag="ps_g", bufs=3)
            for gt in range(4):
                gsl = slice(gt * 128, (gt + 1) * 128)
                nc.tensor.matmul(
                    ps_g1[:, gt, :], bias_g1_sb[0:1, gsl], ones_bf[0:1, :WB],
                    start=True, stop=False,
                )
                for k in range(KH):
                    nc.tensor.matmul(
                        ps_g1[:, gt, :], whh1_sb[:, k, gsl],
                        h1t_prev[:, k, w], start=False, stop=False,
                    )
                for k in range(KH):
                    nc.tensor.matmul(
                        ps_g1[:, gt, :], wih1_sb[:, k, gsl],
                        h0t_next[:, k, w], start=False, stop=(k == KH - 1),
                    )
            h1n = cell(ps_g1, c_l1[:, wc], "l1")
            bh1_in = dram.tile([128, WB], dt.bfloat16, tag="bh1_in", bufs=3)
            bh1_out = dram.tile([NCORES * 128, WB], dt.bfloat16, tag="bh1_out",
                                bufs=3, addr_space=SHARED)
            _exchange(
                nc, dq, h1n[:],
                h1t_next[:, :, w].rearrange("p g c i -> p g (c i)"),
                bh1_in, bh1_out,
                h1t_next[:, :, w],
                bh1_out[:].rearrange("(g p) (c i) -> p g c i", p=128, c=NCORES),
            )
        # --- h1 history for the vocab projection (fp8, k-pair layout) ---
        for k in range(KH):
            nc.vector.tensor_copy(
                hcat[:, k // 2, k % 2, s * B:(s + 1) * B],
                h1t_next[:, k].rearrange("p w c i -> p (w c i)"),
            )
        # --- interleaved vocab projection bursts ---
        if (s + 1) % 4 == 0:
            m = (s + 1) // 4 - 1
            p4_burst(m, MTILES[m][0], MTILES[m][1])
        elif s == S_EFF - 1:
            m = ((s + 1) * B) // 128
            p4_burst(m, m * 128, (s + 1) * B - m * 128)

    # close loop pools
    p4c_cm.__exit__(None, None, None)
    p4ps_cm.__exit__(None, None, None)
    psL_cm.__exit__(None, None, None)
    sbw_cm.__exit__(None, None, None)
    loopers_cm.__exit__(None, None, None)
    dram_cm.__exit__(None, None, None)
    glob_cm.__exit__(None, None, None)


def _prep_inputs(inputs):
    """Host-side sharding/layout prep. Returns list of per-core input dicts."""
    f32 = np.float32
    enc_out = np.asarray(inputs["enc_out"], f32)
    enc_h = np.asarray(inputs["enc_h"], f32)
    enc_c = np.asarray(inputs["enc_c"], f32)
    emb = np.asarray(inputs["embedding"], f32)
    attn_W = np.asarray(inputs["attn_W"], f32)
    attn_b = np.asarray(inputs["attn_b"], f32)
    vv = np.asarray(inputs["v"], f32)
    Wih0 = np.asarray(inputs["Wih0"], f32)
    Whh0 = np.asarray(inputs["Whh0"], f32)
    bih0 = np.asarray(inputs["bih0"], f32)
    bhh0 = np.asarray(inputs["bhh0"], f32)
    Wih1 = np.asarray(inputs["Wih1"], f32)
    Whh1 = np.asarray(inputs["Whh1"], f32)
    bih1 = np.asarray(inputs["bih1"], f32)
    bhh1 = np.asarray(inputs["bhh1"], f32)
    genW = np.asarray(inputs["genW"], f32)
    genb = np.asarray(inputs["genb"], f32)
    q = np.asarray(inputs["question"]).astype(np.int64)

    W1 = attn_W[:, :H]
    W2 = attn_W[:, H:]
    h0 = np.concatenate([enc_h[0], enc_h[1]], 1)  # (B, H) layer 0
    h1 = np.concatenate([enc_h[2], enc_h[3]], 1)  # layer 1
    c0 = np.concatenate([enc_c[0], enc_c[1]], 1)
    c1 = np.concatenate([enc_c[2], enc_c[3]], 1)

    # step-column order: col = w*16 + c2*2 + i  <->  global batch 4*c2 + 2*w + i
    col2gb = np.array(
        [4 * c2 + NBW * w + i for w in range(NW) for c2 in range(NCORES)
         for i in range(NBW)], dtype=np.int64)
    qperm = q[col2gb, :].T.reshape(NSAMP, 1).astype(np.int32)  # (s, col) order

    def bf(x):
        return np.ascontiguousarray(x).astype(BF)

    def f8(x):
        return np.ascontiguousarray(x).astype(F8)

    shared = {
        "w2t": bf(W2.T),
        "w1t": bf(W1.T),
        "attn_bias": bf(attn_b.reshape(1, H)),
        "vvec": bf(vv.reshape(H, 1)),
        "emb_tab": np.ascontiguousarray(emb),
        "qidx": qperm,
        "h0t_init": bf(h0[col2gb].T),
        "h1t_init": bf(h1[col2gb].T),
    }
    maps = []
    for c in range(NCORES):
        # local batches (in (w, i) order) = global ids for this core's slots
        my_gb = [4 * c + NBW * w + i for w in range(NW) for i in range(NBW)]
        sel_own = np.zeros((WB, NBW), f32)
        for i in range(NBW):
            sel_own[c * NBW + i, i] = 1.0
        # gate rows: order i|f|o|g (torch order is i,f,g,o -> pick blocks 0,1,3,2)
        gorder = [0, 1, 3, 2]
        rows = np.concatenate(
            [np.arange(g * H + c * GS, g * H + (c + 1) * GS) for g in gorder]
        )
        gsc = np.repeat([0.5, 0.5, 0.5, 1.0], GS)[:, None].astype(f32)
        wih0_s = Wih0[rows] * gsc  # (NG, E+H); i/f/o halved for 1-tanh cells
        wih0e = np.zeros((EP, NG), f32)
        wih0e[:E] = wih0_s[:, :E].T
        bias0 = (bih0 + bhh0)[rows] * gsc[:, 0]  # (NG,)
        bias_g0c = bias0.reshape(4, GS).T  # (GS, 4)
        vrows = slice(c * VPC, (c + 1) * VPC)
        genw_t = genW[vrows].T  # (H, VPC)
        genw_kp = np.ascontiguousarray(
            genw_t.reshape(KP, 2, 128, VPC).transpose(2, 0, 1, 3)
        )
        m = dict(shared)
        m.update({
            "enc_tr": bf(enc_out[my_gb].transpose(0, 2, 1)),
            "enc_f8": f8(enc_out[my_gb]),
            "wih0e": bf(wih0e),
            "sel_own": bf(sel_own),
            "bias_g0c": np.ascontiguousarray(bias_g0c),
            "wih0c": bf(wih0_s[:, E:].T),
            "whh0": bf((Whh0[rows] * gsc).T),
            "wih1": bf((Wih1[rows] * gsc).T),
            "whh1": bf((Whh1[rows] * gsc).T),
            "bias_g1": bf(((bih1 + bhh1)[rows] * gsc[:, 0]).reshape(1, NG)),
            "c0_l0": np.ascontiguousarray(c0[col2gb, c * GS:(c + 1) * GS].T),
            "c0_l1": np.ascontiguousarray(c1[col2gb, c * GS:(c + 1) * GS].T),
            "genw_kp": f8(genw_kp),
            "genb_v": bf(genb[vrows].reshape(1, VPC)),
        })
        maps.append(m)
    return maps


_CACHED = {}


def _get_compiled():
    if "nc" not in _CACHED:
        nc = bacc.Bacc(
            "TRN2", target_bir_lowering=False, debug=False,
            num_devices=1 if SIM1 else NCORES,
        )
        build(nc)
        nc.compile()
        _CACHED["nc"] = nc
    return _CACHED["nc"]


def run_cores(in_maps, **kw):
    nc = _get_compiled()
    return bass_utils.run_bass_kernel_spmd(nc, in_maps, list(range(NCORES)), **kw)


def kernel(**inputs):
    in_maps = _prep_inputs(inputs)
    res = run_cores(in_maps)
    parts = [res.results[c]["logp"] for c in range(NCORES)]
    full = np.concatenate(parts, axis=1)  # (NSAMP, V) in (s, col) order
    col2gb = np.array(
        [4 * c2 + NBW * w + i for w in range(NW) for c2 in range(NCORES)
         for i in range(NBW)], dtype=np.int64)
    full = full.reshape(S, B, V)
    out = np.empty((B, S, V), np.float32)
    out[col2gb, :, :] = full.transpose(1, 0, 2)
    return np.ascontiguousarray(out)


# revision 52
# speedup vs baseline: 2.8990x; 1.0031x over previous
"""Trainium2 Bass kernel for the attention-LSTM decoder (nn_Decoder).

Strategy (8 NeuronCores), v2 — restructured for the TRN2 cost model
(matmul cost ~ output free size; Act/DVE cost ~ free size; DVE 4x for
bf16 SBUF tensor_scalar):
  - Attention batch-sharded: each core owns B/8 = 4 batches. Energies are
    computed feature-major: DVE adds the per-step hidden bias (4x mode),
    Act does tanh in 2 big instructions per batch. Scores/softmax are
    transpose-free (ones-matmul partition reductions, unnormalized exp
    weights with context post-scaling).
  - LSTM tensor-parallel over gate rows (512/core, gate order i|f|o|g),
    everything feature-major so gate matmuls have small-N outputs and the
    cell state lives as (128, B) tiles. Batches advance in NW=4 waves of 8
    columns, giving 4 independent per-step pipelines whose exchange chains
    (issued alternately on the SP and GpSimd DMA queues) overlap the
    Act-bound tanh work; cell elementwise runs on GpSimd.
  - Vocab projection tensor-parallel over V (4000/core) in fp8 with
    DoubleRow (2 k-tiles per matmul, 0.5 cyc/row), interleaved into the
    recurrent loop per 128-sample mtile; per-mtile exp-sums, logsumexp
    AllReduce, subtract, and f32 output DMA all stream during the loop.
Dtypes: bf16 compute everywhere, fp32 PSUM + cell state, fp8e4m3 for the
ctx encoder operand and the vocab projection (genW and the h1 history).
"""
import os
import sys

sys.path.insert(0, "/opt/trn_rl_repo")

import numpy as np
import ml_dtypes

import concourse.bass as bass
import concourse.bacc as bacc
import concourse.mybir as mybir
import concourse.tile as tile
from concourse import bass_utils
from concourse.masks import make_identity

BF = ml_dtypes.bfloat16
F8 = ml_dtypes.float8_e4m3
dt = mybir.dt
AFT = mybir.ActivationFunctionType
ALU = mybir.AluOpType
PM = mybir.MatmulPerfMode

B, T, H, E, V, S = 32, 512, 1024, 300, 32000, 50
NCORES = 8
BPC = B // NCORES      # 4 batches per core
GS = H // NCORES       # 128-wide hidden slice per core
NG = 4 * GS            # 512 gate rows per core (i|f|o|g blocks of 128)
VPC = V // NCORES      # 4000 vocab rows per core
EP = 384               # padded embedding feature dim (3 k-tiles)
KE = EP // 128         # 3
KH = H // 128          # 8
KT = T // 128          # 4
KP = KH // 2           # 4 k-pairs for fp8 DoubleRow
NSAMP = S * B          # 1600
NW = int(os.environ.get("DECODER_NW", "4"))   # batch waves per step
WB = B // NW           # step-columns per wave
NBW = BPC // NW        # local batches per wave
S_EFF = int(os.environ.get("DECODER_STEPS", str(S)))
SIM1 = os.environ.get("DECODER_SIM", "0") == "1"
RG = [list(range(NCORES))]
SHARED = "Local" if SIM1 else "Shared"

# phase-4 sample tiles: 12 x 128 + 1 x 64
MTILES = [(m * 128, min(128, NSAMP - m * 128)) for m in range((NSAMP + 127) // 128)]
VC_N, VC_W = 16, 250     # vocab chunks for the projection psum
OC_N, OC_W = 8, 500      # output chunks for subtract + DMA


def _exchange(nc, eng, src_flat_ap, dst_bc_ap, stage_tile, shared_tile,
              reload_out_ap, reload_in_ap):
    """AllGather src (sbuf, (128, n)) into a consumer sbuf tile holding all
    8 cores' slices. SIM1 cost proxy: ONE fan-out DMA straight into the
    destination SBUF tile — the cost a remote-DMA-broadcast implementation
    would pay per exchange (same bytes x 8 destinations). Real build:
    stage to dram -> AllGather collective -> reload (collectives need dram).
    `eng` picks the DMA issue queue (SP / Pool)."""
    if SIM1:
        rows, cols = src_flat_ap.shape[0], src_flat_ap.shape[1]
        eng.dma_start(
            dst_bc_ap,
            src_flat_ap.unsqueeze(1).broadcast_to((rows, NCORES, cols)),
        )
    else:
        eng.dma_start(stage_tile[:], src_flat_ap)
        nc.gpsimd.collective_compute(
            "AllGather", mybir.AluOpType.bypass, replica_groups=RG,
            ins=[stage_tile[:].opt()], outs=[shared_tile[:].opt()],
        )
        eng.dma_start(reload_out_ap, reload_in_ap)


def _allreduce(nc, in_ap, out_ap):
    if SIM1:
        nc.gpsimd.dma_start(out_ap, in_ap)
    else:
        nc.gpsimd.collective_compute(
            "AllReduce", mybir.AluOpType.add, replica_groups=RG,
            ins=[in_ap.opt()], outs=[out_ap.opt()],
        )


def build(nc):
    di = {}

    def inp(name, shape, dtype):
        di[name] = nc.dram_tensor(name, list(shape), dtype, kind="ExternalInput")
        return di[name]

    inp("enc_tr", (BPC, H, T), dt.bfloat16)       # feature-major enc (p1b rhs)
    inp("enc_f8", (BPC, T, H), dt.float8e4)       # time-major enc (ctx lhsT)
    inp("w2t", (H, H), dt.bfloat16)
    inp("w1t", (H, H), dt.bfloat16)
    inp("attn_bias", (1, H), dt.bfloat16)
    inp("vvec", (H, 1), dt.bfloat16)
    inp("emb_tab", (V, E), dt.float32)
    inp("qidx", (NSAMP, 1), dt.int32)
    inp("wih0e", (EP, NG), dt.bfloat16)
    inp("bias_g0c", (GS, 4), dt.float32)
    inp("wih0c", (H, NG), dt.bfloat16)
    inp("whh0", (H, NG), dt.bfloat16)
    inp("wih1", (H, NG), dt.bfloat16)
    inp("whh1", (H, NG), dt.bfloat16)
    inp("bias_g1", (1, NG), dt.bfloat16)
    inp("sel_own", (WB, NBW), dt.bfloat16)
    inp("h0t_init", (H, B), dt.bfloat16)
    inp("h1t_init", (H, B), dt.bfloat16)
    inp("c0_l0", (GS, B), dt.float32)
    inp("c0_l1", (GS, B), dt.float32)
    inp("genw_kp", (128, KP, 2, VPC), dt.float8e4)
    inp("genb_v", (1, VPC), dt.bfloat16)
    logp = nc.dram_tensor("logp", [NSAMP, VPC], dt.float32, kind="ExternalOutput")

    with tile.TileContext(nc) as tc:
        _body(nc, tc, di, logp)
    return di


def _body(nc, tc, di, logp):
    glob_cm = tc.tile_pool(name="glob", bufs=1)
    glob = glob_cm.__enter__()
    dram_cm = tc.tile_pool(name="dram", bufs=1, space="DRAM")
    dram = dram_cm.__enter__()

    # ---- global constants ----
    id_bf = glob.tile([128, 128], dt.bfloat16, name="id_bf")
    id_f32 = glob.tile([128, 128], dt.float32, name="id_f32")
    make_identity(nc, id_bf[:])
    make_identity(nc, id_f32[:])
    ones_bf = glob.tile([1, 512], dt.bfloat16, name="ones_bf")
    nc.gpsimd.memset(ones_bf[:], 1.0)
    ones_col = glob.tile([128, 1], dt.bfloat16, name="ones_col")
    nc.gpsimd.memset(ones_col[:], 1.0)
    ones_f32 = glob.tile([1, 128], dt.float32, name="ones_f32")
    nc.gpsimd.memset(ones_f32[:], 1.0)
    sume = glob.tile([128, len(MTILES)], dt.float32, name="sume")
    sume8 = glob.tile([128, OC_N], dt.float32, name="sume8")

    # ---------------- persistent loop tensors ----------------
    loopers_cm = tc.tile_pool(name="loopers", bufs=1)
    loopers = loopers_cm.__enter__()

    w1t_sb = loopers.tile([128, KH, H], dt.bfloat16, name="w1t_sb")
    vvec_sb = loopers.tile([128, KH, 1], dt.bfloat16, name="vvec_sb")
    nc.sync.dma_start(vvec_sb[:], di["vvec"].ap().rearrange("(k p) o -> p k o", p=128))
    attn_b_sb = loopers.tile([1, H], dt.bfloat16, name="attn_b_sb")
    nc.sync.dma_start(attn_b_sb[:], di["attn_bias"].ap())
    wih0c_sb = loopers.tile([128, KH, NG], dt.bfloat16, name="wih0c_sb")
    whh0_sb = loopers.tile([128, KH, NG], dt.bfloat16, name="whh0_sb")
    wih1_sb = loopers.tile([128, KH, NG], dt.bfloat16, name="wih1_sb")
    whh1_sb = loopers.tile([128, KH, NG], dt.bfloat16, name="whh1_sb")
    bias_g1_sb = loopers.tile([1, NG], dt.bfloat16, name="bias_g1_sb")
    nc.sync.dma_start(bias_g1_sb[:], di["bias_g1"].ap())
    enc_f8_sb = loopers.tile([128, BPC, KT, H], dt.float8e4, name="enc_f8_sb")
    # hidden state ping-pong, factored (p, k, w, c, i)
    h0t_pp = [
        loopers.tile([128, KH, NW, NCORES, NBW], dt.bfloat16, name=f"h0t_pp{i}")
        for i in range(2)
    ]
    h1t_pp = [
        loopers.tile([128, KH, NW, NCORES, NBW], dt.bfloat16, name=f"h1t_pp{i}")
        for i in range(2)
    ]
    nc.sync.dma_start(
        h0t_pp[0][:],
        di["h0t_init"].ap().rearrange("(k p) (w c i) -> p k w c i", p=128, w=NW, c=NCORES),
    )
    nc.sync.dma_start(
        h1t_pp[0][:],
        di["h1t_init"].ap().rearrange("(k p) (w c i) -> p k w c i", p=128, w=NW, c=NCORES),
    )
    c_l0 = loopers.tile([128, B], dt.float32, name="c_l0")
    nc.sync.dma_start(c_l0[:], di["c0_l0"].ap())
    c_l1 = loopers.tile([128, B], dt.float32, name="c_l1")
    nc.sync.dma_start(c_l1[:], di["c0_l1"].ap())
    genw_sb = loopers.tile([128, KP, 2, VPC], dt.float8e4, name="genw_sb")
    genb_sb = loopers.tile([1, VPC], dt.bfloat16, name="genb_sb")
    nc.sync.dma_start(genb_sb[:], di["genb_v"].ap())
    bias_g0c_sb = loopers.tile([128, 4], dt.float32, name="bias_g0c_sb")
    nc.sync.dma_start(bias_g0c_sb[:], di["bias_g0c"].ap())
    sel_own_sb = loopers.tile([WB, NBW], dt.bfloat16, name="sel_own_sb")
    nc.sync.dma_start(sel_own_sb[:], di["sel_own"].ap())

    encw2 = loopers.tile([128, BPC, KH, T], dt.bfloat16, name="encw2")
    g_emb = loopers.tile([128, 4, NSAMP], dt.bfloat16, name="g_emb")
    hcat = loopers.tile([128, KP, 2, NSAMP], dt.float8e4, name="hcat")

    # ---- phase 1: embedding gather/transpose, encW2, emb-gate precompute ----
    with tc.tile_pool(name="p1emb", bufs=1) as p1emb:
        emb_t = p1emb.tile([128, KE, NSAMP], dt.bfloat16, name="emb_t")
        nc.gpsimd.memset(emb_t[:], 0.0)
        wih0e_sb = p1emb.tile([128, KE, NG], dt.bfloat16, name="wih0e_sb")
        nc.sync.dma_start(
            wih0e_sb[:], di["wih0e"].ap().rearrange("(k p) g -> p k g", p=128)
        )

        # 1a: gather + transpose to feature-major
        with tc.tile_pool(name="p1e", bufs=3) as p1e, \
             tc.tile_pool(name="p1eps", bufs=3, space="PSUM") as p1eps:
            for (m0, mr) in MTILES:
                idx = p1e.tile([128, 1], dt.int32, tag="idx")
                nc.sync.dma_start(idx[:mr, :], di["qidx"].ap()[m0:m0 + mr, :])
                gath = p1e.tile([128, E], dt.float32, tag="gath")
                nc.gpsimd.indirect_dma_start(
                    out=gath[:mr, :],
                    out_offset=None,
                    in_=di["emb_tab"].ap(),
                    in_offset=bass.IndirectOffsetOnAxis(ap=idx[:mr, 0:1], axis=0),
                )
                for k in range(KE):
                    cw = min(128, E - k * 128)
                    ps = p1eps.tile([128, 128], dt.float32, tag="ps")
                    nc.tensor.transpose(
                        ps[:cw, :mr], gath[:mr, k * 128:k * 128 + cw],
                        id_f32[:mr, :mr]
                    )
                    nc.vector.tensor_copy(emb_t[:cw, k, m0:m0 + mr], ps[:cw, :mr])

        # 1b: encW2[b] feature-major = W2 @ enc[b].T
        with tc.tile_pool(name="p1w", bufs=1) as p1w, \
             tc.tile_pool(name="p1s", bufs=3) as p1s, \
             tc.tile_pool(name="p1ps", bufs=1, space="PSUM") as p1ps:
            w2t_sb = p1w.tile([128, KH, H], dt.bfloat16, name="w2t_sb")
            nc.sync.dma_start(
                w2t_sb[:], di["w2t"].ap().rearrange("(k p) h -> p k h", p=128)
            )
            for b in range(BPC):
                pss = [
                    p1ps.tile([128, T], dt.float32, tag=f"p1p{m}", name=f"p1p{b}_{m}")
                    for m in range(KH)
                ]
                for k in range(KH):
                    rhs = p1s.tile([128, T], dt.bfloat16, tag="rhs")
                    nc.sync.dma_start(
                        rhs[:], di["enc_tr"].ap()[b, k * 128:(k + 1) * 128, :]
                    )
                    for m in range(KH):
                        nc.tensor.matmul(
                            pss[m][:],
                            w2t_sb[:, k, m * 128:(m + 1) * 128],
                            rhs[:],
                            start=(k == 0),
                            stop=(k == KH - 1),
                        )
                for m in range(KH):
                    if m % 2 == 0:
                        nc.vector.tensor_copy(encw2[:, b, m, :], pss[m][:])
                    else:
                        nc.scalar.activation(encw2[:, b, m, :], pss[m][:], AFT.Copy)

        # 1c: embedding gate contributions (bias folded on the copy)
        with tc.tile_pool(name="p1gps", bufs=3, space="PSUM") as p1gps:
            for gt in range(4):
                for ch in range(4):
                    c0 = ch * 400
                    ps = p1gps.tile([128, 400], dt.float32, tag="gps")
                    for ke in range(KE):
                        nc.tensor.matmul(
                            ps[:],
                            wih0e_sb[:, ke, gt * 128:(gt + 1) * 128],
                            emb_t[:, ke, c0:c0 + 400],
                            start=(ke == 0),
                            stop=(ke == KE - 1),
                        )
                    nc.vector.tensor_scalar(
                        g_emb[:, gt, c0:c0 + 400], ps[:],
                        bias_g0c_sb[:, gt:gt + 1], None, ALU.add,
                    )


    # Deferred bulk loads: these are needed only once the recurrent loop
    # reaches them (w1t/enc_f8 at step 0 attention, LSTM weights at gates,
    # genw at the first phase-4 burst) — issuing them after the phase-1
    # operands keeps the DMA engines free for encW2/g_emb startup.
    nc.sync.dma_start(w1t_sb[:], di["w1t"].ap().rearrange("(k p) h -> p k h", p=128))
    for _b in range(BPC):
        nc.sync.dma_start(
            enc_f8_sb[:, _b, :, :],
            di["enc_f8"].ap()[_b].rearrange("(k p) h -> p k h", p=128),
        )
    nc.sync.dma_start(wih0c_sb[:], di["wih0c"].ap().rearrange("(k p) g -> p k g", p=128))
    nc.sync.dma_start(whh0_sb[:], di["whh0"].ap().rearrange("(k p) g -> p k g", p=128))
    nc.sync.dma_start(wih1_sb[:], di["wih1"].ap().rearrange("(k p) g -> p k g", p=128))
    nc.sync.dma_start(whh1_sb[:], di["whh1"].ap().rearrange("(k p) g -> p k g", p=128))
    nc.sync.dma_start(genw_sb[:], di["genw_kp"].ap())

    # ---------------- phase 2: the recurrent loop ----------------
    sbw_cm = tc.tile_pool(name="sbw", bufs=2)
    sbw = sbw_cm.__enter__()
    psL_cm = tc.tile_pool(name="psL", bufs=1, space="PSUM")
    psL = psL_cm.__enter__()
    p4ps_cm = tc.tile_pool(name="p4ps", bufs=1, space="PSUM")
    p4ps = p4ps_cm.__enter__()
    p4c_cm = tc.tile_pool(name="p4c", bufs=1)
    p4c = p4c_cm.__enter__()

    def cell(gps, c_ap, tag):
        """gates i|f|o|g, i/f/o pre-scaled by 0.5 on the host so one tanh
        covers all four (sigmoid(x) = tanh(x/2)/2 + 0.5). Updates c_ap in place,
        returns h (128, WB) bf16. Elementwise runs on GpSimd (SBUF-only ops)
        to keep the DVE queue free for the energy bias-adds."""
        ifog = sbw.tile([128, 4, WB], dt.float32, tag=f"ifog{tag}")
        nc.scalar.activation(ifog[:], gps[:, :, :], AFT.Tanh)
        ifo = sbw.tile([128, 3, WB], dt.float32, tag=f"ifo{tag}")
        nc.gpsimd.tensor_scalar(ifo[:], ifog[:, 0:3, :], 0.5, 0.5, ALU.mult, ALU.add)
        t_fc = sbw.tile([128, WB], dt.float32, tag=f"tfc{tag}")
        nc.gpsimd.tensor_tensor(t_fc[:], ifo[:, 1, :], c_ap, op=ALU.mult)
        t_ig = sbw.tile([128, WB], dt.float32, tag=f"tig{tag}")
        nc.gpsimd.tensor_tensor(t_ig[:], ifo[:, 0, :], ifog[:, 3, :], op=ALU.mult)
        nc.gpsimd.tensor_tensor(c_ap, t_fc[:], t_ig[:], op=ALU.add)
        tc2 = sbw.tile([128, WB], dt.float32, tag=f"tc2{tag}")
        nc.scalar.activation(tc2[:], c_ap, AFT.Tanh)
        h = sbw.tile([128, WB], dt.bfloat16, tag=f"h{tag}")
        nc.gpsimd.tensor_tensor(h[:], ifo[:, 2, :], tc2[:], op=ALU.mult)
        return h

    def p4_burst(m, m0, mr):
        lg = p4c.tile([128, VPC], dt.bfloat16, tag="lgits", bufs=1)
        for vc in range(VC_N):
            v0 = vc * VC_W
            ps = p4ps.tile([128, 256], dt.float32, tag="p4p")
            nc.tensor.matmul(
                ps[:mr, :VC_W], ones_bf[0:1, :mr], genb_sb[0:1, v0:v0 + VC_W],
                start=True, stop=False,
            )
            for kp in range(KP):
                nc.tensor.matmul(
                    ps[:mr, :VC_W],
                    hcat[:, kp, :, m0:m0 + mr],
                    genw_sb[:, kp, :, v0:v0 + VC_W],
                    start=False, stop=(kp == KP - 1),
                    perf_mode=PM.DoubleRow,
                )
            nc.vector.tensor_copy(lg[:mr, v0:v0 + VC_W], ps[:mr, :VC_W])
        for ec in range(4):
            e0 = ec * 1000
            tmp = p4c.tile([128, 1000], dt.bfloat16, tag="etmp", bufs=2)
            nc.scalar.activation(
                tmp[:mr], lg[:mr, e0:e0 + 1000], AFT.Exp,
                accum_out=sume8[:mr, ec:ec + 1],
            )
        nc.vector.tensor_reduce(
            sume[:mr, m:m + 1], sume8[:mr, 0:4], axis=mybir.AxisListType.X,
            op=ALU.add,
        )
        bar_in = dram.tile([128, 1], dt.float32, tag="bar_in", bufs=2)
        nc.gpsimd.dma_start(bar_in[:mr], sume[:mr, m:m + 1])
        bar_out = dram.tile([128, 1], dt.float32, tag="bar_out", bufs=2,
                            addr_space=SHARED)
        _allreduce(nc, bar_in[:], bar_out[:])
        sg = p4c.tile([128, 1], dt.float32, tag="sg", bufs=2)
        nc.gpsimd.dma_start(sg[:], bar_out[:])
        lse = p4c.tile([128, 1], dt.float32, tag="lse", bufs=2)
        nc.scalar.activation(lse[:mr], sg[:mr], AFT.Ln)
        for oc in range(OC_N):
            o0 = oc * OC_W
            lpo = p4c.tile([128, OC_W], dt.float32, tag="lpo", bufs=2)
            nc.vector.tensor_scalar(
                lpo[:mr], lg[:mr, o0:o0 + OC_W], lse[:mr, 0:1], None, ALU.subtract
            )
            nc.gpsimd.dma_start(logp.ap()[m0:m0 + mr, o0:o0 + OC_W], lpo[:mr])

    for s in range(S_EFF):
        h1t_prev = h1t_pp[s % 2]
        h0t_prev = h0t_pp[s % 2]
        h1t_next = h1t_pp[(s + 1) % 2]
        h0t_next = h0t_pp[(s + 1) % 2]

        for w in range(NW):
            wc = slice(w * WB, (w + 1) * WB)
            # DMA issue queue for this wave's exchange chains: SP for wave 0,
            # GpSimd (SWDGE) for wave 1 — avoids cross-chain head-of-line
            # blocking on one sequencer.
            dq = nc.sync if w % 2 == 0 else nc.gpsimd

            # --- hw = W1 h1 + attn_b for this wave's 16 cols, then pick own
            #     2 cols via the per-core sel matrix (SPMD-safe selection) ---
            ps_hw = psL.tile([128, KH, WB], dt.float32, tag="ps_hwx", bufs=2)
            for m in range(KH):
                nc.tensor.matmul(
                    ps_hw[:, m, :],
                    attn_b_sb[0:1, m * 128:(m + 1) * 128],
                    ones_bf[0:1, 0:WB],
                    start=True, stop=False,
                )
                for k in range(KH):
                    nc.tensor.matmul(
                        ps_hw[:, m, :],
                        w1t_sb[:, k, m * 128:(m + 1) * 128],
                        h1t_prev[:, k, w],
                        start=False, stop=(k == KH - 1),
                    )
            hwf = sbw.tile([128, KH, WB], dt.bfloat16, tag="hwf", bufs=2)
            nc.vector.tensor_copy(hwf[:], ps_hw[:])
            ps_t = psL.tile([WB, KH, 128], dt.bfloat16, tag="ps_hwx", bufs=2)
            for m in range(KH):
                nc.tensor.transpose(ps_t[:WB, m, :], hwf[:, m, :], id_bf[:, :])
            hwT = sbw.tile([WB, KH, 128], dt.bfloat16, tag="hwT", bufs=2)
            nc.vector.tensor_copy(hwT[:], ps_t[:WB, :, :])
            ps_own = psL.tile([128, KH, NBW], dt.float32, tag="ps_hwx", bufs=2)
            for m in range(KH):
                nc.tensor.matmul(
                    ps_own[:, m, :], hwT[:WB, m, :], sel_own_sb[:],
                    start=True, stop=True,
                )
            hwt = sbw.tile([128, KH, NBW], dt.float32, tag="hwt", bufs=2)
            nc.vector.tensor_copy(hwt[:], ps_own[:])
            # --- attention for wave's 2 local batches ---
            ps_sc = psL.tile([128, KT, NBW], dt.float32, tag="ps_hwx", bufs=2)
            for i in range(NBW):
                lb = NBW * w + i
                for half in range(2):
                    k0 = half * 4
                    en = sbw.tile([128, 4, T], dt.bfloat16, tag="en", bufs=4)
                    for kk in range(4):
                        nc.vector.tensor_scalar(
                            en[:, kk, :], encw2[:, lb, k0 + kk, :],
                            hwt[:, k0 + kk, i:i + 1], None, ALU.add,
                        )
                    nc.scalar.activation(en[:], en[:], AFT.Tanh)
                    for tk in range(KT):
                        for kk in range(4):
                            nc.tensor.matmul(
                                ps_sc[:, tk, i:i + 1],
                                en[:, kk, tk * 128:(tk + 1) * 128],
                                vvec_sb[:, k0 + kk, :],
                                start=(k0 + kk == 0), stop=(k0 + kk == KH - 1),
                            )
            # --- softmax (unnormalized weights + reciprocal for ctx scale) ---
            exps = sbw.tile([128, KT, NBW], dt.bfloat16, tag="exps", bufs=2)
            nc.scalar.activation(exps[:], ps_sc[:], AFT.Exp)
            ps_den = psL.tile([128, NBW], dt.float32, tag="ps_small", bufs=1)
            for tk in range(KT):
                nc.tensor.matmul(
                    ps_den[0:1, :], ones_col[:, :], exps[:, tk, :],
                    start=(tk == 0), stop=(tk == KT - 1),
                )
            rec = sbw.tile([1, NBW], dt.float32, tag="rec", bufs=2)
            nc.vector.reciprocal(rec[:], ps_den[0:1, :])
            ps_rcb = psL.tile([128, NBW], dt.float32, tag="ps_small", bufs=1)
            nc.tensor.matmul(
                ps_rcb[:, :], ones_f32[:, :], rec[0:1, :], start=True, stop=True
            )
            recb = sbw.tile([128, NBW], dt.float32, tag="recb", bufs=2)
            nc.vector.tensor_copy(recb[:], ps_rcb[:])
            # --- context (feature-major, scaled by 1/den on copy) ---
            ctxw = sbw.tile([128, KH, NBW], dt.bfloat16, tag="ctxw", bufs=2)
            for i in range(NBW):
                lb = NBW * w + i
                ps_cx = psL.tile([128, KH], dt.float32, tag="ps_cx", bufs=1)
                for hk in range(KH):
                    for tk in range(KT):
                        nc.tensor.matmul(
                            ps_cx[:, hk:hk + 1],
                            enc_f8_sb[:, lb, tk, hk * 128:(hk + 1) * 128],
                            exps[:, tk, i:i + 1],
                            start=(tk == 0), stop=(tk == KT - 1),
                        )
                nc.vector.tensor_scalar(
                    ctxw[:, :, i], ps_cx[:], recb[:, i:i + 1], None, ALU.mult
                )
            # --- exchange ctx (AllGather over cores) ---
            bx_in = dram.tile([128, KH * NBW], dt.bfloat16, tag="bx_in", bufs=3)
            bx_out = dram.tile([NCORES * 128, KH * NBW], dt.bfloat16, tag="bx_out",
                               bufs=3, addr_space=SHARED)
            xt = sbw.tile([128, NCORES, KH, NBW], dt.bfloat16, tag="xt", bufs=2)
            _exchange(
                nc, dq, ctxw[:].rearrange("p k i -> p (k i)"),
                xt[:].rearrange("p c k i -> p c (k i)"), bx_in, bx_out,
                xt[:],
                bx_out[:].rearrange("(c p) (k i) -> p c k i", p=128, k=KH),
            )
            # --- LSTM layer 0 gates (N=16) ---
            ps_g0 = psL.tile([128, 4, WB], dt.float32, tag="ps_g", bufs=3)
            for gt in range(4):
                gsl = slice(gt * 128, (gt + 1) * 128)
                nc.tensor.matmul(
                    ps_g0[:, gt, :], id_bf[:],
                    g_emb[:, gt, s * B + w * WB:s * B + (w + 1) * WB],
                    start=True, stop=False,
                )
                for k in range(KH):
                    nc.tensor.matmul(
                        ps_g0[:, gt, :], whh0_sb[:, k, gsl],
                        h0t_prev[:, k, w], start=False, stop=False,
                    )
                for k in range(KH):
                    nc.tensor.matmul(
                        ps_g0[:, gt, :], wih0c_sb[:, k, gsl],
                        xt[:, :, k, :], start=False, stop=(k == KH - 1),
                    )
            h0n = cell(ps_g0, c_l0[:, wc], "l0")
            bh0_in = dram.tile([128, WB], dt.bfloat16, tag="bh0_in", bufs=3)
            bh0_out = dram.tile([NCORES * 128, WB], dt.bfloat16, tag="bh0_out",
                                bufs=3, addr_space=SHARED)
            _exchange(
                nc, dq, h0n[:],
                h0t_next[:, :, w].rearrange("p g c i -> p g (c i)"),
                bh0_in, bh0_out,
                h0t_next[:, :, w],
                bh0_out[:].rearrange("(g p) (c i) -> p g c i", p=128, c=NCORES),
            )
            # --- LSTM layer 1 gates ---
            ps_g1 = psL.tile([128, 4, WB], dt.float32, tag="ps_g", bufs=3)
            for gt in range(4):
                gsl = slice(gt * 128, (gt + 1) * 128)
                nc.tensor.matmul(
                    ps_g1[:, gt, :], bias_g1_sb[0:1, gsl], ones_bf[0:1, :WB],
                    start=True, stop=False,
                )
                for k in range(KH):
                    nc.tensor.matmul(
                        ps_g1[:, gt, :], whh1_sb[:, k, gsl],
                        h1t_prev[:, k, w], start=False, stop=False,
                    )
                for k in range(KH):
                    nc.tensor.matmul(
                        ps_g1[:, gt, :], wih1_sb[:, k, gsl],
                        h0t_next[:, k, w], start=False, stop=(k == KH - 1),
                    )
            h1n = cell(ps_g1, c_l1[:, wc], "l1")
            bh1_in = dram.tile([128, WB], dt.bfloat16, tag="bh1_in", bufs=3)
            bh1_out = dram.tile([NCORES * 128, WB], dt.bfloat16, tag="bh1_out",
                                bufs=3, addr_space=SHARED)
            _exchange(
                nc, dq, h1n[:],
                h1t_next[:, :, w].rearrange("p g c i -> p g (c i)"),
                bh1_in, bh1_out,
                h1t_next[:, :, w],
                bh1_out[:].rearrange("(g p) (c i) -> p g c i", p=128, c=NCORES),
            )
        # --- h1 history for the vocab projection (fp8, k-pair layout) ---
        for k in range(KH):
            nc.vector.tensor_copy(
                hcat[:, k // 2, k % 2, s * B:(s + 1) * B],
                h1t_next[:, k].rearrange("p w c i -> p (w c i)"),
            )
        # --- interleaved vocab projection bursts ---
        if (s + 1) % 4 == 0:
            m = (s + 1) // 4 - 1
            p4_burst(m, MTILES[m][0], MTILES[m][1])
        elif s == S_EFF - 1:
            m = ((s + 1) * B) // 128
            p4_burst(m, m * 128, (s + 1) * B - m * 128)

    # close loop pools
    p4c_cm.__exit__(None, None, None)
    p4ps_cm.__exit__(None, None, None)
    psL_cm.__exit__(None, None, None)
    sbw_cm.__exit__(None, None, None)
    loopers_cm.__exit__(None, None, None)
    dram_cm.__exit__(None, None, None)
    glob_cm.__exit__(None, None, None)


def _prep_inputs(inputs):
    """Host-side sharding/layout prep. Returns list of per-core input dicts."""
    f32 = np.float32
    enc_out = np.asarray(inputs["enc_out"], f32)
    enc_h = np.asarray(inputs["enc_h"], f32)
    enc_c = np.asarray(inputs["enc_c"], f32)
    emb = np.asarray(inputs["embedding"], f32)
    attn_W = np.asarray(inputs["attn_W"], f32)
    attn_b = np.asarray(inputs["attn_b"], f32)
    vv = np.asarray(inputs["v"], f32)
    Wih0 = np.asarray(inputs["Wih0"], f32)
    Whh0 = np.asarray(inputs["Whh0"], f32)
    bih0 = np.asarray(inputs["bih0"], f32)
    bhh0 = np.asarray(inputs["bhh0"], f32)
    Wih1 = np.asarray(inputs["Wih1"], f32)
    Whh1 = np.asarray(inputs["Whh1"], f32)
    bih1 = np.asarray(inputs["bih1"], f32)
    bhh1 = np.asarray(inputs["bhh1"], f32)
    genW = np.asarray(inputs["genW"], f32)
    genb = np.asarray(inputs["genb"], f32)
    q = np.asarray(inputs["question"]).astype(np.int64)

    W1 = attn_W[:, :H]
    W2 = attn_W[:, H:]
    h0 = np.concatenate([enc_h[0], enc_h[1]], 1)  # (B, H) layer 0
    h1 = np.concatenate([enc_h[2], enc_h[3]], 1)  # layer 1
    c0 = np.concatenate([enc_c[0], enc_c[1]], 1)
    c1 = np.concatenate([enc_c[2], enc_c[3]], 1)

    # step-column order: col = w*16 + c2*2 + i  <->  global batch 4*c2 + 2*w + i
    col2gb = np.array(
        [4 * c2 + NBW * w + i for w in range(NW) for c2 in range(NCORES)
         for i in range(NBW)], dtype=np.int64)
    qperm = q[col2gb, :].T.reshape(NSAMP, 1).astype(np.int32)  # (s, col) order

    def bf(x):
        return np.ascontiguousarray(x).astype(BF)

    def f8(x):
        return np.ascontiguousarray(x).astype(F8)

    shared = {
        "w2t": bf(W2.T),
        "w1t": bf(W1.T),
        "attn_bias": bf(attn_b.reshape(1, H)),
        "vvec": bf(vv.reshape(H, 1)),
        "emb_tab": np.ascontiguousarray(emb),
        "qidx": qperm,
        "h0t_init": bf(h0[col2gb].T),
        "h1t_init": bf(h1[col2gb].T),
    }
    maps = []
    for c in range(NCORES):
        # local batches (in (w, i) order) = global ids for this core's slots
        my_gb = [4 * c + NBW * w + i for w in range(NW) for i in range(NBW)]
        sel_own = np.zeros((WB, NBW), f32)
        for i in range(NBW):
            sel_own[c * NBW + i, i] = 1.0
        # gate rows: order i|f|o|g (torch order is i,f,g,o -> pick blocks 0,1,3,2)
        gorder = [0, 1, 3, 2]
        rows = np.concatenate(
            [np.arange(g * H + c * GS, g * H + (c + 1) * GS) for g in gorder]
        )
        gsc = np.repeat([0.5, 0.5, 0.5, 1.0], GS)[:, None].astype(f32)
        wih0_s = Wih0[rows] * gsc  # (NG, E+H); i/f/o halved for 1-tanh cells
        wih0e = np.zeros((EP, NG), f32)
        wih0e[:E] = wih0_s[:, :E].T
        bias0 = (bih0 + bhh0)[rows] * gsc[:, 0]  # (NG,)
        bias_g0c = bias0.reshape(4, GS).T  # (GS, 4)
        vrows = slice(c * VPC, (c + 1) * VPC)
        genw_t = genW[vrows].T  # (H, VPC)
        genw_kp = np.ascontiguousarray(
            genw_t.reshape(KP, 2, 128, VPC).transpose(2, 0, 1, 3)
        )
        m = dict(shared)
        m.update({
            "enc_tr": bf(enc_out[my_gb].transpose(0, 2, 1)),
            "enc_f8": f8(enc_out[my_gb]),
            "wih0e": bf(wih0e),
            "sel_own": bf(sel_own),
            "bias_g0c": np.ascontiguousarray(bias_g0c),
            "wih0c": bf(wih0_s[:, E:].T),
            "whh0": bf((Whh0[rows] * gsc).T),
            "wih1": bf((Wih1[rows] * gsc).T),
            "whh1": bf((Whh1[rows] * gsc).T),
            "bias_g1": bf(((bih1 + bhh1)[rows] * gsc[:, 0]).reshape(1, NG)),
            "c0_l0": np.ascontiguousarray(c0[col2gb, c * GS:(c + 1) * GS].T),
            "c0_l1": np.ascontiguousarray(c1[col2gb, c * GS:(c + 1) * GS].T),
            "genw_kp": f8(genw_kp),
            "genb_v": bf(genb[vrows].reshape(1, VPC)),
        })
        maps.append(m)
    return maps


_CACHED = {}


def _get_compiled():
    if "nc" not in _CACHED:
        nc = bacc.Bacc(
            "TRN2", target_bir_lowering=False, debug=False,
            num_devices=1 if SIM1 else NCORES,
        )
        build(nc)
        nc.compile()
        _CACHED["nc"] = nc
    return _CACHED["nc"]


def run_cores(in_maps, **kw):
    nc = _get_compiled()
    return bass_utils.run_bass_kernel_spmd(nc, in_maps, list(range(NCORES)), **kw)


def kernel(**inputs):
    in_maps = _prep_inputs(inputs)
    res = run_cores(in_maps)
    parts = [res.results[c]["logp"] for c in range(NCORES)]
    full = np.concatenate(parts, axis=1)  # (NSAMP, V) in (s, col) order
    col2gb = np.array(
        [4 * c2 + NBW * w + i for w in range(NW) for c2 in range(NCORES)
         for i in range(NBW)], dtype=np.int64)
    full = full.reshape(S, B, V)
    out = np.empty((B, S, V), np.float32)
    out[col2gb, :, :] = full.transpose(1, 0, 2)
    return np.ascontiguousarray(out)


# revision 59
# speedup vs baseline: 2.9099x; 1.0038x over previous
"""Trainium2 Bass kernel for the attention-LSTM decoder (nn_Decoder).

Strategy (8 NeuronCores), v2 — restructured for the TRN2 cost model
(matmul cost ~ output free size; Act/DVE cost ~ free size; DVE 4x for
bf16 SBUF tensor_scalar):
  - Attention batch-sharded: each core owns B/8 = 4 batches. Energies are
    computed feature-major: DVE adds the per-step hidden bias (4x mode),
    Act does tanh in 2 big instructions per batch. Scores/softmax are
    transpose-free (ones-matmul partition reductions, unnormalized exp
    weights with context post-scaling).
  - LSTM tensor-parallel over gate rows (512/core, gate order i|f|o|g),
    everything feature-major so gate matmuls have small-N outputs and the
    cell state lives as (128, B) tiles. Batches advance in NW=4 waves of 8
    columns, giving 4 independent per-step pipelines whose exchange chains
    (issued alternately on the SP and GpSimd DMA queues) overlap the
    Act-bound tanh work; cell elementwise runs on GpSimd.
  - Vocab projection tensor-parallel over V (4000/core) in fp8 with
    DoubleRow (2 k-tiles per matmul, 0.5 cyc/row), interleaved into the
    recurrent loop per 128-sample mtile; per-mtile exp-sums, logsumexp
    AllReduce, subtract, and f32 output DMA all stream during the loop.
Dtypes: bf16 compute everywhere, fp32 PSUM + cell state, fp8e4m3 for the
ctx encoder operand and the vocab projection (genW and the h1 history).
"""
import os
import sys

sys.path.insert(0, "/opt/trn_rl_repo")

import numpy as np
import ml_dtypes

import concourse.bass as bass
import concourse.bacc as bacc
import concourse.mybir as mybir
import concourse.tile as tile
from concourse import bass_utils
from concourse.masks import make_identity

BF = ml_dtypes.bfloat16
F8 = ml_dtypes.float8_e4m3
dt = mybir.dt
AFT = mybir.ActivationFunctionType
ALU = mybir.AluOpType
PM = mybir.MatmulPerfMode

B, T, H, E, V, S = 32, 512, 1024, 300, 32000, 50
NCORES = 8
BPC = B // NCORES      # 4 batches per core
GS = H // NCORES       # 128-wide hidden slice per core
NG = 4 * GS            # 512 gate rows per core (i|f|o|g blocks of 128)
VPC = V // NCORES      # 4000 vocab rows per core
EP = 384               # padded embedding feature dim (3 k-tiles)
KE = EP // 128         # 3
KH = H // 128          # 8
KT = T // 128          # 4
KP = KH // 2           # 4 k-pairs for fp8 DoubleRow
NSAMP = S * B          # 1600
NW = int(os.environ.get("DECODER_NW", "4"))   # batch waves per step
WB = B // NW           # step-columns per wave
NBW = BPC // NW        # local batches per wave
S_EFF = int(os.environ.get("DECODER_STEPS", str(S)))
SIM1 = os.environ.get("DECODER_SIM", "0") == "1"
RG = [list(range(NCORES))]
SHARED = "Local" if SIM1 else "Shared"

# phase-4 sample tiles: 12 x 128 + 1 x 64
MTILES = [(m * 128, min(128, NSAMP - m * 128)) for m in range((NSAMP + 127) // 128)]
VC_N, VC_W = 16, 250     # vocab chunks for the projection psum
OC_N, OC_W = 8, 500      # output chunks for subtract + DMA


def _exchange(nc, eng, src_flat_ap, dst_bc_ap, stage_tile, shared_tile,
              reload_out_ap, reload_in_ap):
    """AllGather src (sbuf, (128, n)) into a consumer sbuf tile holding all
    8 cores' slices. SIM1 cost proxy: ONE fan-out DMA straight into the
    destination SBUF tile — the cost a remote-DMA-broadcast implementation
    would pay per exchange (same bytes x 8 destinations). Real build:
    stage to dram -> AllGather collective -> reload (collectives need dram).
    `eng` picks the DMA issue queue (SP / Pool)."""
    if SIM1:
        rows, cols = src_flat_ap.shape[0], src_flat_ap.shape[1]
        eng.dma_start(
            dst_bc_ap,
            src_flat_ap.unsqueeze(1).broadcast_to((rows, NCORES, cols)),
        )
    else:
        eng.dma_start(stage_tile[:], src_flat_ap)
        nc.gpsimd.collective_compute(
            "AllGather", mybir.AluOpType.bypass, replica_groups=RG,
            ins=[stage_tile[:].opt()], outs=[shared_tile[:].opt()],
        )
        eng.dma_start(reload_out_ap, reload_in_ap)


def _allreduce(nc, in_ap, out_ap):
    if SIM1:
        nc.gpsimd.dma_start(out_ap, in_ap)
    else:
        nc.gpsimd.collective_compute(
            "AllReduce", mybir.AluOpType.add, replica_groups=RG,
            ins=[in_ap.opt()], outs=[out_ap.opt()],
        )


def build(nc):
    di = {}

    def inp(name, shape, dtype):
        di[name] = nc.dram_tensor(name, list(shape), dtype, kind="ExternalInput")
        return di[name]

    inp("enc_tr", (BPC, H, T), dt.bfloat16)       # feature-major enc (p1b rhs)
    inp("enc_f8", (BPC, T, H), dt.float8e4)       # time-major enc (ctx lhsT)
    inp("w2t", (H, H), dt.bfloat16)
    inp("w1t", (H, H), dt.bfloat16)
    inp("attn_bias", (1, H), dt.bfloat16)
    inp("vvec", (H, 1), dt.bfloat16)
    inp("emb_tab", (V, E), dt.float32)
    inp("qidx", (NSAMP, 1), dt.int32)
    inp("wih0e", (EP, NG), dt.bfloat16)
    inp("bias_g0c", (GS, 4), dt.float32)
    inp("wih0c", (H, NG), dt.bfloat16)
    inp("whh0", (H, NG), dt.bfloat16)
    inp("wih1", (H, NG), dt.bfloat16)
    inp("whh1", (H, NG), dt.bfloat16)
    inp("bias_g1", (1, NG), dt.bfloat16)
    inp("sel_own", (WB, NBW), dt.bfloat16)
    inp("h0t_init", (H, B), dt.bfloat16)
    inp("h1t_init", (H, B), dt.bfloat16)
    inp("c0_l0", (GS, B), dt.float32)
    inp("c0_l1", (GS, B), dt.float32)
    inp("genw_kp", (128, KP, 2, VPC), dt.float8e4)
    inp("genb_v", (1, VPC), dt.bfloat16)
    logp = nc.dram_tensor("logp", [NSAMP, VPC], dt.float32, kind="ExternalOutput")

    with tile.TileContext(nc) as tc:
        _body(nc, tc, di, logp)
    return di


def _body(nc, tc, di, logp):
    glob_cm = tc.tile_pool(name="glob", bufs=1)
    glob = glob_cm.__enter__()
    dram_cm = tc.tile_pool(name="dram", bufs=1, space="DRAM")
    dram = dram_cm.__enter__()

    # ---- global constants ----
    id_bf = glob.tile([128, 128], dt.bfloat16, name="id_bf")
    id_f32 = glob.tile([128, 128], dt.float32, name="id_f32")
    make_identity(nc, id_bf[:])
    make_identity(nc, id_f32[:])
    ones_bf = glob.tile([1, 512], dt.bfloat16, name="ones_bf")
    nc.gpsimd.memset(ones_bf[:], 1.0)
    ones_col = glob.tile([128, 1], dt.bfloat16, name="ones_col")
    nc.gpsimd.memset(ones_col[:], 1.0)
    ones_f32 = glob.tile([1, 128], dt.float32, name="ones_f32")
    nc.gpsimd.memset(ones_f32[:], 1.0)
    sume = glob.tile([128, len(MTILES)], dt.float32, name="sume")
    sume8 = glob.tile([128, OC_N], dt.float32, name="sume8")

    # ---------------- persistent loop tensors ----------------
    loopers_cm = tc.tile_pool(name="loopers", bufs=1)
    loopers = loopers_cm.__enter__()

    w1t_sb = loopers.tile([128, KH, H], dt.bfloat16, name="w1t_sb")
    vvec_sb = loopers.tile([128, KH, 1], dt.bfloat16, name="vvec_sb")
    nc.sync.dma_start(vvec_sb[:], di["vvec"].ap().rearrange("(k p) o -> p k o", p=128))
    attn_b_sb = loopers.tile([1, H], dt.bfloat16, name="attn_b_sb")
    nc.sync.dma_start(attn_b_sb[:], di["attn_bias"].ap())
    wih0c_sb = loopers.tile([128, KH, NG], dt.bfloat16, name="wih0c_sb")
    whh0_sb = loopers.tile([128, KH, NG], dt.bfloat16, name="whh0_sb")
    wih1_sb = loopers.tile([128, KH, NG], dt.bfloat16, name="wih1_sb")
    whh1_sb = loopers.tile([128, KH, NG], dt.bfloat16, name="whh1_sb")
    bias_g1_sb = loopers.tile([1, NG], dt.bfloat16, name="bias_g1_sb")
    nc.sync.dma_start(bias_g1_sb[:], di["bias_g1"].ap())
    enc_f8_sb = loopers.tile([128, BPC, KT, H], dt.float8e4, name="enc_f8_sb")
    # hidden state ping-pong, factored (p, k, w, c, i)
    h0t_pp = [
        loopers.tile([128, KH, NW, NCORES, NBW], dt.bfloat16, name=f"h0t_pp{i}")
        for i in range(2)
    ]
    h1t_pp = [
        loopers.tile([128, KH, NW, NCORES, NBW], dt.bfloat16, name=f"h1t_pp{i}")
        for i in range(2)
    ]
    nc.sync.dma_start(
        h0t_pp[0][:],
        di["h0t_init"].ap().rearrange("(k p) (w c i) -> p k w c i", p=128, w=NW, c=NCORES),
    )
    nc.sync.dma_start(
        h1t_pp[0][:],
        di["h1t_init"].ap().rearrange("(k p) (w c i) -> p k w c i", p=128, w=NW, c=NCORES),
    )
    c_l0 = loopers.tile([128, B], dt.float32, name="c_l0")
    nc.sync.dma_start(c_l0[:], di["c0_l0"].ap())
    c_l1 = loopers.tile([128, B], dt.float32, name="c_l1")
    nc.sync.dma_start(c_l1[:], di["c0_l1"].ap())
    genw_sb = loopers.tile([128, KP, 2, VPC], dt.float8e4, name="genw_sb")
    genb_sb = loopers.tile([1, VPC], dt.bfloat16, name="genb_sb")
    nc.sync.dma_start(genb_sb[:], di["genb_v"].ap())
    bias_g0c_sb = loopers.tile([128, 4], dt.float32, name="bias_g0c_sb")
    nc.sync.dma_start(bias_g0c_sb[:], di["bias_g0c"].ap())
    sel_own_sb = loopers.tile([WB, NBW], dt.bfloat16, name="sel_own_sb")
    nc.sync.dma_start(sel_own_sb[:], di["sel_own"].ap())

    encw2 = loopers.tile([128, BPC, KH, T], dt.bfloat16, name="encw2")
    g_emb = loopers.tile([128, 4, NSAMP], dt.bfloat16, name="g_emb")
    hcat = loopers.tile([128, KP, 2, NSAMP], dt.float8e4, name="hcat")

    # ---- phase 1: embedding gather/transpose, encW2, emb-gate precompute ----
    with tc.tile_pool(name="p1emb", bufs=1) as p1emb:
        emb_t = p1emb.tile([128, KE, NSAMP], dt.bfloat16, name="emb_t")
        nc.gpsimd.memset(emb_t[:], 0.0)
        wih0e_sb = p1emb.tile([128, KE, NG], dt.bfloat16, name="wih0e_sb")
        nc.sync.dma_start(
            wih0e_sb[:], di["wih0e"].ap().rearrange("(k p) g -> p k g", p=128)
        )

        # 1a: gather + transpose to feature-major
        with tc.tile_pool(name="p1e", bufs=3) as p1e, \
             tc.tile_pool(name="p1eps", bufs=3, space="PSUM") as p1eps:
            for (m0, mr) in MTILES:
                idx = p1e.tile([128, 1], dt.int32, tag="idx")
                nc.sync.dma_start(idx[:mr, :], di["qidx"].ap()[m0:m0 + mr, :])
                gath = p1e.tile([128, E], dt.float32, tag="gath")
                nc.gpsimd.indirect_dma_start(
                    out=gath[:mr, :],
                    out_offset=None,
                    in_=di["emb_tab"].ap(),
                    in_offset=bass.IndirectOffsetOnAxis(ap=idx[:mr, 0:1], axis=0),
                )
                for k in range(KE):
                    cw = min(128, E - k * 128)
                    ps = p1eps.tile([128, 128], dt.float32, tag="ps")
                    nc.tensor.transpose(
                        ps[:cw, :mr], gath[:mr, k * 128:k * 128 + cw],
                        id_f32[:mr, :mr]
                    )
                    nc.vector.tensor_copy(emb_t[:cw, k, m0:m0 + mr], ps[:cw, :mr])

        # 1b: encW2[b] feature-major = W2 @ enc[b].T
        with tc.tile_pool(name="p1w", bufs=1) as p1w, \
             tc.tile_pool(name="p1s", bufs=3) as p1s, \
             tc.tile_pool(name="p1ps", bufs=1, space="PSUM") as p1ps:
            w2t_sb = p1w.tile([128, KH, H], dt.bfloat16, name="w2t_sb")
            nc.sync.dma_start(
                w2t_sb[:], di["w2t"].ap().rearrange("(k p) h -> p k h", p=128)
            )
            for b in range(BPC):
                pss = [
                    p1ps.tile([128, T], dt.float32, tag=f"p1p{m}", name=f"p1p{b}_{m}")
                    for m in range(KH)
                ]
                for k in range(KH):
                    rhs = p1s.tile([128, T], dt.bfloat16, tag="rhs")
                    nc.sync.dma_start(
                        rhs[:], di["enc_tr"].ap()[b, k * 128:(k + 1) * 128, :]
                    )
                    for m in range(KH):
                        nc.tensor.matmul(
                            pss[m][:],
                            w2t_sb[:, k, m * 128:(m + 1) * 128],
                            rhs[:],
                            start=(k == 0),
                            stop=(k == KH - 1),
                        )
                for m in range(KH):
                    if m % 2 == 0:
                        nc.vector.tensor_copy(encw2[:, b, m, :], pss[m][:])
                    else:
                        nc.scalar.activation(encw2[:, b, m, :], pss[m][:], AFT.Copy)

        # 1c: embedding gate contributions (bias folded on the copy)
        with tc.tile_pool(name="p1gps", bufs=3, space="PSUM") as p1gps:
            for gt in range(4):
                for ch in range(4):
                    c0 = ch * 400
                    ps = p1gps.tile([128, 400], dt.float32, tag="gps")
                    for ke in range(KE):
                        nc.tensor.matmul(
                            ps[:],
                            wih0e_sb[:, ke, gt * 128:(gt + 1) * 128],
                            emb_t[:, ke, c0:c0 + 400],
                            start=(ke == 0),
                            stop=(ke == KE - 1),
                        )
                    nc.vector.tensor_scalar(
                        g_emb[:, gt, c0:c0 + 400], ps[:],
                        bias_g0c_sb[:, gt:gt + 1], None, ALU.add,
                    )


    # Deferred bulk loads: these are needed only once the recurrent loop
    # reaches them (w1t/enc_f8 at step 0 attention, LSTM weights at gates,
    # genw at the first phase-4 burst) — issuing them after the phase-1
    # operands keeps the DMA engines free for encW2/g_emb startup.
    nc.sync.dma_start(w1t_sb[:], di["w1t"].ap().rearrange("(k p) h -> p k h", p=128))
    for _b in range(BPC):
        nc.sync.dma_start(
            enc_f8_sb[:, _b, :, :],
            di["enc_f8"].ap()[_b].rearrange("(k p) h -> p k h", p=128),
        )
    nc.sync.dma_start(wih0c_sb[:], di["wih0c"].ap().rearrange("(k p) g -> p k g", p=128))
    nc.sync.dma_start(whh0_sb[:], di["whh0"].ap().rearrange("(k p) g -> p k g", p=128))
    nc.sync.dma_start(wih1_sb[:], di["wih1"].ap().rearrange("(k p) g -> p k g", p=128))
    nc.sync.dma_start(whh1_sb[:], di["whh1"].ap().rearrange("(k p) g -> p k g", p=128))
    nc.sync.dma_start(genw_sb[:], di["genw_kp"].ap())

    # ---------------- phase 2: the recurrent loop ----------------
    sbw_cm = tc.tile_pool(name="sbw", bufs=2)
    sbw = sbw_cm.__enter__()
    psL_cm = tc.tile_pool(name="psL", bufs=1, space="PSUM")
    psL = psL_cm.__enter__()
    p4ps_cm = tc.tile_pool(name="p4ps", bufs=1, space="PSUM")
    p4ps = p4ps_cm.__enter__()
    p4c_cm = tc.tile_pool(name="p4c", bufs=1)
    p4c = p4c_cm.__enter__()

    def cell(gps, c_ap, tag):
        """gates i|f|o|g, i/f/o pre-scaled by 0.5 on the host so one tanh
        covers all four (sigmoid(x) = tanh(x/2)/2 + 0.5). Updates c_ap in place,
        returns h (128, WB) bf16. Elementwise runs on GpSimd (SBUF-only ops)
        to keep the DVE queue free for the energy bias-adds."""
        ifog = sbw.tile([128, 4, WB], dt.float32, tag=f"ifog{tag}")
        nc.scalar.activation(ifog[:], gps[:, :, :], AFT.Tanh)
        ifo = sbw.tile([128, 3, WB], dt.float32, tag=f"ifo{tag}")
        nc.gpsimd.tensor_scalar(ifo[:], ifog[:, 0:3, :], 0.5, 0.5, ALU.mult, ALU.add)
        t_fc = sbw.tile([128, WB], dt.float32, tag=f"tfc{tag}")
        nc.gpsimd.tensor_tensor(t_fc[:], ifo[:, 1, :], c_ap, op=ALU.mult)
        t_ig = sbw.tile([128, WB], dt.float32, tag=f"tig{tag}")
        nc.gpsimd.tensor_tensor(t_ig[:], ifo[:, 0, :], ifog[:, 3, :], op=ALU.mult)
        nc.gpsimd.tensor_tensor(c_ap, t_fc[:], t_ig[:], op=ALU.add)
        tc2 = sbw.tile([128, WB], dt.float32, tag=f"tc2{tag}")
        nc.scalar.activation(tc2[:], c_ap, AFT.Tanh)
        h = sbw.tile([128, WB], dt.bfloat16, tag=f"h{tag}")
        nc.gpsimd.tensor_tensor(h[:], ifo[:, 2, :], tc2[:], op=ALU.mult)
        return h

    def p4_mm(m, m0, mr):
        lg = p4c.tile([128, VPC], dt.bfloat16, tag="lgits", bufs=1)
        for vc in range(VC_N):
            v0 = vc * VC_W
            ps = p4ps.tile([128, 256], dt.float32, tag="p4p")
            nc.tensor.matmul(
                ps[:mr, :VC_W], ones_bf[0:1, :mr], genb_sb[0:1, v0:v0 + VC_W],
                start=True, stop=False,
            )
            for kp in range(KP):
                nc.tensor.matmul(
                    ps[:mr, :VC_W],
                    hcat[:, kp, :, m0:m0 + mr],
                    genw_sb[:, kp, :, v0:v0 + VC_W],
                    start=False, stop=(kp == KP - 1),
                    perf_mode=PM.DoubleRow,
                )
            nc.vector.tensor_copy(lg[:mr, v0:v0 + VC_W], ps[:mr, :VC_W])
        return lg

    def p4_tail(m, m0, mr, lg):
        for ec in range(4):
            e0 = ec * 1000
            tmp = p4c.tile([128, 1000], dt.bfloat16, tag="etmp", bufs=2)
            nc.scalar.activation(
                tmp[:mr], lg[:mr, e0:e0 + 1000], AFT.Exp,
                accum_out=sume8[:mr, ec:ec + 1],
            )
        nc.vector.tensor_reduce(
            sume[:mr, m:m + 1], sume8[:mr, 0:4], axis=mybir.AxisListType.X,
            op=ALU.add,
        )
        bar_in = dram.tile([128, 1], dt.float32, tag="bar_in", bufs=2)
        nc.gpsimd.dma_start(bar_in[:mr], sume[:mr, m:m + 1])
        bar_out = dram.tile([128, 1], dt.float32, tag="bar_out", bufs=2,
                            addr_space=SHARED)
        _allreduce(nc, bar_in[:], bar_out[:])
        sg = p4c.tile([128, 1], dt.float32, tag="sg", bufs=2)
        nc.gpsimd.dma_start(sg[:], bar_out[:])
        lse = p4c.tile([128, 1], dt.float32, tag="lse", bufs=2)
        nc.scalar.activation(lse[:mr], sg[:mr], AFT.Ln)
        for oc in range(OC_N):
            o0 = oc * OC_W
            lpo = p4c.tile([128, OC_W], dt.float32, tag="lpo", bufs=2)
            nc.vector.tensor_scalar(
                lpo[:mr], lg[:mr, o0:o0 + OC_W], lse[:mr, 0:1], None, ALU.subtract
            )
            nc.gpsimd.dma_start(logp.ap()[m0:m0 + mr, o0:o0 + OC_W], lpo[:mr])

    lg_pend = {}
    for s in range(S_EFF):
        h1t_prev = h1t_pp[s % 2]
        h0t_prev = h0t_pp[s % 2]
        h1t_next = h1t_pp[(s + 1) % 2]
        h0t_next = h0t_pp[(s + 1) % 2]

        for w in range(NW):
            wc = slice(w * WB, (w + 1) * WB)
            # DMA issue queue for this wave's exchange chains: SP for wave 0,
            # GpSimd (SWDGE) for wave 1 — avoids cross-chain head-of-line
            # blocking on one sequencer.
            dq = nc.sync if w % 2 == 0 else nc.gpsimd

            # --- hw = W1 h1 + attn_b for this wave's 16 cols, then pick own
            #     2 cols via the per-core sel matrix (SPMD-safe selection) ---
            ps_hw = psL.tile([128, KH, WB], dt.float32, tag="ps_hwx", bufs=2)
            for m in range(KH):
                nc.tensor.matmul(
                    ps_hw[:, m, :],
                    attn_b_sb[0:1, m * 128:(m + 1) * 128],
                    ones_bf[0:1, 0:WB],
                    start=True, stop=False,
                )
                for k in range(KH):
                    nc.tensor.matmul(
                        ps_hw[:, m, :],
                        w1t_sb[:, k, m * 128:(m + 1) * 128],
                        h1t_prev[:, k, w],
                        start=False, stop=(k == KH - 1),
                    )
            hwf = sbw.tile([128, KH, WB], dt.bfloat16, tag="hwf", bufs=2)
            nc.vector.tensor_copy(hwf[:], ps_hw[:])
            ps_t = psL.tile([WB, KH, 128], dt.bfloat16, tag="ps_hwx", bufs=2)
            for m in range(KH):
                nc.tensor.transpose(ps_t[:WB, m, :], hwf[:, m, :], id_bf[:, :])
            hwT = sbw.tile([WB, KH, 128], dt.bfloat16, tag="hwT", bufs=2)
            nc.vector.tensor_copy(hwT[:], ps_t[:WB, :, :])
            ps_own = psL.tile([128, KH, NBW], dt.float32, tag="ps_hwx", bufs=2)
            for m in range(KH):
                nc.tensor.matmul(
                    ps_own[:, m, :], hwT[:WB, m, :], sel_own_sb[:],
                    start=True, stop=True,
                )
            hwt = sbw.tile([128, KH, NBW], dt.float32, tag="hwt", bufs=2)
            nc.vector.tensor_copy(hwt[:], ps_own[:])
            # --- attention for wave's 2 local batches ---
            ps_sc = psL.tile([128, KT, NBW], dt.float32, tag="ps_hwx", bufs=2)
            for i in range(NBW):
                lb = NBW * w + i
                for half in range(2):
                    k0 = half * 4
                    en = sbw.tile([128, 4, T], dt.bfloat16, tag="en", bufs=4)
                    for kk in range(4):
                        nc.vector.tensor_scalar(
                            en[:, kk, :], encw2[:, lb, k0 + kk, :],
                            hwt[:, k0 + kk, i:i + 1], None, ALU.add,
                        )
                    nc.scalar.activation(en[:], en[:], AFT.Tanh)
                    for tk in range(KT):
                        for kk in range(4):
                            nc.tensor.matmul(
                                ps_sc[:, tk, i:i + 1],
                                en[:, kk, tk * 128:(tk + 1) * 128],
                                vvec_sb[:, k0 + kk, :],
                                start=(k0 + kk == 0), stop=(k0 + kk == KH - 1),
                            )
            # --- softmax (unnormalized weights + reciprocal for ctx scale) ---
            exps = sbw.tile([128, KT, NBW], dt.bfloat16, tag="exps", bufs=2)
            nc.scalar.activation(exps[:], ps_sc[:], AFT.Exp)
            ps_den = psL.tile([128, NBW], dt.float32, tag="ps_small", bufs=1)
            for tk in range(KT):
                nc.tensor.matmul(
                    ps_den[0:1, :], ones_col[:, :], exps[:, tk, :],
                    start=(tk == 0), stop=(tk == KT - 1),
                )
            rec = sbw.tile([1, NBW], dt.float32, tag="rec", bufs=2)
            nc.vector.reciprocal(rec[:], ps_den[0:1, :])
            ps_rcb = psL.tile([128, NBW], dt.float32, tag="ps_small", bufs=1)
            nc.tensor.matmul(
                ps_rcb[:, :], ones_f32[:, :], rec[0:1, :], start=True, stop=True
            )
            recb = sbw.tile([128, NBW], dt.float32, tag="recb", bufs=2)
            nc.vector.tensor_copy(recb[:], ps_rcb[:])
            # --- context (feature-major, scaled by 1/den on copy) ---
            ctxw = sbw.tile([128, KH, NBW], dt.bfloat16, tag="ctxw", bufs=2)
            for i in range(NBW):
                lb = NBW * w + i
                ps_cx = psL.tile([128, KH], dt.float32, tag="ps_cx", bufs=1)
                for hk in range(KH):
                    for tk in range(KT):
                        nc.tensor.matmul(
                            ps_cx[:, hk:hk + 1],
                            enc_f8_sb[:, lb, tk, hk * 128:(hk + 1) * 128],
                            exps[:, tk, i:i + 1],
                            start=(tk == 0), stop=(tk == KT - 1),
                        )
                nc.vector.tensor_scalar(
                    ctxw[:, :, i], ps_cx[:], recb[:, i:i + 1], None, ALU.mult
                )
            # --- exchange ctx (AllGather over cores) ---
            bx_in = dram.tile([128, KH * NBW], dt.bfloat16, tag="bx_in", bufs=3)
            bx_out = dram.tile([NCORES * 128, KH * NBW], dt.bfloat16, tag="bx_out",
                               bufs=3, addr_space=SHARED)
            xt = sbw.tile([128, NCORES, KH, NBW], dt.bfloat16, tag="xt", bufs=2)
            _exchange(
                nc, dq, ctxw[:].rearrange("p k i -> p (k i)"),
                xt[:].rearrange("p c k i -> p c (k i)"), bx_in, bx_out,
                xt[:],
                bx_out[:].rearrange("(c p) (k i) -> p c k i", p=128, k=KH),
            )
            # --- LSTM layer 0 gates (N=16) ---
            ps_g0 = psL.tile([128, 4, WB], dt.float32, tag="ps_g", bufs=3)
            for gt in range(4):
                gsl = slice(gt * 128, (gt + 1) * 128)
                nc.tensor.matmul(
                    ps_g0[:, gt, :], id_bf[:],
                    g_emb[:, gt, s * B + w * WB:s * B + (w + 1) * WB],
                    start=True, stop=False,
                )
                for k in range(KH):
                    nc.tensor.matmul(
                        ps_g0[:, gt, :], whh0_sb[:, k, gsl],
                        h0t_prev[:, k, w], start=False, stop=False,
                    )
                for k in range(KH):
                    nc.tensor.matmul(
                        ps_g0[:, gt, :], wih0c_sb[:, k, gsl],
                        xt[:, :, k, :], start=False, stop=(k == KH - 1),
                    )
            h0n = cell(ps_g0, c_l0[:, wc], "l0")
            bh0_in = dram.tile([128, WB], dt.bfloat16, tag="bh0_in", bufs=3)
            bh0_out = dram.tile([NCORES * 128, WB], dt.bfloat16, tag="bh0_out",
                                bufs=3, addr_space=SHARED)
            _exchange(
                nc, dq, h0n[:],
                h0t_next[:, :, w].rearrange("p g c i -> p g (c i)"),
                bh0_in, bh0_out,
                h0t_next[:, :, w],
                bh0_out[:].rearrange("(g p) (c i) -> p g c i", p=128, c=NCORES),
            )
            # --- LSTM layer 1 gates ---
            ps_g1 = psL.tile([128, 4, WB], dt.float32, tag="ps_g", bufs=3)
            for gt in range(4):
                gsl = slice(gt * 128, (gt + 1) * 128)
                nc.tensor.matmul(
                    ps_g1[:, gt, :], bias_g1_sb[0:1, gsl], ones_bf[0:1, :WB],
                    start=True, stop=False,
                )
                for k in range(KH):
                    nc.tensor.matmul(
                        ps_g1[:, gt, :], whh1_sb[:, k, gsl],
                        h1t_prev[:, k, w], start=False, stop=False,
                    )
                for k in range(KH):
                    nc.tensor.matmul(
                        ps_g1[:, gt, :], wih1_sb[:, k, gsl],
                        h0t_next[:, k, w], start=False, stop=(k == KH - 1),
                    )
            h1n = cell(ps_g1, c_l1[:, wc], "l1")
            bh1_in = dram.tile([128, WB], dt.bfloat16, tag="bh1_in", bufs=3)
            bh1_out = dram.tile([NCORES * 128, WB], dt.bfloat16, tag="bh1_out",
                                bufs=3, addr_space=SHARED)
            _exchange(
                nc, dq, h1n[:],
                h1t_next[:, :, w].rearrange("p g c i -> p g (c i)"),
                bh1_in, bh1_out,
                h1t_next[:, :, w],
                bh1_out[:].rearrange("(g p) (c i) -> p g c i", p=128, c=NCORES),
            )
        # --- h1 history for the vocab projection (fp8, k-pair layout) ---
        for k in range(KH):
            nc.vector.tensor_copy(
                hcat[:, k // 2, k % 2, s * B:(s + 1) * B],
                h1t_next[:, k].rearrange("p w c i -> p (w c i)"),
            )
        # --- interleaved vocab projection bursts (matmuls at step 4m+3,
        #     exp/lse/subtract/output tail one step later to spread load) ---
        if s % 4 == 0 and s > 0 and (s // 4 - 1) in lg_pend:
            m = s // 4 - 1
            p4_tail(m, MTILES[m][0], MTILES[m][1], lg_pend.pop(m))
        if (s + 1) % 4 == 0:
            m = (s + 1) // 4 - 1
            lg_pend[m] = p4_mm(m, MTILES[m][0], MTILES[m][1])
        if s == S_EFF - 1:
            for m in sorted(lg_pend):
                p4_tail(m, MTILES[m][0], MTILES[m][1], lg_pend.pop(m))
            if (s + 1) * B % 128 != 0:
                m = ((s + 1) * B) // 128
                lg2 = p4_mm(m, m * 128, (s + 1) * B - m * 128)
                p4_tail(m, m * 128, (s + 1) * B - m * 128, lg2)

    # close loop pools
    p4c_cm.__exit__(None, None, None)
    p4ps_cm.__exit__(None, None, None)
    psL_cm.__exit__(None, None, None)
    sbw_cm.__exit__(None, None, None)
    loopers_cm.__exit__(None, None, None)
    dram_cm.__exit__(None, None, None)
    glob_cm.__exit__(None, None, None)


def _prep_inputs(inputs):
    """Host-side sharding/layout prep. Returns list of per-core input dicts."""
    f32 = np.float32
    enc_out = np.asarray(inputs["enc_out"], f32)
    enc_h = np.asarray(inputs["enc_h"], f32)
    enc_c = np.asarray(inputs["enc_c"], f32)
    emb = np.asarray(inputs["embedding"], f32)
    attn_W = np.asarray(inputs["attn_W"], f32)
    attn_b = np.asarray(inputs["attn_b"], f32)
    vv = np.asarray(inputs["v"], f32)
    Wih0 = np.asarray(inputs["Wih0"], f32)
    Whh0 = np.asarray(inputs["Whh0"], f32)
    bih0 = np.asarray(inputs["bih0"], f32)
    bhh0 = np.asarray(inputs["bhh0"], f32)
    Wih1 = np.asarray(inputs["Wih1"], f32)
    Whh1 = np.asarray(inputs["Whh1"], f32)
    bih1 = np.asarray(inputs["bih1"], f32)
    bhh1 = np.asarray(inputs["bhh1"], f32)
    genW = np.asarray(inputs["genW"], f32)
    genb = np.asarray(inputs["genb"], f32)
    q = np.asarray(inputs["question"]).astype(np.int64)

    W1 = attn_W[:, :H]
    W2 = attn_W[:, H:]
    h0 = np.concatenate([enc_h[0], enc_h[1]], 1)  # (B, H) layer 0
    h1 = np.concatenate([enc_h[2], enc_h[3]], 1)  # layer 1
    c0 = np.concatenate([enc_c[0], enc_c[1]], 1)
    c1 = np.concatenate([enc_c[2], enc_c[3]], 1)

    # step-column order: col = w*16 + c2*2 + i  <->  global batch 4*c2 + 2*w + i
    col2gb = np.array(
        [4 * c2 + NBW * w + i for w in range(NW) for c2 in range(NCORES)
         for i in range(NBW)], dtype=np.int64)
    qperm = q[col2gb, :].T.reshape(NSAMP, 1).astype(np.int32)  # (s, col) order

    def bf(x):
        return np.ascontiguousarray(x).astype(BF)

    def f8(x):
        return np.ascontiguousarray(x).astype(F8)

    shared = {
        "w2t": bf(W2.T),
        "w1t": bf(W1.T),
        "attn_bias": bf(attn_b.reshape(1, H)),
        "vvec": bf(vv.reshape(H, 1)),
        "emb_tab": np.ascontiguousarray(emb),
        "qidx": qperm,
        "h0t_init": bf(h0[col2gb].T),
        "h1t_init": bf(h1[col2gb].T),
    }
    maps = []
    for c in range(NCORES):
        # local batches (in (w, i) order) = global ids for this core's slots
        my_gb = [4 * c + NBW * w + i for w in range(NW) for i in range(NBW)]
        sel_own = np.zeros((WB, NBW), f32)
        for i in range(NBW):
            sel_own[c * NBW + i, i] = 1.0
        # gate rows: order i|f|o|g (torch order is i,f,g,o -> pick blocks 0,1,3,2)
        gorder = [0, 1, 3, 2]
        rows = np.concatenate(
            [np.arange(g * H + c * GS, g * H + (c + 1) * GS) for g in gorder]
        )
        gsc = np.repeat([0.5, 0.5, 0.5, 1.0], GS)[:, None].astype(f32)
        wih0_s = Wih0[rows] * gsc  # (NG, E+H); i/f/o halved for 1-tanh cells
        wih0e = np.zeros((EP, NG), f32)
        wih0e[:E] = wih0_s[:, :E].T
        bias0 = (bih0 + bhh0)[rows] * gsc[:, 0]  # (NG,)
        bias_g0c = bias0.reshape(4, GS).T  # (GS, 4)
        vrows = slice(c * VPC, (c + 1) * VPC)
        genw_t = genW[vrows].T  # (H, VPC)
        genw_kp = np.ascontiguousarray(
            genw_t.reshape(KP, 2, 128, VPC).transpose(2, 0, 1, 3)
        )
        m = dict(shared)
        m.update({
            "enc_tr": bf(enc_out[my_gb].transpose(0, 2, 1)),
            "enc_f8": f8(enc_out[my_gb]),
            "wih0e": bf(wih0e),
            "sel_own": bf(sel_own),
            "bias_g0c": np.ascontiguousarray(bias_g0c),
            "wih0c": bf(wih0_s[:, E:].T),
            "whh0": bf((Whh0[rows] * gsc).T),
            "wih1": bf((Wih1[rows] * gsc).T),
            "whh1": bf((Whh1[rows] * gsc).T),
            "bias_g1": bf(((bih1 + bhh1)[rows] * gsc[:, 0]).reshape(1, NG)),
            "c0_l0": np.ascontiguousarray(c0[col2gb, c * GS:(c + 1) * GS].T),
            "c0_l1": np.ascontiguousarray(c1[col2gb, c * GS:(c + 1) * GS].T),
            "genw_kp": f8(genw_kp),
            "genb_v": bf(genb[vrows].reshape(1, VPC)),
        })
        maps.append(m)
    return maps


_CACHED = {}


def _get_compiled():
    if "nc" not in _CACHED:
        nc = bacc.Bacc(
            "TRN2", target_bir_lowering=False, debug=False,
            num_devices=1 if SIM1 else NCORES,
        )
        build(nc)
        nc.compile()
        _CACHED["nc"] = nc
    return _CACHED["nc"]


def run_cores(in_maps, **kw):
    nc = _get_compiled()
    return bass_utils.run_bass_kernel_spmd(nc, in_maps, list(range(NCORES)), **kw)


def kernel(**inputs):
    in_maps = _prep_inputs(inputs)
    res = run_cores(in_maps)
    parts = [res.results[c]["logp"] for c in range(NCORES)]
    full = np.concatenate(parts, axis=1)  # (NSAMP, V) in (s, col) order
    col2gb = np.array(
        [4 * c2 + NBW * w + i for w in range(NW) for c2 in range(NCORES)
         for i in range(NBW)], dtype=np.int64)
    full = full.reshape(S, B, V)
    out = np.empty((B, S, V), np.float32)
    out[col2gb, :, :] = full.transpose(1, 0, 2)
    return np.ascontiguousarray(out)


# revision 60
# speedup vs baseline: 2.9168x; 1.0024x over previous
"""Trainium2 Bass kernel for the attention-LSTM decoder (nn_Decoder).

Strategy (8 NeuronCores), v2 — restructured for the TRN2 cost model
(matmul cost ~ output free size; Act/DVE cost ~ free size; DVE 4x for
bf16 SBUF tensor_scalar):
  - Attention batch-sharded: each core owns B/8 = 4 batches. Energies are
    computed feature-major: DVE adds the per-step hidden bias (4x mode),
    Act does tanh in 2 big instructions per batch. Scores/softmax are
    transpose-free (ones-matmul partition reductions, unnormalized exp
    weights with context post-scaling).
  - LSTM tensor-parallel over gate rows (512/core, gate order i|f|o|g),
    everything feature-major so gate matmuls have small-N outputs and the
    cell state lives as (128, B) tiles. Batches advance in NW=4 waves of 8
    columns, giving 4 independent per-step pipelines whose exchange chains
    (issued alternately on the SP and GpSimd DMA queues) overlap the
    Act-bound tanh work; cell elementwise runs on GpSimd.
  - Vocab projection tensor-parallel over V (4000/core) in fp8 with
    DoubleRow (2 k-tiles per matmul, 0.5 cyc/row), interleaved into the
    recurrent loop per 128-sample mtile; per-mtile exp-sums, logsumexp
    AllReduce, subtract, and f32 output DMA all stream during the loop.
Dtypes: bf16 compute everywhere, fp32 PSUM + cell state, fp8e4m3 for the
ctx encoder operand and the vocab projection (genW and the h1 history).
"""
import os
import sys

sys.path.insert(0, "/opt/trn_rl_repo")

import numpy as np
import ml_dtypes

import concourse.bass as bass
import concourse.bacc as bacc
import concourse.mybir as mybir
import concourse.tile as tile
from concourse import bass_utils
from concourse.masks import make_identity

BF = ml_dtypes.bfloat16
F8 = ml_dtypes.float8_e4m3
dt = mybir.dt
AFT = mybir.ActivationFunctionType
ALU = mybir.AluOpType
PM = mybir.MatmulPerfMode

B, T, H, E, V, S = 32, 512, 1024, 300, 32000, 50
NCORES = 8
BPC = B // NCORES      # 4 batches per core
GS = H // NCORES       # 128-wide hidden slice per core
NG = 4 * GS            # 512 gate rows per core (i|f|o|g blocks of 128)
VPC = V // NCORES      # 4000 vocab rows per core
EP = 384               # padded embedding feature dim (3 k-tiles)
KE = EP // 128         # 3
KH = H // 128          # 8
KT = T // 128          # 4
KP = KH // 2           # 4 k-pairs for fp8 DoubleRow
NSAMP = S * B          # 1600
NW = int(os.environ.get("DECODER_NW", "4"))   # batch waves per step
WB = B // NW           # step-columns per wave
NBW = BPC // NW        # local batches per wave
S_EFF = int(os.environ.get("DECODER_STEPS", str(S)))
SIM1 = os.environ.get("DECODER_SIM", "0") == "1"
RG = [list(range(NCORES))]
SHARED = "Local" if SIM1 else "Shared"

# phase-4 sample tiles: 12 x 128 + 1 x 64
MTILES = [(m * 128, min(128, NSAMP - m * 128)) for m in range((NSAMP + 127) // 128)]
VC_N, VC_W = 16, 250     # vocab chunks for the projection psum
OC_N, OC_W = 8, 500      # output chunks for subtract + DMA


def _exchange(nc, eng, src_flat_ap, dst_bc_ap, stage_tile, shared_tile,
              reload_out_ap, reload_in_ap):
    """AllGather src (sbuf, (128, n)) into a consumer sbuf tile holding all
    8 cores' slices. SIM1 cost proxy: ONE fan-out DMA straight into the
    destination SBUF tile — the cost a remote-DMA-broadcast implementation
    would pay per exchange (same bytes x 8 destinations). Real build:
    stage to dram -> AllGather collective -> reload (collectives need dram).
    `eng` picks the DMA issue queue (SP / Pool)."""
    if SIM1:
        rows, cols = src_flat_ap.shape[0], src_flat_ap.shape[1]
        eng.dma_start(
            dst_bc_ap,
            src_flat_ap.unsqueeze(1).broadcast_to((rows, NCORES, cols)),
        )
    else:
        eng.dma_start(stage_tile[:], src_flat_ap)
        nc.gpsimd.collective_compute(
            "AllGather", mybir.AluOpType.bypass, replica_groups=RG,
            ins=[stage_tile[:].opt()], outs=[shared_tile[:].opt()],
        )
        eng.dma_start(reload_out_ap, reload_in_ap)


def _allreduce(nc, in_ap, out_ap):
    if SIM1:
        nc.gpsimd.dma_start(out_ap, in_ap)
    else:
        nc.gpsimd.collective_compute(
            "AllReduce", mybir.AluOpType.add, replica_groups=RG,
            ins=[in_ap.opt()], outs=[out_ap.opt()],
        )


def build(nc):
    di = {}

    def inp(name, shape, dtype):
        di[name] = nc.dram_tensor(name, list(shape), dtype, kind="ExternalInput")
        return di[name]

    inp("enc_tr", (BPC, H, T), dt.bfloat16)       # feature-major enc (p1b rhs)
    inp("enc_f8", (BPC, T, H), dt.float8e4)       # time-major enc (ctx lhsT)
    inp("w2t", (H, H), dt.bfloat16)
    inp("w1t", (H, H), dt.bfloat16)
    inp("attn_bias", (1, H), dt.bfloat16)
    inp("vvec", (H, 1), dt.bfloat16)
    inp("emb_tab", (V, E), dt.float32)
    inp("qidx", (NSAMP, 1), dt.int32)
    inp("wih0e", (EP, NG), dt.bfloat16)
    inp("bias_g0c", (GS, 4), dt.float32)
    inp("wih0c", (H, NG), dt.bfloat16)
    inp("whh0", (H, NG), dt.bfloat16)
    inp("wih1", (H, NG), dt.bfloat16)
    inp("whh1", (H, NG), dt.bfloat16)
    inp("bias_g1", (1, NG), dt.bfloat16)
    inp("sel_own", (WB, NBW), dt.bfloat16)
    inp("h0t_init", (H, B), dt.bfloat16)
    inp("h1t_init", (H, B), dt.bfloat16)
    inp("c0_l0", (GS, B), dt.float32)
    inp("c0_l1", (GS, B), dt.float32)
    inp("genw_kp", (128, KP, 2, VPC), dt.float8e4)
    inp("genb_v", (1, VPC), dt.bfloat16)
    logp = nc.dram_tensor("logp", [NSAMP, VPC], dt.float32, kind="ExternalOutput")

    with tile.TileContext(nc) as tc:
        _body(nc, tc, di, logp)
    return di


def _body(nc, tc, di, logp):
    glob_cm = tc.tile_pool(name="glob", bufs=1)
    glob = glob_cm.__enter__()
    dram_cm = tc.tile_pool(name="dram", bufs=1, space="DRAM")
    dram = dram_cm.__enter__()

    # ---- global constants ----
    id_bf = glob.tile([128, 128], dt.bfloat16, name="id_bf")
    id_f32 = glob.tile([128, 128], dt.float32, name="id_f32")
    make_identity(nc, id_bf[:])
    make_identity(nc, id_f32[:])
    ones_bf = glob.tile([1, 512], dt.bfloat16, name="ones_bf")
    nc.gpsimd.memset(ones_bf[:], 1.0)
    ones_col = glob.tile([128, 1], dt.bfloat16, name="ones_col")
    nc.gpsimd.memset(ones_col[:], 1.0)
    ones_f32 = glob.tile([1, 128], dt.float32, name="ones_f32")
    nc.gpsimd.memset(ones_f32[:], 1.0)
    sume = glob.tile([128, len(MTILES)], dt.float32, name="sume")
    sume8 = glob.tile([128, OC_N], dt.float32, name="sume8")

    # ---------------- persistent loop tensors ----------------
    loopers_cm = tc.tile_pool(name="loopers", bufs=1)
    loopers = loopers_cm.__enter__()

    w1t_sb = loopers.tile([128, KH, H], dt.bfloat16, name="w1t_sb")
    vvec_sb = loopers.tile([128, KH, 1], dt.bfloat16, name="vvec_sb")
    nc.sync.dma_start(vvec_sb[:], di["vvec"].ap().rearrange("(k p) o -> p k o", p=128))
    attn_b_sb = loopers.tile([1, H], dt.bfloat16, name="attn_b_sb")
    nc.sync.dma_start(attn_b_sb[:], di["attn_bias"].ap())
    wih0c_sb = loopers.tile([128, KH, NG], dt.bfloat16, name="wih0c_sb")
    whh0_sb = loopers.tile([128, KH, NG], dt.bfloat16, name="whh0_sb")
    wih1_sb = loopers.tile([128, KH, NG], dt.bfloat16, name="wih1_sb")
    whh1_sb = loopers.tile([128, KH, NG], dt.bfloat16, name="whh1_sb")
    bias_g1_sb = loopers.tile([1, NG], dt.bfloat16, name="bias_g1_sb")
    nc.sync.dma_start(bias_g1_sb[:], di["bias_g1"].ap())
    enc_f8_sb = loopers.tile([128, BPC, KT, H], dt.float8e4, name="enc_f8_sb")
    # hidden state ping-pong, factored (p, k, w, c, i)
    h0t_pp = [
        loopers.tile([128, KH, NW, NCORES, NBW], dt.bfloat16, name=f"h0t_pp{i}")
        for i in range(2)
    ]
    h1t_pp = [
        loopers.tile([128, KH, NW, NCORES, NBW], dt.bfloat16, name=f"h1t_pp{i}")
        for i in range(2)
    ]
    nc.sync.dma_start(
        h0t_pp[0][:],
        di["h0t_init"].ap().rearrange("(k p) (w c i) -> p k w c i", p=128, w=NW, c=NCORES),
    )
    nc.sync.dma_start(
        h1t_pp[0][:],
        di["h1t_init"].ap().rearrange("(k p) (w c i) -> p k w c i", p=128, w=NW, c=NCORES),
    )
    c_l0 = loopers.tile([128, B], dt.float32, name="c_l0")
    nc.sync.dma_start(c_l0[:], di["c0_l0"].ap())
    c_l1 = loopers.tile([128, B], dt.float32, name="c_l1")
    nc.sync.dma_start(c_l1[:], di["c0_l1"].ap())
    genw_sb = loopers.tile([128, KP, 2, VPC], dt.float8e4, name="genw_sb")
    genb_sb = loopers.tile([1, VPC], dt.bfloat16, name="genb_sb")
    nc.sync.dma_start(genb_sb[:], di["genb_v"].ap())
    bias_g0c_sb = loopers.tile([128, 4], dt.float32, name="bias_g0c_sb")
    nc.sync.dma_start(bias_g0c_sb[:], di["bias_g0c"].ap())
    sel_own_sb = loopers.tile([WB, NBW], dt.bfloat16, name="sel_own_sb")
    nc.sync.dma_start(sel_own_sb[:], di["sel_own"].ap())

    encw2 = loopers.tile([128, BPC, KH, T], dt.bfloat16, name="encw2")
    g_emb = loopers.tile([128, 4, NSAMP], dt.bfloat16, name="g_emb")
    hcat = loopers.tile([128, KP, 2, NSAMP], dt.float8e4, name="hcat")

    # ---- phase 1: embedding gather/transpose, encW2, emb-gate precompute ----
    with tc.tile_pool(name="p1emb", bufs=1) as p1emb:
        emb_t = p1emb.tile([128, KE, NSAMP], dt.bfloat16, name="emb_t")
        nc.gpsimd.memset(emb_t[:], 0.0)
        wih0e_sb = p1emb.tile([128, KE, NG], dt.bfloat16, name="wih0e_sb")
        nc.sync.dma_start(
            wih0e_sb[:], di["wih0e"].ap().rearrange("(k p) g -> p k g", p=128)
        )

        # 1a: gather + transpose to feature-major
        with tc.tile_pool(name="p1e", bufs=3) as p1e, \
             tc.tile_pool(name="p1eps", bufs=3, space="PSUM") as p1eps:
            for (m0, mr) in MTILES:
                idx = p1e.tile([128, 1], dt.int32, tag="idx")
                nc.sync.dma_start(idx[:mr, :], di["qidx"].ap()[m0:m0 + mr, :])
                gath = p1e.tile([128, E], dt.float32, tag="gath")
                nc.gpsimd.indirect_dma_start(
                    out=gath[:mr, :],
                    out_offset=None,
                    in_=di["emb_tab"].ap(),
                    in_offset=bass.IndirectOffsetOnAxis(ap=idx[:mr, 0:1], axis=0),
                )
                for k in range(KE):
                    cw = min(128, E - k * 128)
                    ps = p1eps.tile([128, 128], dt.float32, tag="ps")
                    nc.tensor.transpose(
                        ps[:cw, :mr], gath[:mr, k * 128:k * 128 + cw],
                        id_f32[:mr, :mr]
                    )
                    nc.vector.tensor_copy(emb_t[:cw, k, m0:m0 + mr], ps[:cw, :mr])

        # 1b: encW2[b] feature-major = W2 @ enc[b].T
        with tc.tile_pool(name="p1w", bufs=1) as p1w, \
             tc.tile_pool(name="p1s", bufs=3) as p1s, \
             tc.tile_pool(name="p1ps", bufs=1, space="PSUM") as p1ps:
            w2t_sb = p1w.tile([128, KH, H], dt.bfloat16, name="w2t_sb")
            nc.sync.dma_start(
                w2t_sb[:], di["w2t"].ap().rearrange("(k p) h -> p k h", p=128)
            )
            for b in range(BPC):
                pss = [
                    p1ps.tile([128, T], dt.float32, tag=f"p1p{m}", name=f"p1p{b}_{m}")
                    for m in range(KH)
                ]
                for k in range(KH):
                    rhs = p1s.tile([128, T], dt.bfloat16, tag="rhs")
                    nc.sync.dma_start(
                        rhs[:], di["enc_tr"].ap()[b, k * 128:(k + 1) * 128, :]
                    )
                    for m in range(KH):
                        nc.tensor.matmul(
                            pss[m][:],
                            w2t_sb[:, k, m * 128:(m + 1) * 128],
                            rhs[:],
                            start=(k == 0),
                            stop=(k == KH - 1),
                        )
                for m in range(KH):
                    if m % 2 == 0:
                        nc.vector.tensor_copy(encw2[:, b, m, :], pss[m][:])
                    else:
                        nc.scalar.activation(encw2[:, b, m, :], pss[m][:], AFT.Copy)

        # 1c: embedding gate contributions (bias folded on the copy)
        with tc.tile_pool(name="p1gps", bufs=3, space="PSUM") as p1gps:
            for gt in range(4):
                for ch in range(4):
                    c0 = ch * 400
                    ps = p1gps.tile([128, 400], dt.float32, tag="gps")
                    for ke in range(KE):
                        nc.tensor.matmul(
                            ps[:],
                            wih0e_sb[:, ke, gt * 128:(gt + 1) * 128],
                            emb_t[:, ke, c0:c0 + 400],
                            start=(ke == 0),
                            stop=(ke == KE - 1),
                        )
                    nc.vector.tensor_scalar(
                        g_emb[:, gt, c0:c0 + 400], ps[:],
                        bias_g0c_sb[:, gt:gt + 1], None, ALU.add,
                    )


    # Deferred bulk loads: these are needed only once the recurrent loop
    # reaches them (w1t/enc_f8 at step 0 attention, LSTM weights at gates,
    # genw at the first phase-4 burst) — issuing them after the phase-1
    # operands keeps the DMA engines free for encW2/g_emb startup.
    nc.sync.dma_start(w1t_sb[:], di["w1t"].ap().rearrange("(k p) h -> p k h", p=128))
    for _b in range(BPC):
        nc.sync.dma_start(
            enc_f8_sb[:, _b, :, :],
            di["enc_f8"].ap()[_b].rearrange("(k p) h -> p k h", p=128),
        )
    nc.sync.dma_start(wih0c_sb[:], di["wih0c"].ap().rearrange("(k p) g -> p k g", p=128))
    nc.sync.dma_start(whh0_sb[:], di["whh0"].ap().rearrange("(k p) g -> p k g", p=128))
    nc.sync.dma_start(wih1_sb[:], di["wih1"].ap().rearrange("(k p) g -> p k g", p=128))
    nc.sync.dma_start(whh1_sb[:], di["whh1"].ap().rearrange("(k p) g -> p k g", p=128))
    nc.sync.dma_start(genw_sb[:], di["genw_kp"].ap())

    # ---------------- phase 2: the recurrent loop ----------------
    sbw_cm = tc.tile_pool(name="sbw", bufs=2)
    sbw = sbw_cm.__enter__()
    psL_cm = tc.tile_pool(name="psL", bufs=1, space="PSUM")
    psL = psL_cm.__enter__()
    p4ps_cm = tc.tile_pool(name="p4ps", bufs=1, space="PSUM")
    p4ps = p4ps_cm.__enter__()
    p4c_cm = tc.tile_pool(name="p4c", bufs=1)
    p4c = p4c_cm.__enter__()

    def cell(gps, c_ap, tag):
        """gates i|f|o|g, i/f/o pre-scaled by 0.5 on the host so one tanh
        covers all four (sigmoid(x) = tanh(x/2)/2 + 0.5). Updates c_ap in place,
        returns h (128, WB) bf16. Elementwise runs on GpSimd (SBUF-only ops)
        to keep the DVE queue free for the energy bias-adds."""
        ifog = sbw.tile([128, 4, WB], dt.float32, tag=f"ifog{tag}")
        nc.scalar.activation(ifog[:], gps[:, :, :], AFT.Tanh)
        ifo = sbw.tile([128, 3, WB], dt.float32, tag=f"ifo{tag}")
        nc.gpsimd.tensor_scalar(ifo[:], ifog[:, 0:3, :], 0.5, 0.5, ALU.mult, ALU.add)
        t_fc = sbw.tile([128, WB], dt.float32, tag=f"tfc{tag}")
        nc.gpsimd.tensor_tensor(t_fc[:], ifo[:, 1, :], c_ap, op=ALU.mult)
        t_ig = sbw.tile([128, WB], dt.float32, tag=f"tig{tag}")
        nc.gpsimd.tensor_tensor(t_ig[:], ifo[:, 0, :], ifog[:, 3, :], op=ALU.mult)
        nc.gpsimd.tensor_tensor(c_ap, t_fc[:], t_ig[:], op=ALU.add)
        tc2 = sbw.tile([128, WB], dt.float32, tag=f"tc2{tag}")
        nc.scalar.activation(tc2[:], c_ap, AFT.Tanh)
        h = sbw.tile([128, WB], dt.bfloat16, tag=f"h{tag}")
        nc.gpsimd.tensor_tensor(h[:], ifo[:, 2, :], tc2[:], op=ALU.mult)
        return h

    def p4_mm(m, m0, mr):
        lg = p4c.tile([128, VPC], dt.bfloat16, tag="lgits", bufs=1)
        for vc in range(VC_N):
            v0 = vc * VC_W
            ps = p4ps.tile([128, 256], dt.float32, tag="p4p")
            nc.tensor.matmul(
                ps[:mr, :VC_W], ones_bf[0:1, :mr], genb_sb[0:1, v0:v0 + VC_W],
                start=True, stop=False,
            )
            for kp in range(KP):
                nc.tensor.matmul(
                    ps[:mr, :VC_W],
                    hcat[:, kp, :, m0:m0 + mr],
                    genw_sb[:, kp, :, v0:v0 + VC_W],
                    start=False, stop=(kp == KP - 1),
                    perf_mode=PM.DoubleRow,
                )
            nc.vector.tensor_copy(lg[:mr, v0:v0 + VC_W], ps[:mr, :VC_W])
        return lg

    def p4_tail(m, m0, mr, lg):
        for ec in range(4):
            e0 = ec * 1000
            tmp = p4c.tile([128, 1000], dt.bfloat16, tag="etmp", bufs=2)
            nc.scalar.activation(
                tmp[:mr], lg[:mr, e0:e0 + 1000], AFT.Exp,
                accum_out=sume8[:mr, ec:ec + 1],
            )
        nc.vector.tensor_reduce(
            sume[:mr, m:m + 1], sume8[:mr, 0:4], axis=mybir.AxisListType.X,
            op=ALU.add,
        )
        bar_in = dram.tile([128, 1], dt.float32, tag="bar_in", bufs=2)
        nc.gpsimd.dma_start(bar_in[:mr], sume[:mr, m:m + 1])
        bar_out = dram.tile([128, 1], dt.float32, tag="bar_out", bufs=2,
                            addr_space=SHARED)
        _allreduce(nc, bar_in[:], bar_out[:])
        sg = p4c.tile([128, 1], dt.float32, tag="sg", bufs=2)
        nc.gpsimd.dma_start(sg[:], bar_out[:])
        lse = p4c.tile([128, 1], dt.float32, tag="lse", bufs=2)
        nc.scalar.activation(lse[:mr], sg[:mr], AFT.Ln)
        for oc in range(OC_N):
            o0 = oc * OC_W
            lpo = p4c.tile([128, OC_W], dt.float32, tag="lpo", bufs=2)
            nc.vector.tensor_scalar(
                lpo[:mr], lg[:mr, o0:o0 + OC_W], lse[:mr, 0:1], None, ALU.subtract
            )
            nc.gpsimd.dma_start(logp.ap()[m0:m0 + mr, o0:o0 + OC_W], lpo[:mr])

    lg_pend = {}
    for s in range(S_EFF):
        h1t_prev = h1t_pp[s % 2]
        h0t_prev = h0t_pp[s % 2]
        h1t_next = h1t_pp[(s + 1) % 2]
        h0t_next = h0t_pp[(s + 1) % 2]

        for w in range(NW):
            wc = slice(w * WB, (w + 1) * WB)
            # DMA issue queue for this wave's exchange chains: SP for wave 0,
            # GpSimd (SWDGE) for wave 1 — avoids cross-chain head-of-line
            # blocking on one sequencer.
            dq = nc.sync if w % 2 == 0 else nc.gpsimd

            # --- hw = W1 h1 + attn_b for this wave's 16 cols, then pick own
            #     2 cols via the per-core sel matrix (SPMD-safe selection) ---
            ps_hw = psL.tile([128, KH, WB], dt.float32, tag="ps_hwx", bufs=2)
            for m in range(KH):
                nc.tensor.matmul(
                    ps_hw[:, m, :],
                    attn_b_sb[0:1, m * 128:(m + 1) * 128],
                    ones_bf[0:1, 0:WB],
                    start=True, stop=False,
                )
                for k in range(KH):
                    nc.tensor.matmul(
                        ps_hw[:, m, :],
                        w1t_sb[:, k, m * 128:(m + 1) * 128],
                        h1t_prev[:, k, w],
                        start=False, stop=(k == KH - 1),
                    )
            hwf = sbw.tile([128, KH, WB], dt.bfloat16, tag="hwf", bufs=2)
            nc.vector.tensor_copy(hwf[:], ps_hw[:])
            ps_t = psL.tile([WB, KH, 128], dt.bfloat16, tag="ps_hwx", bufs=2)
            for m in range(KH):
                nc.tensor.transpose(ps_t[:WB, m, :], hwf[:, m, :], id_bf[:, :])
            hwT = sbw.tile([WB, KH, 128], dt.bfloat16, tag="hwT", bufs=1)
            nc.vector.tensor_copy(hwT[:], ps_t[:WB, :, :])
            ps_own = psL.tile([128, KH, NBW], dt.float32, tag="ps_hwx", bufs=2)
            for m in range(KH):
                nc.tensor.matmul(
                    ps_own[:, m, :], hwT[:WB, m, :], sel_own_sb[:],
                    start=True, stop=True,
                )
            hwt = sbw.tile([128, KH, NBW], dt.float32, tag="hwt", bufs=2)
            nc.vector.tensor_copy(hwt[:], ps_own[:])
            # --- attention for wave's 2 local batches ---
            ps_sc = psL.tile([128, KT, NBW], dt.float32, tag="ps_hwx", bufs=2)
            for i in range(NBW):
                lb = NBW * w + i
                for half in range(2):
                    k0 = half * 4
                    en = sbw.tile([128, 4, T], dt.bfloat16, tag="en", bufs=5)
                    for kk in range(4):
                        nc.vector.tensor_scalar(
                            en[:, kk, :], encw2[:, lb, k0 + kk, :],
                            hwt[:, k0 + kk, i:i + 1], None, ALU.add,
                        )
                    nc.scalar.activation(en[:], en[:], AFT.Tanh)
                    for tk in range(KT):
                        for kk in range(4):
                            nc.tensor.matmul(
                                ps_sc[:, tk, i:i + 1],
                                en[:, kk, tk * 128:(tk + 1) * 128],
                                vvec_sb[:, k0 + kk, :],
                                start=(k0 + kk == 0), stop=(k0 + kk == KH - 1),
                            )
            # --- softmax (unnormalized weights + reciprocal for ctx scale) ---
            exps = sbw.tile([128, KT, NBW], dt.bfloat16, tag="exps", bufs=2)
            nc.scalar.activation(exps[:], ps_sc[:], AFT.Exp)
            ps_den = psL.tile([128, NBW], dt.float32, tag="ps_small", bufs=1)
            for tk in range(KT):
                nc.tensor.matmul(
                    ps_den[0:1, :], ones_col[:, :], exps[:, tk, :],
                    start=(tk == 0), stop=(tk == KT - 1),
                )
            rec = sbw.tile([1, NBW], dt.float32, tag="rec", bufs=2)
            nc.vector.reciprocal(rec[:], ps_den[0:1, :])
            ps_rcb = psL.tile([128, NBW], dt.float32, tag="ps_small", bufs=1)
            nc.tensor.matmul(
                ps_rcb[:, :], ones_f32[:, :], rec[0:1, :], start=True, stop=True
            )
            recb = sbw.tile([128, NBW], dt.float32, tag="recb", bufs=2)
            nc.vector.tensor_copy(recb[:], ps_rcb[:])
            # --- context (feature-major, scaled by 1/den on copy) ---
            ctxw = sbw.tile([128, KH, NBW], dt.bfloat16, tag="ctxw", bufs=2)
            for i in range(NBW):
                lb = NBW * w + i
                ps_cx = psL.tile([128, KH], dt.float32, tag="ps_cx", bufs=1)
                for hk in range(KH):
                    for tk in range(KT):
                        nc.tensor.matmul(
                            ps_cx[:, hk:hk + 1],
                            enc_f8_sb[:, lb, tk, hk * 128:(hk + 1) * 128],
                            exps[:, tk, i:i + 1],
                            start=(tk == 0), stop=(tk == KT - 1),
                        )
                nc.vector.tensor_scalar(
                    ctxw[:, :, i], ps_cx[:], recb[:, i:i + 1], None, ALU.mult
                )
            # --- exchange ctx (AllGather over cores) ---
            bx_in = dram.tile([128, KH * NBW], dt.bfloat16, tag="bx_in", bufs=3)
            bx_out = dram.tile([NCORES * 128, KH * NBW], dt.bfloat16, tag="bx_out",
                               bufs=3, addr_space=SHARED)
            xt = sbw.tile([128, NCORES, KH, NBW], dt.bfloat16, tag="xt", bufs=2)
            _exchange(
                nc, dq, ctxw[:].rearrange("p k i -> p (k i)"),
                xt[:].rearrange("p c k i -> p c (k i)"), bx_in, bx_out,
                xt[:],
                bx_out[:].rearrange("(c p) (k i) -> p c k i", p=128, k=KH),
            )
            # --- LSTM layer 0 gates (N=16) ---
            ps_g0 = psL.tile([128, 4, WB], dt.float32, tag="ps_g", bufs=3)
            for gt in range(4):
                gsl = slice(gt * 128, (gt + 1) * 128)
                nc.tensor.matmul(
                    ps_g0[:, gt, :], id_bf[:],
                    g_emb[:, gt, s * B + w * WB:s * B + (w + 1) * WB],
                    start=True, stop=False,
                )
                for k in range(KH):
                    nc.tensor.matmul(
                        ps_g0[:, gt, :], whh0_sb[:, k, gsl],
                        h0t_prev[:, k, w], start=False, stop=False,
                    )
                for k in range(KH):
                    nc.tensor.matmul(
                        ps_g0[:, gt, :], wih0c_sb[:, k, gsl],
                        xt[:, :, k, :], start=False, stop=(k == KH - 1),
                    )
            h0n = cell(ps_g0, c_l0[:, wc], "l0")
            bh0_in = dram.tile([128, WB], dt.bfloat16, tag="bh0_in", bufs=3)
            bh0_out = dram.tile([NCORES * 128, WB], dt.bfloat16, tag="bh0_out",
                                bufs=3, addr_space=SHARED)
            _exchange(
                nc, dq, h0n[:],
                h0t_next[:, :, w].rearrange("p g c i -> p g (c i)"),
                bh0_in, bh0_out,
                h0t_next[:, :, w],
                bh0_out[:].rearrange("(g p) (c i) -> p g c i", p=128, c=NCORES),
            )
            # --- LSTM layer 1 gates ---
            ps_g1 = psL.tile([128, 4, WB], dt.float32, tag="ps_g", bufs=3)
            for gt in range(4):
                gsl = slice(gt * 128, (gt + 1) * 128)
                nc.tensor.matmul(
                    ps_g1[:, gt, :], bias_g1_sb[0:1, gsl], ones_bf[0:1, :WB],
                    start=True, stop=False,
                )
                for k in range(KH):
                    nc.tensor.matmul(
                        ps_g1[:, gt, :], whh1_sb[:, k, gsl],
                        h1t_prev[:, k, w], start=False, stop=False,
                    )
                for k in range(KH):
                    nc.tensor.matmul(
                        ps_g1[:, gt, :], wih1_sb[:, k, gsl],
                        h0t_next[:, k, w], start=False, stop=(k == KH - 1),
                    )
            h1n = cell(ps_g1, c_l1[:, wc], "l1")
            bh1_in = dram.tile([128, WB], dt.bfloat16, tag="bh1_in", bufs=3)
            bh1_out = dram.tile([NCORES * 128, WB], dt.bfloat16, tag="bh1_out",
                                bufs=3, addr_space=SHARED)
            _exchange(
                nc, dq, h1n[:],
                h1t_next[:, :, w].rearrange("p g c i -> p g (c i)"),
                bh1_in, bh1_out,
                h1t_next[:, :, w],
                bh1_out[:].rearrange("(g p) (c i) -> p g c i", p=128, c=NCORES),
            )
        # --- h1 history for the vocab projection (fp8, k-pair layout) ---
        for k in range(KH):
            nc.vector.tensor_copy(
                hcat[:, k // 2, k % 2, s * B:(s + 1) * B],
                h1t_next[:, k].rearrange("p w c i -> p (w c i)"),
            )
        # --- interleaved vocab projection bursts (matmuls at step 4m+3,
        #     exp/lse/subtract/output tail one step later to spread load) ---
        if s % 4 == 0 and s > 0 and (s // 4 - 1) in lg_pend:
            m = s // 4 - 1
            p4_tail(m, MTILES[m][0], MTILES[m][1], lg_pend.pop(m))
        if (s + 1) % 4 == 0:
            m = (s + 1) // 4 - 1
            lg_pend[m] = p4_mm(m, MTILES[m][0], MTILES[m][1])
        if s == S_EFF - 1:
            for m in sorted(lg_pend):
                p4_tail(m, MTILES[m][0], MTILES[m][1], lg_pend.pop(m))
            if (s + 1) * B % 128 != 0:
                m = ((s + 1) * B) // 128
                lg2 = p4_mm(m, m * 128, (s + 1) * B - m * 128)
                p4_tail(m, m * 128, (s + 1) * B - m * 128, lg2)

    # close loop pools
    p4c_cm.__exit__(None, None, None)
    p4ps_cm.__exit__(None, None, None)
    psL_cm.__exit__(None, None, None)
    sbw_cm.__exit__(None, None, None)
    loopers_cm.__exit__(None, None, None)
    dram_cm.__exit__(None, None, None)
    glob_cm.__exit__(None, None, None)


def _prep_inputs(inputs):
    """Host-side sharding/layout prep. Returns list of per-core input dicts."""
    f32 = np.float32
    enc_out = np.asarray(inputs["enc_out"], f32)
    enc_h = np.asarray(inputs["enc_h"], f32)
    enc_c = np.asarray(inputs["enc_c"], f32)
    emb = np.asarray(inputs["embedding"], f32)
    attn_W = np.asarray(inputs["attn_W"], f32)
    attn_b = np.asarray(inputs["attn_b"], f32)
    vv = np.asarray(inputs["v"], f32)
    Wih0 = np.asarray(inputs["Wih0"], f32)
    Whh0 = np.asarray(inputs["Whh0"], f32)
    bih0 = np.asarray(inputs["bih0"], f32)
    bhh0 = np.asarray(inputs["bhh0"], f32)
    Wih1 = np.asarray(inputs["Wih1"], f32)
    Whh1 = np.asarray(inputs["Whh1"], f32)
    bih1 = np.asarray(inputs["bih1"], f32)
    bhh1 = np.asarray(inputs["bhh1"], f32)
    genW = np.asarray(inputs["genW"], f32)
    genb = np.asarray(inputs["genb"], f32)
    q = np.asarray(inputs["question"]).astype(np.int64)

    W1 = attn_W[:, :H]
    W2 = attn_W[:, H:]
    h0 = np.concatenate([enc_h[0], enc_h[1]], 1)  # (B, H) layer 0
    h1 = np.concatenate([enc_h[2], enc_h[3]], 1)  # layer 1
    c0 = np.concatenate([enc_c[0], enc_c[1]], 1)
    c1 = np.concatenate([enc_c[2], enc_c[3]], 1)

    # step-column order: col = w*16 + c2*2 + i  <->  global batch 4*c2 + 2*w + i
    col2gb = np.array(
        [4 * c2 + NBW * w + i for w in range(NW) for c2 in range(NCORES)
         for i in range(NBW)], dtype=np.int64)
    qperm = q[col2gb, :].T.reshape(NSAMP, 1).astype(np.int32)  # (s, col) order

    def bf(x):
        return np.ascontiguousarray(x).astype(BF)

    def f8(x):
        return np.ascontiguousarray(x).astype(F8)

    shared = {
        "w2t": bf(W2.T),
        "w1t": bf(W1.T),
        "attn_bias": bf(attn_b.reshape(1, H)),
        "vvec": bf(vv.reshape(H, 1)),
        "emb_tab": np.ascontiguousarray(emb),
        "qidx": qperm,
        "h0t_init": bf(h0[col2gb].T),
        "h1t_init": bf(h1[col2gb].T),
    }
    maps = []
    for c in range(NCORES):
        # local batches (in (w, i) order) = global ids for this core's slots
        my_gb = [4 * c + NBW * w + i for w in range(NW) for i in range(NBW)]
        sel_own = np.zeros((WB, NBW), f32)
        for i in range(NBW):
            sel_own[c * NBW + i, i] = 1.0
        # gate rows: order i|f|o|g (torch order is i,f,g,o -> pick blocks 0,1,3,2)
        gorder = [0, 1, 3, 2]
        rows = np.concatenate(
            [np.arange(g * H + c * GS, g * H + (c + 1) * GS) for g in gorder]
        )
        gsc = np.repeat([0.5, 0.5, 0.5, 1.0], GS)[:, None].astype(f32)
        wih0_s = Wih0[rows] * gsc  # (NG, E+H); i/f/o halved for 1-tanh cells
        wih0e = np.zeros((EP, NG), f32)
        wih0e[:E] = wih0_s[:, :E].T
        bias0 = (bih0 + bhh0)[rows] * gsc[:, 0]  # (NG,)
        bias_g0c = bias0.reshape(4, GS).T  # (GS, 4)
        vrows = slice(c * VPC, (c + 1) * VPC)
        genw_t = genW[vrows].T  # (H, VPC)
        genw_kp = np.ascontiguousarray(
            genw_t.reshape(KP, 2, 128, VPC).transpose(2, 0, 1, 3)
        )
        m = dict(shared)
        m.update({
            "enc_tr": bf(enc_out[my_gb].transpose(0, 2, 1)),
            "enc_f8": f8(enc_out[my_gb]),
            "wih0e": bf(wih0e),
            "sel_own": bf(sel_own),
            "bias_g0c": np.ascontiguousarray(bias_g0c),
            "wih0c": bf(wih0_s[:, E:].T),
            "whh0": bf((Whh0[rows] * gsc).T),
            "wih1": bf((Wih1[rows] * gsc).T),
            "whh1": bf((Whh1[rows] * gsc).T),
            "bias_g1": bf(((bih1 + bhh1)[rows] * gsc[:, 0]).reshape(1, NG)),
            "c0_l0": np.ascontiguousarray(c0[col2gb, c * GS:(c + 1) * GS].T),
            "c0_l1": np.ascontiguousarray(c1[col2gb, c * GS:(c + 1) * GS].T),
            "genw_kp": f8(genw_kp),
            "genb_v": bf(genb[vrows].reshape(1, VPC)),
        })
        maps.append(m)
    return maps


_CACHED = {}


def _get_compiled():
    if "nc" not in _CACHED:
        nc = bacc.Bacc(
            "TRN2", target_bir_lowering=False, debug=False,
            num_devices=1 if SIM1 else NCORES,
        )
        build(nc)
        nc.compile()
        _CACHED["nc"] = nc
    return _CACHED["nc"]


def run_cores(in_maps, **kw):
    nc = _get_compiled()
    return bass_utils.run_bass_kernel_spmd(nc, in_maps, list(range(NCORES)), **kw)


def kernel(**inputs):
    in_maps = _prep_inputs(inputs)
    res = run_cores(in_maps)
    parts = [res.results[c]["logp"] for c in range(NCORES)]
    full = np.concatenate(parts, axis=1)  # (NSAMP, V) in (s, col) order
    col2gb = np.array(
        [4 * c2 + NBW * w + i for w in range(NW) for c2 in range(NCORES)
         for i in range(NBW)], dtype=np.int64)
    full = full.reshape(S, B, V)
    out = np.empty((B, S, V), np.float32)
    out[col2gb, :, :] = full.transpose(1, 0, 2)
    return np.ascontiguousarray(out)
